# revision 25
# baseline (speedup 1.0000x reference)
"""BitNet DiT on 8 Trainium2 NeuronCores — data-parallel over batch (2 images/core).

Host: patchify, time-embedding + adaLN modulation vectors, BitNet weight
quantization (ternary * per-tensor scale) -> bf16 upload.
Device: full 12-block DiT forward per core in a single Bass/Tile kernel.
BitNet matmuls run as exact integer arithmetic in bf16 (|values| <= 127,
fp32 accumulate). Attention runs in fp32r via transposed-logits + ones-column
softmax-denominator trick.

v2: qT/kT produced directly by weight-side matmuls (no activation
transposes for attention), pipelined attention heads, packed scalar
chains, Sqrt-based rstd (no act-table thrash), Pool-engine offload,
batched quantize-transpose packs, fp32r patch/head matmuls.
"""
import math
import os
import sys
import numpy as np

sys.path.insert(0, "/opt/trn_rl_repo")

import ml_dtypes  # noqa: E402
import concourse.bass as bass  # noqa: E402
import concourse.mybir as mybir  # noqa: E402
import concourse.tile as tile  # noqa: E402
from concourse import bacc  # noqa: E402
from concourse.bass_utils import run_bass_kernel_spmd  # noqa: E402
from concourse.masks import make_identity  # noqa: E402

F32 = mybir.dt.float32
F32R = mybir.dt.float32r
BF16 = mybir.dt.bfloat16
AX = mybir.AxisListType
OP = mybir.AluOpType
AF = mybir.ActivationFunctionType

DIM = 768
DEPTH = int(os.environ.get("KERNEL_DEPTH", "12"))
HEADS = 12
HD = 64
PATCH = 16
IMG = 256
CIN = 3
HID = 4 * DIM
EPS = 1e-6
P = 128
T = 512            # tokens per core (2 images x 256)
NT = T // P        # 4 token tiles
NTOK = 256         # tokens per image
KD = DIM // P      # 6
KH = HID // P      # 24
MAGIC = float(np.float32(3 * 2**22))  # 12582912.0 RNE round-to-int magic

_CACHED = {}


def _mm_chunks(n, c=512):
    out = []
    s = 0
    while s < n:
        e = min(s + c, n)
        out.append((s, e))
        s = e
    return out


def build_program(depth=DEPTH):
    nc = bacc.Bacc("TRN2", target_bir_lowering=False, debug=False, num_devices=8)

    xpT_d = nc.declare_dram_parameter("xpT", [DIM, T], F32R, isOutput=False)
    posb_d = nc.declare_dram_parameter("posb", [NTOK, DIM], F32, isOutput=False)
    patchWT_d = nc.declare_dram_parameter("patchWT", [DIM, DIM], F32R, isOutput=False)
    headWT_d = nc.declare_dram_parameter("headWT", [DIM, DIM], F32R, isOutput=False)
    headb_d = nc.declare_dram_parameter("headb", [1, DIM], F32, isOutput=False)
    wqkv_d = nc.declare_dram_parameter("wqkv", [depth, DIM, 3 * DIM], BF16, isOutput=False)
    wproj_d = nc.declare_dram_parameter("wproj", [depth, DIM, DIM], BF16, isOutput=False)
    wfc1_d = nc.declare_dram_parameter("wfc1", [depth, DIM, HID], BF16, isOutput=False)
    wfc2_d = nc.declare_dram_parameter("wfc2", [depth, HID, DIM], BF16, isOutput=False)
    # modulation vectors: [block, norm(2), img(2), A/B(2), 768] host-broadcast to 128 parts
    mods_d = nc.declare_dram_parameter("mods", [depth, 2, P, 2, 2, DIM], F32, isOutput=False)
    wscl_d = nc.declare_dram_parameter("wscl", [1, 4 * depth], F32, isOutput=False)
    out_d = nc.declare_dram_parameter("zout", [T, DIM], F32, isOutput=True)

    with tile.TileContext(nc) as tc:
        from contextlib import ExitStack
        with ExitStack() as _ctx:
            constp = _ctx.enter_context(tc.tile_pool(name="const", bufs=1))
            residp = _ctx.enter_context(tc.tile_pool(name="resid", bufs=1))
            fm6p = _ctx.enter_context(tc.tile_pool(name="fm6", bufs=2))
            xqTp = _ctx.enter_context(tc.tile_pool(name="xqT", bufs=1))
            wp = _ctx.enter_context(tc.tile_pool(name="w", bufs=3))
            modp = _ctx.enter_context(tc.tile_pool(name="mod", bufs=1))
            tmp_ = _ctx.enter_context(tc.tile_pool(name="tm", bufs=3))
            gp = _ctx.enter_context(tc.tile_pool(name="g", bufs=3))
            qtp = _ctx.enter_context(tc.tile_pool(name="qt", bufs=1))
            xqp = _ctx.enter_context(tc.tile_pool(name="xq", bufs=1))
            eTp = _ctx.enter_context(tc.tile_pool(name="eT", bufs=2))
            scp = _ctx.enter_context(tc.tile_pool(name="sc", bufs=48))
            cbp = _ctx.enter_context(tc.tile_pool(name="cb", bufs=1))
            ps_mm = _ctx.enter_context(tc.tile_pool(name="ps_mm", bufs=2, space="PSUM"))
            ps_tp = _ctx.enter_context(tc.tile_pool(name="ps_tp", bufs=2, space="PSUM"))
            ps_lt = _ctx.enter_context(tc.tile_pool(name="ps_lt", bufs=2, space="PSUM"))
            ps_oa = _ctx.enter_context(tc.tile_pool(name="ps_oa", bufs=2, space="PSUM"))

            idf = constp.tile([P, P], F32)
            make_identity(nc, idf[:])
            idb = constp.tile([P, P], BF16)
            nc.vector.tensor_copy(idb[:], idf[:])
            idr = constp.tile([P, P], F32R)
            nc.vector.tensor_copy(idr[:], idf[:])

            # broadcast w_scales to all partitions
            wsrow = constp.tile([1, 4 * depth], F32)
            nc.sync.dma_start(wsrow[:], wscl_d[:])
            wsb = constp.tile([P, 4 * depth], F32)
            nc.gpsimd.partition_broadcast(wsb[:], wsrow[0:1, :])

            z = residp.tile([P, NT, DIM], F32)
            v_aug = residp.tile([P, NT, HEADS, HD + 1], F32)
            nc.vector.memset(v_aug[:, :, :, HD], 1.0)

            # ---------------- patch embed (fp32r matmuls) ----------------
            posb_sb = wp.tile([P, 2, DIM], F32, tag="w")
            nc.sync.dma_start(posb_sb[:], posb_d.rearrange("(a p) d -> p a d", p=P))
            xpT = fm6p.tile([P, KD, T], F32R, tag="fm6")
            nc.sync.dma_start(xpT[:], xpT_d.rearrange("(o p) t -> p o t", p=P))
            pw = wp.tile([P, KD, DIM], F32R, tag="w")
            nc.sync.dma_start(pw[:], patchWT_d.rearrange("(o p) d -> p o d", p=P))
            for t in range(NT):
                for (cs, ce) in _mm_chunks(DIM):
                    pt = ps_mm.tile([P, 512], F32, tag="mm", name="pmm")[:, : ce - cs]
                    for k in range(KD):
                        nc.tensor.matmul(pt[:], xpT[:, k, t * P:(t + 1) * P],
                                         pw[:, k, cs:ce], start=(k == 0), stop=(k == KD - 1))
                    nc.vector.tensor_tensor(z[:, t, cs:ce], pt[:], posb_sb[:, t % 2, cs:ce], OP.add)

            def load_w(dram, b, kchunks, width, dtype=BF16):
                half = kchunks // 2
                tiles = []
                for i in range(2):
                    wt = wp.tile([P, half, width], dtype, tag="w")
                    nc.sync.dma_start(
                        wt[:],
                        dram[b, i * half * P:(i + 1) * half * P, :].rearrange(
                            "(o p) f -> p o f", p=P))
                    tiles.append(wt)
                return tiles, half

            xqT = xqTp.tile([P, KH, T], BF16, tag="xqT")

            def amax_of(src_ap, dst_slice, eng=None):
                (eng or nc.vector).tensor_reduce(dst_slice, src_ap, axis=AX.X, op=OP.max,
                                                 apply_absolute_value=True)

            def scales_of(amax_pack, ws_idx, n):
                """[P,n] packed: s127 = 127/clip(amax,1e-5); c = clip*ws."""
                acs = scp.tile([P, 8], F32, tag="sc", name="acs")[:, :n]
                nc.vector.tensor_scalar_max(acs[:], amax_pack, 1e-5)
                rs = scp.tile([P, 8], F32, tag="sc", name="rs")[:, :n]
                nc.vector.reciprocal(rs[:], acs[:])
                s127 = scp.tile([P, 8], F32, tag="sc", name="s127")[:, :n]
                nc.vector.tensor_scalar_mul(s127[:], rs[:], 127.0)
                cs = scp.tile([P, 8], F32, tag="sc", name="cs")[:, :n]
                nc.vector.tensor_scalar(cs[:], acs[:], wsb[:, ws_idx:ws_idx + 1],
                                        None, OP.mult)
                return s127, cs

            def quant_data(src_ap, t, kchunks, s127_slice):
                """round(src*s127) -> bf16 ints, transposed into xqT cols t."""
                xq = xqp.tile([P, HID], BF16, tag="xq", name="xq")[:, :kchunks * P]
                for g0 in range(0, kchunks, 12):
                    gn = min(12, kchunks - g0)
                    tmp = qtp.tile([P, 12 * P], F32, tag="qt", name="qtmp")[:, :gn * P]
                    nc.gpsimd.tensor_scalar(tmp[:], src_ap[:, g0 * P:(g0 + gn) * P],
                                            s127_slice, MAGIC, OP.mult, OP.add)
                    nc.vector.tensor_scalar(xq[:, g0 * P:(g0 + gn) * P], tmp[:],
                                            MAGIC, None, OP.subtract)
                k0 = 0
                while k0 < kchunks:
                    g = min(8, kchunks - k0)
                    pack = ps_tp.tile([P, 8, P], BF16, tag="tp", name="tpack")
                    for j in range(g):
                        nc.tensor.transpose(pack[:, j, :], xq[:, (k0 + j) * P:(k0 + j + 1) * P],
                                            idb[:])
                    nc.vector.tensor_copy(xqT[:, k0:k0 + g, t * P:(t + 1) * P],
                                          pack[:, :g, :])
                    k0 += g

            def rstd_of(ssq_pack, n):
                """rstd = 1/sqrt(ssq/DIM + EPS) packed [P,n]."""
                ms = scp.tile([P, 8], F32, tag="sc", name="ms")[:, :n]
                nc.vector.tensor_scalar(ms[:], ssq_pack, 1.0 / DIM, EPS, OP.mult, OP.add)
                rr = scp.tile([P, 8], F32, tag="sc", name="rr")[:, :n]
                nc.vector.reciprocal(rr[:], ms[:])
                rstd = scp.tile([P, 8], F32, tag="sc", name="rstd")[:, :n]
                nc.scalar.activation(rstd[:], rr[:], AF.Sqrt)
                return rstd

            def norm_mod(t, mt, rstd_slice, dst):
                img = t // 2
                nc.vector.scalar_tensor_tensor(dst, z[:, t, :], rstd_slice,
                                               mt[:, img, 0, :], OP.mult, OP.mult)
                nc.vector.tensor_tensor(dst, dst, mt[:, img, 1, :], OP.add)

            def make_cB(crow, cs_pack, j0, n):
                """cs [P,n] (token-partitions) -> crow row segs j0.. (partition 0)."""
                cT = ps_tp.tile([P, NT, P], F32, tag="tp", name="cT")
                for j in range(n):
                    nc.tensor.transpose(cT[0:1, j, :], cs_pack[:, j:j + 1], idf[:])
                nc.vector.tensor_copy(crow[0:1, j0:j0 + n, :], cT[0:1, 0:n, :])

            def bcast_cB(cB, crow):
                for j in range(NT):
                    nc.gpsimd.partition_broadcast(cB[:, j * P:(j + 1) * P], crow[0:1, j, :])

            def ssq_of(src_ap, dst_slice):
                sq = tmp_.tile([P, DIM], F32, tag="tm", name="sqscratch")
                nc.scalar.activation(sq[:], src_ap, AF.Square, accum_out=dst_slice)

            # ---- prologue: norm1+quant of block 0 ----
            mt1 = modp.tile([P, 2, 2, DIM], F32, tag="mod", name="mt1")
            nc.sync.dma_start(mt1[:], mods_d[0, 0])
            ssq0 = scp.tile([P, 8], F32, tag="sc", name="ssq0")[:, :NT]
            for t in range(NT):
                ssq_of(z[:, t, :], ssq0[:, t:t + 1])
            rstd0 = rstd_of(ssq0[:], NT)
            h_nxt = gp.tile([P, NT, DIM], F32, tag="g", name="h_nxt")
            for t in range(NT):
                norm_mod(t, mt1, rstd0[:, t:t + 1], h_nxt[:, t, :])
            amax0 = scp.tile([P, 8], F32, tag="sc", name="amax0")[:, :NT]
            for t in range(NT):
                amax_of(h_nxt[:, t, :], amax0[:, t:t + 1])
            s127n, csn = scales_of(amax0[:], 0, NT)
            crow1 = cbp.tile([1, NT, P], F32, tag="crow", name="crow1")
            make_cB(crow1, csn[:], 0, NT)
            cB1 = cbp.tile([P, T], F32, tag="cb", name="cB1")
            bcast_cB(cB1, crow1)
            for t in range(NT):
                quant_data(h_nxt[:, t, :], t, KD, s127n[:, t:t + 1])
            csn_sl = [csn[:, t:t + 1] for t in range(NT)]
            wq_tiles, wq_half = load_w(wqkv_d, 0, KD, 3 * DIM)

            for b in range(depth):
                import contextlib
                def sc_(nm):
                    return nc.named_scope(f"b{b}_{nm}") if b == 5 else contextlib.nullcontext()
                mt2 = modp.tile([P, 2, 2, DIM], F32, tag="mod", name="mt2")
                nc.sync.dma_start(mt2[:], mods_d[b, 1])

                _p2 = sc_("p2qkv"); _p2.__enter__()
                # --- v (token layout) ---
                for t in range(NT):
                    for (cs0, ce0) in _mm_chunks(DIM):
                        pt = ps_mm.tile([P, 512], F32, tag="mm", name="pmm")[:, : ce0 - cs0]
                        for k in range(KD):
                            wt = wq_tiles[k // wq_half]
                            nc.tensor.matmul(pt[:], xqT[:, k, t * P:(t + 1) * P],
                                             wt[:, k % wq_half, 2 * DIM + cs0:2 * DIM + ce0],
                                             start=(k == 0), stop=(k == KD - 1))
                        h0 = cs0 // HD
                        h1 = ce0 // HD
                        nc.scalar.activation(
                            v_aug[:, t, h0:h1, 0:HD], pt[:],
                            AF.Identity, scale=csn_sl[t])
                # --- qT / kT (feature-partition layout, no transposes) ---
                q_fm = fm6p.tile([P, KD, T], F32R, tag="fm6", name="q_fm")
                k_fm = fm6p.tile([P, KD, T], F32R, tag="fm6", name="k_fm")
                for which, fm, coff in ((0, q_fm, 0), (1, k_fm, DIM)):
                    for qc in range(KD):
                        pt = ps_mm.tile([P, 512], F32, tag="mm", name="pmm")
                        for k in range(KD):
                            wt = wq_tiles[k // wq_half]
                            nc.tensor.matmul(pt[:], wt[:, k % wq_half, coff + qc * P:coff + (qc + 1) * P],
                                             xqT[:, k, :], start=(k == 0), stop=(k == KD - 1))
                        nc.vector.tensor_tensor(fm[:, qc, :], pt[:], cB1[:], OP.mult)
                _p2.__exit__(None, None, None)

                # --- attention: pipelined heads; o-quant, proj and norm2
                #     chains overlapped into the head pipeline ---
                _p3 = sc_("p3attn"); _p3.__enter__()
                wp_tiles, wp_half = load_w(wproj_d, b, KD, DIM)
                o_tm = gp.tile([P, NT, DIM], F32, tag="g", name="o_tm")
                h_all2 = gp.tile([P, NT, DIM], F32, tag="g", name="h_all2")
                amax_o01 = scp.tile([P, 8], F32, tag="sc", name="amaxo01")[:, :2]
                amax_o23 = scp.tile([P, 8], F32, tag="sc", name="amaxo23")[:, :2]
                ssq2 = scp.tile([P, 8], F32, tag="sc", name="ssq2")[:, :NT]
                amax2 = scp.tile([P, 8], F32, tag="sc", name="amax2")[:, :NT]
                s127o = [None, None]
                cpso = [None, None]

                state = [None] * (2 * HEADS)

                def lt_of(i):
                    img, hh = divmod(i, HEADS)
                    po = (hh % 2) * HD
                    ch = hh // 2
                    lt = ps_lt.tile([P, 2, NTOK], F32, tag="lt", name="lt")
                    for mt in range(2):
                        nc.tensor.matmul(
                            lt[:, mt, :],
                            k_fm[po:po + HD, ch, img * NTOK + mt * P: img * NTOK + (mt + 1) * P],
                            q_fm[po:po + HD, ch, img * NTOK: (img + 1) * NTOK],
                            start=True, stop=True)
                    state[i] = lt

                def exp_of(i):
                    lt = state[i]
                    eT = eTp.tile([P, 2, NTOK], F32, tag="eT", name="eT")
                    nc.scalar.activation(eT[:], lt[:], AF.Exp, scale=0.125)
                    state[i] = eT

                def oa_of(i):
                    img, hh = divmod(i, HEADS)
                    eT = state[i]
                    for nt in range(2):
                        oa = ps_oa.tile([P, HD + 1], F32, tag="oa", name="oa")
                        for mt in range(2):
                            nc.tensor.matmul(
                                oa[:], eT[:, mt, nt * P:(nt + 1) * P],
                                v_aug[:, img * 2 + mt, hh, :],
                                start=(mt == 0), stop=(mt == 1))
                        rinv = scp.tile([P, 1], F32, tag="sc", name="rinv")
                        nc.vector.reciprocal(rinv[:], oa[:, HD:HD + 1])
                        nc.scalar.activation(
                            o_tm[:, img * 2 + nt, hh * HD:(hh + 1) * HD],
                            oa[:, 0:HD], AF.Identity, scale=rinv[:])
                    state[i] = None

                def oquant(t):
                    half = t // 2
                    quant_data(o_tm[:, t, :], t, KD, s127o[half][:, (t % 2):(t % 2) + 1])

                def proj_tile(t):
                    cps_sl = cpso[t // 2][:, (t % 2):(t % 2) + 1]
                    for (cs0, ce0) in _mm_chunks(DIM):
                        pt = ps_mm.tile([P, 512], F32, tag="mm", name="pmm")[:, : ce0 - cs0]
                        for k in range(KD):
                            wt = wp_tiles[k // wp_half]
                            nc.tensor.matmul(pt[:], xqT[:, k, t * P:(t + 1) * P],
                                             wt[:, k % wp_half, cs0:ce0],
                                             start=(k == 0), stop=(k == KD - 1))
                        nc.vector.scalar_tensor_tensor(z[:, t, cs0:ce0], pt[:], cps_sl,
                                                       z[:, t, cs0:ce0], OP.mult, OP.add)

                def n2chain(t):
                    ssq_of(z[:, t, :], ssq2[:, t:t + 1])
                    r = rstd_of(ssq2[:, t:t + 1], 1)
                    norm_mod(t, mt2, r[:, 0:1], h_all2[:, t, :])
                    amax_of(h_all2[:, t, :], amax2[:, t:t + 1])

                lt_of(0)
                for i in range(1, 2 * HEADS):
                    exp_of(i - 1)
                    lt_of(i)
                    oa_of(i - 1)
                    if i == 13:
                        amax_of(o_tm[:, 0, :], amax_o01[:, 0:1])
                        amax_of(o_tm[:, 1, :], amax_o01[:, 1:2])
                        s127o[0], cpso[0] = scales_of(amax_o01[:], 4 * b + 1, 2)
                        oquant(0)
                    if i == 15:
                        oquant(1)
                    if i == 17:
                        proj_tile(0)
                        n2chain(0)
                    if i == 20:
                        proj_tile(1)
                        n2chain(1)
                exp_of(2 * HEADS - 1)
                oa_of(2 * HEADS - 1)
                amax_of(o_tm[:, 2, :], amax_o23[:, 0:1])
                amax_of(o_tm[:, 3, :], amax_o23[:, 1:2])
                s127o[1], cpso[1] = scales_of(amax_o23[:], 4 * b + 1, 2)
                oquant(2)
                proj_tile(2)
                n2chain(2)
                oquant(3)
                proj_tile(3)
                s1272_01, c3_01 = scales_of(amax2[:, 0:2], 4 * b + 2, 2)
                quant_data(h_all2[:, 0, :], 0, KD, s1272_01[:, 0:1])
                quant_data(h_all2[:, 1, :], 1, KD, s1272_01[:, 1:2])
                n2chain(3)
                _p3.__exit__(None, None, None)

                # --- fc1/gelu + g-quant pipeline (n2 tail chains under cover) ---
                _p5 = sc_("p5fc1"); _p5.__enter__()
                wf1_tiles, wf1_half = load_w(wfc1_d, b, KD, HID)
                wf2_tiles, wf2_half = load_w(wfc2_d, b, KH, DIM)
                gs = [None] * NT
                c4g = [None] * NT
                s1272_23 = None
                c3_23 = None

                def gquant(t):
                    amax_g = scp.tile([P, 1], F32, tag="sc", name="amaxg")
                    amax_of(gs[t][:], amax_g[:])
                    s127g, cg = scales_of(amax_g[:], 4 * b + 3, 1)
                    quant_data(gs[t][:], t, KH, s127g[:, 0:1])
                    c4g[t] = cg

                for t in range(NT):
                    if t == 2:
                        quant_data(h_all2[:, 2, :], 2, KD, s1272_23[:, 0:1])
                    if t == 3:
                        quant_data(h_all2[:, 3, :], 3, KD, s1272_23[:, 1:2])
                    g = gp.tile([P, HID], F32, tag="g")
                    gs[t] = g
                    c3_sl = (c3_01[:, t:t + 1] if t < 2 else c3_23[:, t - 2:t - 1])
                    for (cs0, ce0) in _mm_chunks(HID):
                        pt = ps_mm.tile([P, 512], F32, tag="mm", name="pmm")[:, : ce0 - cs0]
                        for k in range(KD):
                            wt = wf1_tiles[k // wf1_half]
                            nc.tensor.matmul(pt[:], xqT[:, k, t * P:(t + 1) * P],
                                             wt[:, k % wf1_half, cs0:ce0],
                                             start=(k == 0), stop=(k == KD - 1))
                        nc.scalar.activation(g[:, cs0:ce0], pt[:], AF.Gelu_apprx_tanh,
                                             scale=c3_sl)
                    if t == 1:
                        s1272_23, c3_23 = scales_of(amax2[:, 2:4], 4 * b + 2, 2)
                    if t > 0:
                        gquant(t - 1)
                gquant(NT - 1)
                _p5.__exit__(None, None, None)

                # --- fc2 + residual, fused with next block's norm1 ---
                _p6 = sc_("p6fc2"); _p6.__enter__()
                fuse = b + 1 < depth
                if fuse:
                    mt1_nxt = modp.tile([P, 2, 2, DIM], F32, tag="mod", name="mt1n")
                    nc.sync.dma_start(mt1_nxt[:], mods_d[b + 1, 0])
                    h_nxt = gp.tile([P, NT, DIM], F32, tag="g", name="h_nxt")
                    ssqn = scp.tile([P, 8], F32, tag="sc", name="ssqn")[:, :NT]
                    amaxn = scp.tile([P, 8], F32, tag="sc", name="amaxn")[:, :NT]
                    s127n_h = [None, None]
                    csn_h = [None, None]

                    def p1chain(t):
                        ssq_of(z[:, t, :], ssqn[:, t:t + 1])
                        r = rstd_of(ssqn[:, t:t + 1], 1)
                        norm_mod(t, mt1_nxt, r[:, 0:1], h_nxt[:, t, :])
                        amax_of(h_nxt[:, t, :], amaxn[:, t:t + 1])

                for t in range(NT):
                    for (cs0, ce0) in _mm_chunks(DIM):
                        pt = ps_mm.tile([P, 512], F32, tag="mm", name="pmm")[:, : ce0 - cs0]
                        for k in range(KH):
                            wt = wf2_tiles[k // wf2_half]
                            nc.tensor.matmul(pt[:], xqT[:, k, t * P:(t + 1) * P],
                                             wt[:, k % wf2_half, cs0:ce0],
                                             start=(k == 0), stop=(k == KH - 1))
                        nc.vector.scalar_tensor_tensor(z[:, t, cs0:ce0], pt[:], c4g[t][:, 0:1],
                                                       z[:, t, cs0:ce0], OP.mult, OP.add)
                    if fuse:
                        p1chain(t)
                        if t == 1:
                            s127n_h[0], csn_h[0] = scales_of(amaxn[:, 0:2], 4 * (b + 1), 2)
                        if t == 2:
                            quant_data(h_nxt[:, 0, :], 0, KD, s127n_h[0][:, 0:1])
                        if t == 3:
                            quant_data(h_nxt[:, 1, :], 1, KD, s127n_h[0][:, 1:2])
                if fuse:
                    s127n_h[1], csn_h[1] = scales_of(amaxn[:, 2:4], 4 * (b + 1), 2)
                    crow1 = cbp.tile([1, NT, P], F32, tag="crow", name="crow1")
                    make_cB(crow1, csn_h[0][:], 0, 2)
                    make_cB(crow1, csn_h[1][:], 2, 2)
                    cB1 = cbp.tile([P, T], F32, tag="cb", name="cB1")
                    bcast_cB(cB1, crow1)
                    quant_data(h_nxt[:, 2, :], 2, KD, s127n_h[1][:, 0:1])
                    quant_data(h_nxt[:, 3, :], 3, KD, s127n_h[1][:, 1:2])
                    csn_sl = [csn_h[0][:, 0:1], csn_h[0][:, 1:2],
                              csn_h[1][:, 0:1], csn_h[1][:, 1:2]]
                    wq_tiles, wq_half = load_w(wqkv_d, b + 1, KD, 3 * DIM)
                _p6.__exit__(None, None, None)

            # ---------------- final norm + head (fp32r) ----------------
            hw = wp.tile([P, KD, DIM], F32R, tag="w")
            nc.sync.dma_start(hw[:], headWT_d.rearrange("(o p) d -> p o d", p=P))
            hbrow = tmp_.tile([1, DIM], F32, tag="tm", name="hbrow")
            nc.sync.dma_start(hbrow[:], headb_d[:])
            hbb = gp.tile([P, DIM], F32, tag="g", name="hbb")
            nc.gpsimd.partition_broadcast(hbb[:], hbrow[0:1, :])
            ssqf = scp.tile([P, 8], F32, tag="sc", name="ssqf")[:, :NT]
            for t in range(NT):
                ssq_of(z[:, t, :], ssqf[:, t:t + 1])
            rstdf = rstd_of(ssqf[:], NT)
            for t in range(NT):
                zn = tmp_.tile([P, DIM], F32R, tag="tm")
                nc.vector.tensor_scalar_mul(zn[:], z[:, t, :], rstdf[:, t:t + 1])
                znT = tmp_.tile([P, DIM], F32R, tag="tm")
                for k in range(KD):
                    ptf = ps_tp.tile([P, P], F32R, tag="tp", name="ptf")
                    nc.tensor.transpose(ptf[:], zn[:, k * P:(k + 1) * P], idr[:])
                    nc.vector.tensor_copy(znT[:, k * P:(k + 1) * P], ptf[:])
                for (cs0, ce0) in _mm_chunks(DIM):
                    pt = ps_mm.tile([P, 512], F32, tag="mm", name="pmm")[:, : ce0 - cs0]
                    for k in range(KD):
                        nc.tensor.matmul(pt[:], znT[:, k * P:(k + 1) * P],
                                         hw[:, k, cs0:ce0], start=(k == 0), stop=(k == KD - 1))
                    ot = tmp_.tile([P, DIM], F32, tag="tm", name="ot")[:, : ce0 - cs0]
                    nc.vector.tensor_tensor(ot[:], pt[:], hbb[:, cs0:ce0], OP.add)
                    nc.sync.dma_start(out_d[t * P:(t + 1) * P, cs0:ce0], ot[:])

    nc.compile()
    return nc


# ---------------------------------------------------------------------------
# host-side numerics (numpy, fp32 — matches jax CPU within ~1e-7)

def _gelu_tanh(x):
    x = x.astype(np.float32)
    c = np.float32(math.sqrt(2.0 / math.pi))
    return np.float32(0.5) * x * (np.float32(1.0) +
                                  np.tanh(c * (x + np.float32(0.044715) * x * x * x)))


def _time_embedding(t, t_w1, t_b1, t_w2, t_b2):
    half = DIM // 2
    freqs = np.exp(-np.log(10000.0) * np.arange(half, dtype=np.float32) / (half - 1)).astype(np.float32)
    args = t[:, None].astype(np.float32) * freqs[None, :]
    emb = np.concatenate([np.sin(args), np.cos(args)], axis=-1).astype(np.float32)
    h = _gelu_tanh(emb @ t_w1.T + t_b1)
    return (h @ t_w2.T + t_b2).astype(np.float32)


def _quant_w(w):
    ws = np.float32(np.mean(np.abs(w), dtype=np.float64)) + np.float32(1e-5)
    wq = np.clip(np.round(w.astype(np.float32) / ws), -1.0, 1.0)
    return wq, ws


def _prepare(inputs):
    x = np.asarray(inputs["x"], np.float32)
    t = np.asarray(inputs["t"], np.float32)
    B = x.shape[0]
    n_cores = 8
    per = B // n_cores  # 2
    p = PATCH
    hh = IMG // p

    xp = x.reshape(B, CIN, hh, p, hh, p).transpose(0, 2, 4, 1, 3, 5).reshape(B, hh * hh, CIN * p * p)

    t_emb = _time_embedding(t, inputs["t_w1"], inputs["t_b1"], inputs["t_w2"], inputs["t_b2"])
    silu = (t_emb / (1.0 + np.exp(-t_emb))).astype(np.float32)

    depth = DEPTH
    mods = np.zeros((depth, 2, B, 2, DIM), np.float32)  # [blk, norm, img, A/B, D]
    wscl = np.zeros((4 * depth,), np.float32)
    wq_all, wp_all, wf1_all, wf2_all = [], [], [], []
    for b in range(depth):
        mod = silu @ np.asarray(inputs["blk_ada_w"][b], np.float32).T + np.asarray(
            inputs["blk_ada_b"][b], np.float32)
        sh1, sc1, sh2, sc2 = np.split(mod, 4, axis=-1)
        n1 = np.asarray(inputs["blk_norm1"][b], np.float32)
        n2 = np.asarray(inputs["blk_norm2"][b], np.float32)
        mods[b, 0, :, 0, :] = n1[None, :] * (1.0 + sc1)
        mods[b, 0, :, 1, :] = sh1
        mods[b, 1, :, 0, :] = n2[None, :] * (1.0 + sc2)
        mods[b, 1, :, 1, :] = sh2

        for j, (nm, lst) in enumerate([("blk_qkv", wq_all), ("blk_proj", wp_all),
                                       ("blk_fc1", wf1_all), ("blk_fc2", wf2_all)]):
            wq, ws = _quant_w(np.asarray(inputs[nm][b], np.float32))
            lst.append(np.ascontiguousarray(wq.T).astype(ml_dtypes.bfloat16))
            wscl[4 * b + j] = ws / np.float32(127.0)

    wqkv = np.stack(wq_all)
    wproj = np.stack(wp_all)
    wfc1 = np.stack(wf1_all)
    wfc2 = np.stack(wf2_all)

    posb = (np.asarray(inputs["pos_embed"][0], np.float32) +
            np.asarray(inputs["patch_b"], np.float32)[None, :]).astype(np.float32)
    patchWT = np.ascontiguousarray(np.asarray(inputs["patch_w"], np.float32).T)
    norm_w = np.asarray(inputs["norm_w"], np.float32)
    headWT = np.ascontiguousarray(np.asarray(inputs["head_w"], np.float32).T * norm_w[:, None])
    headb = np.asarray(inputs["head_b"], np.float32)[None, :]

    key = ("prog", depth)
    if key not in _CACHED:
        _CACHED[key] = build_program(depth)
    nc = _CACHED[key]

    in_maps = []
    for c in range(n_cores):
        imgs = slice(c * per, (c + 1) * per)
        xpT = np.ascontiguousarray(xp[imgs].reshape(per * hh * hh, CIN * p * p).T)
        in_maps.append(dict(
            xpT=xpT, posb=posb, patchWT=patchWT, headWT=headWT, headb=headb,
            wqkv=wqkv, wproj=wproj, wfc1=wfc1, wfc2=wfc2,
            mods=np.ascontiguousarray(
                np.broadcast_to(mods[:, :, None, imgs], (depth, 2, 128, per, 2, DIM))),
            wscl=wscl[None, :],
        ))

    return nc, in_maps


def _assemble(res, B=16, per=2):
    p = PATCH
    hh = IMG // p
    out = np.zeros((B, CIN, IMG, IMG), np.float32)
    for c in range(B // per):
        zo = res.results[c]["zout"]  # [512, 768]
        for i in range(per):
            zi = zo[i * 256:(i + 1) * 256]
            out[c * per + i] = zi.reshape(hh, hh, CIN, p, p).transpose(2, 0, 3, 1, 4).reshape(CIN, IMG, IMG)
    return out


def kernel(**inputs):
    nc, in_maps = _prepare(inputs)
    res = run_bass_kernel_spmd(nc, in_maps, list(range(len(in_maps))), trace=False)
    return _assemble(res)


# revision 27
# speedup vs baseline: 1.1929x; 1.1929x over previous
"""BitNet DiT on 8 Trainium2 NeuronCores — data-parallel over batch (2 images/core).

Host: patchify, time-embedding + adaLN modulation vectors, BitNet weight
quantization (ternary * per-tensor scale) -> bf16 upload.
Device: full 12-block DiT forward per core in a single Bass/Tile kernel.
BitNet matmuls run as exact integer arithmetic in bf16 (|values| <= 127,
fp32 accumulate). Attention runs in fp32r via transposed-logits + ones-column
softmax-denominator trick.

v2: qT/kT produced directly by weight-side matmuls (no activation
transposes for attention), pipelined attention heads, packed scalar
chains, Sqrt-based rstd (no act-table thrash), Pool-engine offload,
batched quantize-transpose packs, fp32r patch/head matmuls.
"""
import math
import os
import sys
import numpy as np

sys.path.insert(0, "/opt/trn_rl_repo")

import ml_dtypes  # noqa: E402
import concourse.bass as bass  # noqa: E402
import concourse.mybir as mybir  # noqa: E402
import concourse.tile as tile  # noqa: E402
from concourse import bacc  # noqa: E402
from concourse.bass_utils import run_bass_kernel_spmd  # noqa: E402
from concourse.masks import make_identity  # noqa: E402

F32 = mybir.dt.float32
F32R = mybir.dt.float32r
BF16 = mybir.dt.bfloat16
AX = mybir.AxisListType
OP = mybir.AluOpType
AF = mybir.ActivationFunctionType

DIM = 768
DEPTH = int(os.environ.get("KERNEL_DEPTH", "12"))
HEADS = 12
HD = 64
PATCH = 16
IMG = 256
CIN = 3
HID = 4 * DIM
EPS = 1e-6
P = 128
T = 512            # tokens per core (2 images x 256)
NT = T // P        # 4 token tiles
NTOK = 256         # tokens per image
KD = DIM // P      # 6
KH = HID // P      # 24
MAGIC = float(np.float32(3 * 2**22))  # 12582912.0 RNE round-to-int magic

_CACHED = {}


def _mm_chunks(n, c=512):
    out = []
    s = 0
    while s < n:
        e = min(s + c, n)
        out.append((s, e))
        s = e
    return out


def build_program(depth=DEPTH):
    nc = bacc.Bacc("TRN2", target_bir_lowering=False, debug=False, num_devices=8)

    xpT_d = nc.declare_dram_parameter("xpT", [DIM, T], F32R, isOutput=False)
    posb_d = nc.declare_dram_parameter("posb", [NTOK, DIM], F32, isOutput=False)
    patchWT_d = nc.declare_dram_parameter("patchWT", [DIM, DIM], F32R, isOutput=False)
    headWT_d = nc.declare_dram_parameter("headWT", [DIM, DIM], F32R, isOutput=False)
    headb_d = nc.declare_dram_parameter("headb", [1, DIM], F32, isOutput=False)
    wqkv_d = nc.declare_dram_parameter("wqkv", [depth, DIM, 3 * DIM], BF16, isOutput=False)
    wproj_d = nc.declare_dram_parameter("wproj", [depth, DIM, DIM], BF16, isOutput=False)
    wfc1_d = nc.declare_dram_parameter("wfc1", [depth, DIM, HID], BF16, isOutput=False)
    wfc2_d = nc.declare_dram_parameter("wfc2", [depth, HID, DIM], BF16, isOutput=False)
    # modulation vectors: [block, norm(2), img(2), A/B(2), 768] host-broadcast to 128 parts
    mods_d = nc.declare_dram_parameter("mods", [depth, 2, P, 2, 2, DIM], F32, isOutput=False)
    wscl_d = nc.declare_dram_parameter("wscl", [1, 4 * depth], F32, isOutput=False)
    out_d = nc.declare_dram_parameter("zout", [T, DIM], F32, isOutput=True)

    with tile.TileContext(nc) as tc:
        from contextlib import ExitStack
        with ExitStack() as _ctx:
            constp = _ctx.enter_context(tc.tile_pool(name="const", bufs=1))
            residp = _ctx.enter_context(tc.tile_pool(name="resid", bufs=1))
            fm6p = _ctx.enter_context(tc.tile_pool(name="fm6", bufs=2))
            xqTp = _ctx.enter_context(tc.tile_pool(name="xqT", bufs=1))
            wp = _ctx.enter_context(tc.tile_pool(name="w", bufs=3))
            modp = _ctx.enter_context(tc.tile_pool(name="mod", bufs=1))
            tmp_ = _ctx.enter_context(tc.tile_pool(name="tm", bufs=3))
            gp = _ctx.enter_context(tc.tile_pool(name="g", bufs=3))
            qtp = _ctx.enter_context(tc.tile_pool(name="qt", bufs=1))
            xqp = _ctx.enter_context(tc.tile_pool(name="xq", bufs=1))
            eTp = _ctx.enter_context(tc.tile_pool(name="eT", bufs=2))
            scp = _ctx.enter_context(tc.tile_pool(name="sc", bufs=48))
            cbp = _ctx.enter_context(tc.tile_pool(name="cb", bufs=1))
            ps_mm = _ctx.enter_context(tc.tile_pool(name="ps_mm", bufs=2, space="PSUM"))
            ps_tp = _ctx.enter_context(tc.tile_pool(name="ps_tp", bufs=2, space="PSUM"))
            ps_lt = _ctx.enter_context(tc.tile_pool(name="ps_lt", bufs=2, space="PSUM"))
            ps_oa = _ctx.enter_context(tc.tile_pool(name="ps_oa", bufs=2, space="PSUM"))

            idf = constp.tile([P, P], F32)
            make_identity(nc, idf[:])
            idb = constp.tile([P, P], BF16)
            nc.vector.tensor_copy(idb[:], idf[:])
            idr = constp.tile([P, P], F32R)
            nc.vector.tensor_copy(idr[:], idf[:])

            # broadcast w_scales to all partitions
            wsrow = constp.tile([1, 4 * depth], F32)
            nc.sync.dma_start(wsrow[:], wscl_d[:])
            wsb = constp.tile([P, 4 * depth], F32)
            nc.gpsimd.partition_broadcast(wsb[:], wsrow[0:1, :])

            z = residp.tile([P, NT, DIM], F32)
            v_aug = residp.tile([P, NT, HEADS, HD + 1], F32)
            nc.vector.memset(v_aug[:, :, :, HD], 1.0)

            # ---------------- patch embed (fp32r matmuls) ----------------
            posb_sb = wp.tile([P, 2, DIM], F32, tag="w")
            nc.sync.dma_start(posb_sb[:], posb_d.rearrange("(a p) d -> p a d", p=P))
            xpT = fm6p.tile([P, KD, T], F32R, tag="fm6")
            nc.sync.dma_start(xpT[:], xpT_d.rearrange("(o p) t -> p o t", p=P))
            pw = wp.tile([P, KD, DIM], F32R, tag="w")
            nc.sync.dma_start(pw[:], patchWT_d.rearrange("(o p) d -> p o d", p=P))
            for t in range(NT):
                for (cs, ce) in _mm_chunks(DIM):
                    pt = ps_mm.tile([P, 512], F32, tag="mm", name="pmm")[:, : ce - cs]
                    for k in range(KD):
                        nc.tensor.matmul(pt[:], xpT[:, k, t * P:(t + 1) * P],
                                         pw[:, k, cs:ce], start=(k == 0), stop=(k == KD - 1))
                    nc.vector.tensor_tensor(z[:, t, cs:ce], pt[:], posb_sb[:, t % 2, cs:ce], OP.add)

            def load_w(dram, b, kchunks, width, dtype=BF16):
                half = kchunks // 2
                tiles = []
                for i in range(2):
                    wt = wp.tile([P, half, width], dtype, tag="w")
                    nc.sync.dma_start(
                        wt[:],
                        dram[b, i * half * P:(i + 1) * half * P, :].rearrange(
                            "(o p) f -> p o f", p=P))
                    tiles.append(wt)
                return tiles, half

            xqT = xqTp.tile([P, KH, T], BF16, tag="xqT")

            def amax_of(src_ap, dst_slice, eng=None):
                (eng or nc.vector).tensor_reduce(dst_slice, src_ap, axis=AX.X, op=OP.max,
                                                 apply_absolute_value=True)

            def scales_of(amax_pack, ws_idx, n):
                """[P,n] packed: s127 = 127/clip(amax,1e-5); c = clip*ws."""
                acs = scp.tile([P, 8], F32, tag="sc", name="acs")[:, :n]
                nc.vector.tensor_scalar_max(acs[:], amax_pack, 1e-5)
                rs = scp.tile([P, 8], F32, tag="sc", name="rs")[:, :n]
                nc.vector.reciprocal(rs[:], acs[:])
                s127 = scp.tile([P, 8], F32, tag="sc", name="s127")[:, :n]
                nc.vector.tensor_scalar_mul(s127[:], rs[:], 127.0)
                cs = scp.tile([P, 8], F32, tag="sc", name="cs")[:, :n]
                nc.vector.tensor_scalar(cs[:], acs[:], wsb[:, ws_idx:ws_idx + 1],
                                        None, OP.mult)
                return s127, cs

            def quant_data(src_ap, t, kchunks, s127_slice):
                """round(src*s127) -> bf16 ints, transposed into xqT cols t."""
                xq = xqp.tile([P, HID], BF16, tag="xq", name="xq")[:, :kchunks * P]
                for g0 in range(0, kchunks, 12):
                    gn = min(12, kchunks - g0)
                    tmp = qtp.tile([P, 12 * P], F32, tag="qt", name="qtmp")[:, :gn * P]
                    nc.gpsimd.tensor_scalar(tmp[:], src_ap[:, g0 * P:(g0 + gn) * P],
                                            s127_slice, MAGIC, OP.mult, OP.add)
                    nc.vector.tensor_scalar(xq[:, g0 * P:(g0 + gn) * P], tmp[:],
                                            MAGIC, None, OP.subtract)
                k0 = 0
                while k0 < kchunks:
                    g = min(8, kchunks - k0)
                    pack = ps_tp.tile([P, 8, P], BF16, tag="tp", name="tpack")
                    for j in range(g):
                        nc.tensor.transpose(pack[:, j, :], xq[:, (k0 + j) * P:(k0 + j + 1) * P],
                                            idb[:])
                    nc.vector.tensor_copy(xqT[:, k0:k0 + g, t * P:(t + 1) * P],
                                          pack[:, :g, :])
                    k0 += g

            def rstd_of(ssq_pack, n):
                """rstd = 1/sqrt(ssq/DIM + EPS) packed [P,n]."""
                ms = scp.tile([P, 8], F32, tag="sc", name="ms")[:, :n]
                nc.vector.tensor_scalar(ms[:], ssq_pack, 1.0 / DIM, EPS, OP.mult, OP.add)
                rr = scp.tile([P, 8], F32, tag="sc", name="rr")[:, :n]
                nc.vector.reciprocal(rr[:], ms[:])
                rstd = scp.tile([P, 8], F32, tag="sc", name="rstd")[:, :n]
                nc.scalar.activation(rstd[:], rr[:], AF.Sqrt)
                return rstd

            def norm_mod(t, mt, rstd_slice, dst):
                img = t // 2
                nc.vector.scalar_tensor_tensor(dst, z[:, t, :], rstd_slice,
                                               mt[:, img, 0, :], OP.mult, OP.mult)
                nc.vector.tensor_tensor(dst, dst, mt[:, img, 1, :], OP.add)

            def make_cB(crow, cs_pack, j0, n):
                """cs [P,n] (token-partitions) -> crow row segs j0.. (partition 0)."""
                cT = ps_tp.tile([P, NT, P], F32, tag="tp", name="cT")
                for j in range(n):
                    nc.tensor.transpose(cT[0:1, j, :], cs_pack[:, j:j + 1], idf[:])
                nc.vector.tensor_copy(crow[0:1, j0:j0 + n, :], cT[0:1, 0:n, :])

            def bcast_cB(cB, crow):
                for j in range(NT):
                    nc.gpsimd.partition_broadcast(cB[:, j * P:(j + 1) * P], crow[0:1, j, :])

            def ssq_of(src_ap, dst_slice):
                sq = tmp_.tile([P, DIM], F32, tag="tm", name="sqscratch")
                nc.scalar.activation(sq[:], src_ap, AF.Square, accum_out=dst_slice)

            # ---- prologue: norm1+quant of block 0 ----
            mt1 = modp.tile([P, 2, 2, DIM], F32, tag="mod", name="mt1")
            nc.sync.dma_start(mt1[:], mods_d[0, 0])
            ssq0 = scp.tile([P, 8], F32, tag="sc", name="ssq0")[:, :NT]
            for t in range(NT):
                ssq_of(z[:, t, :], ssq0[:, t:t + 1])
            rstd0 = rstd_of(ssq0[:], NT)
            h_nxt = gp.tile([P, NT, DIM], F32, tag="g", name="h_nxt")
            for t in range(NT):
                norm_mod(t, mt1, rstd0[:, t:t + 1], h_nxt[:, t, :])
            amax0 = scp.tile([P, 8], F32, tag="sc", name="amax0")[:, :NT]
            for t in range(NT):
                amax_of(h_nxt[:, t, :], amax0[:, t:t + 1])
            s127n, csn = scales_of(amax0[:], 0, NT)
            crow1 = cbp.tile([1, NT, P], F32, tag="crow", name="crow1")
            make_cB(crow1, csn[:], 0, NT)
            cB1 = cbp.tile([P, T], F32, tag="cb", name="cB1")
            bcast_cB(cB1, crow1)
            for t in range(NT):
                quant_data(h_nxt[:, t, :], t, KD, s127n[:, t:t + 1])
            csn_sl = [csn[:, t:t + 1] for t in range(NT)]
            wq_tiles, wq_half = load_w(wqkv_d, 0, KD, 3 * DIM)

            for b in range(depth):
                import contextlib
                def sc_(nm):
                    return nc.named_scope(f"b{b}_{nm}") if b == 5 else contextlib.nullcontext()
                mt2 = modp.tile([P, 2, 2, DIM], F32, tag="mod", name="mt2")
                nc.sync.dma_start(mt2[:], mods_d[b, 1])

                _p2 = sc_("p2qkv"); _p2.__enter__()
                # --- v (token layout) ---
                for t in range(NT):
                    for (cs0, ce0) in _mm_chunks(DIM):
                        pt = ps_mm.tile([P, 512], F32, tag="mm", name="pmm")[:, : ce0 - cs0]
                        for k in range(KD):
                            wt = wq_tiles[k // wq_half]
                            nc.tensor.matmul(pt[:], xqT[:, k, t * P:(t + 1) * P],
                                             wt[:, k % wq_half, 2 * DIM + cs0:2 * DIM + ce0],
                                             start=(k == 0), stop=(k == KD - 1))
                        h0 = cs0 // HD
                        h1 = ce0 // HD
                        nc.scalar.activation(
                            v_aug[:, t, h0:h1, 0:HD], pt[:],
                            AF.Identity, scale=csn_sl[t])
                # --- qT / kT (feature-partition layout, no transposes) ---
                q_fm = fm6p.tile([P, KD, T], F32R, tag="fm6", name="q_fm")
                k_fm = fm6p.tile([P, KD, T], F32R, tag="fm6", name="k_fm")
                for which, fm, coff in ((0, q_fm, 0), (1, k_fm, DIM)):
                    for qc in range(KD):
                        pt = ps_mm.tile([P, 512], F32, tag="mm", name="pmm")
                        for k in range(KD):
                            wt = wq_tiles[k // wq_half]
                            nc.tensor.matmul(pt[:], wt[:, k % wq_half, coff + qc * P:coff + (qc + 1) * P],
                                             xqT[:, k, :], start=(k == 0), stop=(k == KD - 1))
                        nc.vector.tensor_tensor(fm[:, qc, :], pt[:], cB1[:], OP.mult)
                _p2.__exit__(None, None, None)

                # --- attention: pipelined heads; o-quant, proj and norm2
                #     chains overlapped into the head pipeline ---
                _p3 = sc_("p3attn"); _p3.__enter__()
                wp_tiles, wp_half = load_w(wproj_d, b, KD, DIM)
                o_tm = gp.tile([P, NT, DIM], F32, tag="g", name="o_tm")
                h_all2 = gp.tile([P, NT, DIM], F32, tag="g", name="h_all2")
                amax_o01 = scp.tile([P, 8], F32, tag="sc", name="amaxo01")[:, :2]
                amax_o23 = scp.tile([P, 8], F32, tag="sc", name="amaxo23")[:, :2]
                ssq2 = scp.tile([P, 8], F32, tag="sc", name="ssq2")[:, :NT]
                amax2 = scp.tile([P, 8], F32, tag="sc", name="amax2")[:, :NT]
                s127o = [None, None]
                cpso = [None, None]

                state = [None] * (2 * HEADS)

                def lt_of(i):
                    img, hh = divmod(i, HEADS)
                    po = (hh % 2) * HD
                    ch = hh // 2
                    lt = ps_lt.tile([P, 2, NTOK], F32, tag="lt", name="lt")
                    for mt in range(2):
                        nc.tensor.matmul(
                            lt[:, mt, :],
                            k_fm[po:po + HD, ch, img * NTOK + mt * P: img * NTOK + (mt + 1) * P],
                            q_fm[po:po + HD, ch, img * NTOK: (img + 1) * NTOK],
                            start=True, stop=True)
                    state[i] = lt

                def exp_of(i):
                    lt = state[i]
                    eT = eTp.tile([P, 2, NTOK], F32, tag="eT", name="eT")
                    nc.scalar.activation(eT[:], lt[:], AF.Exp, scale=0.125)
                    state[i] = eT

                def oa_of(i):
                    img, hh = divmod(i, HEADS)
                    eT = state[i]
                    for nt in range(2):
                        oa = ps_oa.tile([P, HD + 1], F32, tag="oa", name="oa")
                        for mt in range(2):
                            nc.tensor.matmul(
                                oa[:], eT[:, mt, nt * P:(nt + 1) * P],
                                v_aug[:, img * 2 + mt, hh, :],
                                start=(mt == 0), stop=(mt == 1))
                        rinv = scp.tile([P, 1], F32, tag="sc", name="rinv")
                        nc.vector.reciprocal(rinv[:], oa[:, HD:HD + 1])
                        nc.scalar.activation(
                            o_tm[:, img * 2 + nt, hh * HD:(hh + 1) * HD],
                            oa[:, 0:HD], AF.Identity, scale=rinv[:])
                    state[i] = None

                def oquant(t):
                    half = t // 2
                    quant_data(o_tm[:, t, :], t, KD, s127o[half][:, (t % 2):(t % 2) + 1])

                def proj_tile(t):
                    cps_sl = cpso[t // 2][:, (t % 2):(t % 2) + 1]
                    for (cs0, ce0) in _mm_chunks(DIM):
                        pt = ps_mm.tile([P, 512], F32, tag="mm", name="pmm")[:, : ce0 - cs0]
                        for k in range(KD):
                            wt = wp_tiles[k // wp_half]
                            nc.tensor.matmul(pt[:], xqT[:, k, t * P:(t + 1) * P],
                                             wt[:, k % wp_half, cs0:ce0],
                                             start=(k == 0), stop=(k == KD - 1))
                        tmp2 = tmp_.tile([P, DIM], F32, tag="tm", name="tmp2")[:, : ce0 - cs0]
                        nc.scalar.activation(tmp2[:], pt[:], AF.Identity, scale=cps_sl)
                        nc.vector.tensor_tensor(z[:, t, cs0:ce0], z[:, t, cs0:ce0], tmp2[:], OP.add)

                def n2chain(t):
                    ssq_of(z[:, t, :], ssq2[:, t:t + 1])
                    r = rstd_of(ssq2[:, t:t + 1], 1)
                    norm_mod(t, mt2, r[:, 0:1], h_all2[:, t, :])
                    amax_of(h_all2[:, t, :], amax2[:, t:t + 1])

                lt_of(0)
                for i in range(1, 2 * HEADS):
                    exp_of(i - 1)
                    lt_of(i)
                    oa_of(i - 1)
                    if i == 13:
                        amax_of(o_tm[:, 0, :], amax_o01[:, 0:1])
                        amax_of(o_tm[:, 1, :], amax_o01[:, 1:2])
                        s127o[0], cpso[0] = scales_of(amax_o01[:], 4 * b + 1, 2)
                        oquant(0)
                    if i == 15:
                        oquant(1)
                    if i == 17:
                        proj_tile(0)
                        n2chain(0)
                    if i == 20:
                        proj_tile(1)
                        n2chain(1)
                exp_of(2 * HEADS - 1)
                oa_of(2 * HEADS - 1)
                amax_of(o_tm[:, 2, :], amax_o23[:, 0:1])
                amax_of(o_tm[:, 3, :], amax_o23[:, 1:2])
                s127o[1], cpso[1] = scales_of(amax_o23[:], 4 * b + 1, 2)
                oquant(2)
                proj_tile(2)
                n2chain(2)
                oquant(3)
                proj_tile(3)
                s1272_01, c3_01 = scales_of(amax2[:, 0:2], 4 * b + 2, 2)
                quant_data(h_all2[:, 0, :], 0, KD, s1272_01[:, 0:1])
                quant_data(h_all2[:, 1, :], 1, KD, s1272_01[:, 1:2])
                n2chain(3)
                _p3.__exit__(None, None, None)

                # --- fc1/gelu + g-quant pipeline (n2 tail chains under cover) ---
                _p5 = sc_("p5fc1"); _p5.__enter__()
                wf1_tiles, wf1_half = load_w(wfc1_d, b, KD, HID)
                wf2_tiles, wf2_half = load_w(wfc2_d, b, KH, DIM)
                gs = [None] * NT
                c4g = [None] * NT
                s1272_23 = None
                c3_23 = None

                def gquant(t):
                    amax_g = scp.tile([P, 1], F32, tag="sc", name="amaxg")
                    amax_of(gs[t][:], amax_g[:])
                    s127g, cg = scales_of(amax_g[:], 4 * b + 3, 1)
                    quant_data(gs[t][:], t, KH, s127g[:, 0:1])
                    c4g[t] = cg

                for t in range(NT):
                    if t == 2:
                        quant_data(h_all2[:, 2, :], 2, KD, s1272_23[:, 0:1])
                    if t == 3:
                        quant_data(h_all2[:, 3, :], 3, KD, s1272_23[:, 1:2])
                    g = gp.tile([P, HID], F32, tag="g")
                    gs[t] = g
                    c3_sl = (c3_01[:, t:t + 1] if t < 2 else c3_23[:, t - 2:t - 1])
                    for (cs0, ce0) in _mm_chunks(HID):
                        pt = ps_mm.tile([P, 512], F32, tag="mm", name="pmm")[:, : ce0 - cs0]
                        for k in range(KD):
                            wt = wf1_tiles[k // wf1_half]
                            nc.tensor.matmul(pt[:], xqT[:, k, t * P:(t + 1) * P],
                                             wt[:, k % wf1_half, cs0:ce0],
                                             start=(k == 0), stop=(k == KD - 1))
                        nc.scalar.activation(g[:, cs0:ce0], pt[:], AF.Gelu_apprx_tanh,
                                             scale=c3_sl)
                    if t == 1:
                        s1272_23, c3_23 = scales_of(amax2[:, 2:4], 4 * b + 2, 2)
                    if t > 0:
                        gquant(t - 1)
                gquant(NT - 1)
                _p5.__exit__(None, None, None)

                # --- fc2 + residual, fused with next block's norm1 ---
                _p6 = sc_("p6fc2"); _p6.__enter__()
                fuse = b + 1 < depth
                if fuse:
                    mt1_nxt = modp.tile([P, 2, 2, DIM], F32, tag="mod", name="mt1n")
                    nc.sync.dma_start(mt1_nxt[:], mods_d[b + 1, 0])
                    h_nxt = gp.tile([P, NT, DIM], F32, tag="g", name="h_nxt")
                    ssqn = scp.tile([P, 8], F32, tag="sc", name="ssqn")[:, :NT]
                    amaxn = scp.tile([P, 8], F32, tag="sc", name="amaxn")[:, :NT]
                    s127n_h = [None, None]
                    csn_h = [None, None]

                    def p1chain(t):
                        ssq_of(z[:, t, :], ssqn[:, t:t + 1])
                        r = rstd_of(ssqn[:, t:t + 1], 1)
                        norm_mod(t, mt1_nxt, r[:, 0:1], h_nxt[:, t, :])
                        amax_of(h_nxt[:, t, :], amaxn[:, t:t + 1])

                for t in range(NT):
                    for (cs0, ce0) in _mm_chunks(DIM):
                        pt = ps_mm.tile([P, 512], F32, tag="mm", name="pmm")[:, : ce0 - cs0]
                        for k in range(KH):
                            wt = wf2_tiles[k // wf2_half]
                            nc.tensor.matmul(pt[:], xqT[:, k, t * P:(t + 1) * P],
                                             wt[:, k % wf2_half, cs0:ce0],
                                             start=(k == 0), stop=(k == KH - 1))
                        tmp2 = tmp_.tile([P, DIM], F32, tag="tm", name="tmp2")[:, : ce0 - cs0]
                        nc.scalar.activation(tmp2[:], pt[:], AF.Identity, scale=c4g[t][:, 0:1])
                        nc.vector.tensor_tensor(z[:, t, cs0:ce0], z[:, t, cs0:ce0], tmp2[:], OP.add)
                    if fuse:
                        p1chain(t)
                        if t == 1:
                            s127n_h[0], csn_h[0] = scales_of(amaxn[:, 0:2], 4 * (b + 1), 2)
                        if t == 2:
                            quant_data(h_nxt[:, 0, :], 0, KD, s127n_h[0][:, 0:1])
                        if t == 3:
                            quant_data(h_nxt[:, 1, :], 1, KD, s127n_h[0][:, 1:2])
                if fuse:
                    s127n_h[1], csn_h[1] = scales_of(amaxn[:, 2:4], 4 * (b + 1), 2)
                    crow1 = cbp.tile([1, NT, P], F32, tag="crow", name="crow1")
                    make_cB(crow1, csn_h[0][:], 0, 2)
                    make_cB(crow1, csn_h[1][:], 2, 2)
                    cB1 = cbp.tile([P, T], F32, tag="cb", name="cB1")
                    bcast_cB(cB1, crow1)
                    quant_data(h_nxt[:, 2, :], 2, KD, s127n_h[1][:, 0:1])
                    quant_data(h_nxt[:, 3, :], 3, KD, s127n_h[1][:, 1:2])
                    csn_sl = [csn_h[0][:, 0:1], csn_h[0][:, 1:2],
                              csn_h[1][:, 0:1], csn_h[1][:, 1:2]]
                    wq_tiles, wq_half = load_w(wqkv_d, b + 1, KD, 3 * DIM)
                _p6.__exit__(None, None, None)

            # ---------------- final norm + head (fp32r) ----------------
            hw = wp.tile([P, KD, DIM], F32R, tag="w")
            nc.sync.dma_start(hw[:], headWT_d.rearrange("(o p) d -> p o d", p=P))
            hbrow = tmp_.tile([1, DIM], F32, tag="tm", name="hbrow")
            nc.sync.dma_start(hbrow[:], headb_d[:])
            hbb = gp.tile([P, DIM], F32, tag="g", name="hbb")
            nc.gpsimd.partition_broadcast(hbb[:], hbrow[0:1, :])
            ssqf = scp.tile([P, 8], F32, tag="sc", name="ssqf")[:, :NT]
            for t in range(NT):
                ssq_of(z[:, t, :], ssqf[:, t:t + 1])
            rstdf = rstd_of(ssqf[:], NT)
            for t in range(NT):
                zn = tmp_.tile([P, DIM], F32R, tag="tm")
                nc.vector.tensor_scalar_mul(zn[:], z[:, t, :], rstdf[:, t:t + 1])
                znT = tmp_.tile([P, DIM], F32R, tag="tm")
                for k in range(KD):
                    ptf = ps_tp.tile([P, P], F32R, tag="tp", name="ptf")
                    nc.tensor.transpose(ptf[:], zn[:, k * P:(k + 1) * P], idr[:])
                    nc.vector.tensor_copy(znT[:, k * P:(k + 1) * P], ptf[:])
                for (cs0, ce0) in _mm_chunks(DIM):
                    pt = ps_mm.tile([P, 512], F32, tag="mm", name="pmm")[:, : ce0 - cs0]
                    for k in range(KD):
                        nc.tensor.matmul(pt[:], znT[:, k * P:(k + 1) * P],
                                         hw[:, k, cs0:ce0], start=(k == 0), stop=(k == KD - 1))
                    ot = tmp_.tile([P, DIM], F32, tag="tm", name="ot")[:, : ce0 - cs0]
                    nc.vector.tensor_tensor(ot[:], pt[:], hbb[:, cs0:ce0], OP.add)
                    nc.sync.dma_start(out_d[t * P:(t + 1) * P, cs0:ce0], ot[:])

    nc.compile()
    return nc


# ---------------------------------------------------------------------------
# host-side numerics (numpy, fp32 — matches jax CPU within ~1e-7)

def _gelu_tanh(x):
    x = x.astype(np.float32)
    c = np.float32(math.sqrt(2.0 / math.pi))
    return np.float32(0.5) * x * (np.float32(1.0) +
                                  np.tanh(c * (x + np.float32(0.044715) * x * x * x)))


def _time_embedding(t, t_w1, t_b1, t_w2, t_b2):
    half = DIM // 2
    freqs = np.exp(-np.log(10000.0) * np.arange(half, dtype=np.float32) / (half - 1)).astype(np.float32)
    args = t[:, None].astype(np.float32) * freqs[None, :]
    emb = np.concatenate([np.sin(args), np.cos(args)], axis=-1).astype(np.float32)
    h = _gelu_tanh(emb @ t_w1.T + t_b1)
    return (h @ t_w2.T + t_b2).astype(np.float32)


def _quant_w(w):
    ws = np.float32(np.mean(np.abs(w), dtype=np.float64)) + np.float32(1e-5)
    wq = np.clip(np.round(w.astype(np.float32) / ws), -1.0, 1.0)
    return wq, ws


def _prepare(inputs):
    x = np.asarray(inputs["x"], np.float32)
    t = np.asarray(inputs["t"], np.float32)
    B = x.shape[0]
    n_cores = 8
    per = B // n_cores  # 2
    p = PATCH
    hh = IMG // p

    xp = x.reshape(B, CIN, hh, p, hh, p).transpose(0, 2, 4, 1, 3, 5).reshape(B, hh * hh, CIN * p * p)

    t_emb = _time_embedding(t, inputs["t_w1"], inputs["t_b1"], inputs["t_w2"], inputs["t_b2"])
    silu = (t_emb / (1.0 + np.exp(-t_emb))).astype(np.float32)

    depth = DEPTH
    mods = np.zeros((depth, 2, B, 2, DIM), np.float32)  # [blk, norm, img, A/B, D]
    wscl = np.zeros((4 * depth,), np.float32)
    wq_all, wp_all, wf1_all, wf2_all = [], [], [], []
    for b in range(depth):
        mod = silu @ np.asarray(inputs["blk_ada_w"][b], np.float32).T + np.asarray(
            inputs["blk_ada_b"][b], np.float32)
        sh1, sc1, sh2, sc2 = np.split(mod, 4, axis=-1)
        n1 = np.asarray(inputs["blk_norm1"][b], np.float32)
        n2 = np.asarray(inputs["blk_norm2"][b], np.float32)
        mods[b, 0, :, 0, :] = n1[None, :] * (1.0 + sc1)
        mods[b, 0, :, 1, :] = sh1
        mods[b, 1, :, 0, :] = n2[None, :] * (1.0 + sc2)
        mods[b, 1, :, 1, :] = sh2

        for j, (nm, lst) in enumerate([("blk_qkv", wq_all), ("blk_proj", wp_all),
                                       ("blk_fc1", wf1_all), ("blk_fc2", wf2_all)]):
            wq, ws = _quant_w(np.asarray(inputs[nm][b], np.float32))
            lst.append(np.ascontiguousarray(wq.T).astype(ml_dtypes.bfloat16))
            wscl[4 * b + j] = ws / np.float32(127.0)

    wqkv = np.stack(wq_all)
    wproj = np.stack(wp_all)
    wfc1 = np.stack(wf1_all)
    wfc2 = np.stack(wf2_all)

    posb = (np.asarray(inputs["pos_embed"][0], np.float32) +
            np.asarray(inputs["patch_b"], np.float32)[None, :]).astype(np.float32)
    patchWT = np.ascontiguousarray(np.asarray(inputs["patch_w"], np.float32).T)
    norm_w = np.asarray(inputs["norm_w"], np.float32)
    headWT = np.ascontiguousarray(np.asarray(inputs["head_w"], np.float32).T * norm_w[:, None])
    headb = np.asarray(inputs["head_b"], np.float32)[None, :]

    key = ("prog", depth)
    if key not in _CACHED:
        _CACHED[key] = build_program(depth)
    nc = _CACHED[key]

    in_maps = []
    for c in range(n_cores):
        imgs = slice(c * per, (c + 1) * per)
        xpT = np.ascontiguousarray(xp[imgs].reshape(per * hh * hh, CIN * p * p).T)
        in_maps.append(dict(
            xpT=xpT, posb=posb, patchWT=patchWT, headWT=headWT, headb=headb,
            wqkv=wqkv, wproj=wproj, wfc1=wfc1, wfc2=wfc2,
            mods=np.ascontiguousarray(
                np.broadcast_to(mods[:, :, None, imgs], (depth, 2, 128, per, 2, DIM))),
            wscl=wscl[None, :],
        ))

    return nc, in_maps


def _assemble(res, B=16, per=2):
    p = PATCH
    hh = IMG // p
    out = np.zeros((B, CIN, IMG, IMG), np.float32)
    for c in range(B // per):
        zo = res.results[c]["zout"]  # [512, 768]
        for i in range(per):
            zi = zo[i * 256:(i + 1) * 256]
            out[c * per + i] = zi.reshape(hh, hh, CIN, p, p).transpose(2, 0, 3, 1, 4).reshape(CIN, IMG, IMG)
    return out


def kernel(**inputs):
    nc, in_maps = _prepare(inputs)
    res = run_bass_kernel_spmd(nc, in_maps, list(range(len(in_maps))), trace=False)
    return _assemble(res)


# revision 29
# speedup vs baseline: 1.2736x; 1.0677x over previous
"""BitNet DiT on 8 Trainium2 NeuronCores — data-parallel over batch (2 images/core).

Host: patchify, time-embedding + adaLN modulation vectors, BitNet weight
quantization (ternary * per-tensor scale) -> bf16 upload.
Device: full 12-block DiT forward per core in a single Bass/Tile kernel.
BitNet matmuls run as exact integer arithmetic in bf16 (|values| <= 127,
fp32 accumulate). Attention runs in fp32r via transposed-logits + ones-column
softmax-denominator trick.

v2: qT/kT produced directly by weight-side matmuls (no activation
transposes for attention), pipelined attention heads, packed scalar
chains, Sqrt-based rstd (no act-table thrash), Pool-engine offload,
batched quantize-transpose packs, fp32r patch/head matmuls.
"""
import math
import os
import sys
import numpy as np

sys.path.insert(0, "/opt/trn_rl_repo")

import ml_dtypes  # noqa: E402
import concourse.bass as bass  # noqa: E402
import concourse.mybir as mybir  # noqa: E402
import concourse.tile as tile  # noqa: E402
from concourse import bacc  # noqa: E402
from concourse.bass_utils import run_bass_kernel_spmd  # noqa: E402
from concourse.masks import make_identity  # noqa: E402

F32 = mybir.dt.float32
F32R = mybir.dt.float32r
BF16 = mybir.dt.bfloat16
AX = mybir.AxisListType
OP = mybir.AluOpType
AF = mybir.ActivationFunctionType

DIM = 768
DEPTH = int(os.environ.get("KERNEL_DEPTH", "12"))
HEADS = 12
HD = 64
PATCH = 16
IMG = 256
CIN = 3
HID = 4 * DIM
EPS = 1e-6
P = 128
T = 512            # tokens per core (2 images x 256)
NT = T // P        # 4 token tiles
NTOK = 256         # tokens per image
KD = DIM // P      # 6
KH = HID // P      # 24
MAGIC = float(np.float32(3 * 2**22))  # 12582912.0 RNE round-to-int magic

_CACHED = {}


def _mm_chunks(n, c=512):
    out = []
    s = 0
    while s < n:
        e = min(s + c, n)
        out.append((s, e))
        s = e
    return out


def build_program(depth=DEPTH):
    nc = bacc.Bacc("TRN2", target_bir_lowering=False, debug=False, num_devices=8)

    xpT_d = nc.declare_dram_parameter("xpT", [DIM, T], F32R, isOutput=False)
    posb_d = nc.declare_dram_parameter("posb", [NTOK, DIM], F32, isOutput=False)
    patchWT_d = nc.declare_dram_parameter("patchWT", [DIM, DIM], F32R, isOutput=False)
    headWT_d = nc.declare_dram_parameter("headWT", [DIM, DIM], F32R, isOutput=False)
    headb_d = nc.declare_dram_parameter("headb", [1, DIM], F32, isOutput=False)
    wqkv_d = nc.declare_dram_parameter("wqkv", [depth, DIM, 3 * DIM], BF16, isOutput=False)
    wproj_d = nc.declare_dram_parameter("wproj", [depth, DIM, DIM], BF16, isOutput=False)
    wfc1_d = nc.declare_dram_parameter("wfc1", [depth, DIM, HID], BF16, isOutput=False)
    wfc2_d = nc.declare_dram_parameter("wfc2", [depth, HID, DIM], BF16, isOutput=False)
    # modulation vectors: [block, norm(2), img(2), A/B(2), 768] host-broadcast to 128 parts
    mods_d = nc.declare_dram_parameter("mods", [depth, 2, P, 2, 2, DIM], F32, isOutput=False)
    wscl_d = nc.declare_dram_parameter("wscl", [1, 4 * depth], F32, isOutput=False)
    out_d = nc.declare_dram_parameter("zout", [T, DIM], F32, isOutput=True)

    with tile.TileContext(nc) as tc:
        from contextlib import ExitStack
        with ExitStack() as _ctx:
            constp = _ctx.enter_context(tc.tile_pool(name="const", bufs=1))
            residp = _ctx.enter_context(tc.tile_pool(name="resid", bufs=1))
            fm6p = _ctx.enter_context(tc.tile_pool(name="fm6", bufs=2))
            xqTp = _ctx.enter_context(tc.tile_pool(name="xqT", bufs=1))
            wp = _ctx.enter_context(tc.tile_pool(name="w", bufs=3))
            modp = _ctx.enter_context(tc.tile_pool(name="mod", bufs=1))
            tmp_ = _ctx.enter_context(tc.tile_pool(name="tm", bufs=3))
            gp = _ctx.enter_context(tc.tile_pool(name="g", bufs=3))
            qtp = _ctx.enter_context(tc.tile_pool(name="qt", bufs=1))
            xqp = _ctx.enter_context(tc.tile_pool(name="xq", bufs=1))
            eTp = _ctx.enter_context(tc.tile_pool(name="eT", bufs=2))
            scp = _ctx.enter_context(tc.tile_pool(name="sc", bufs=48))
            cbp = _ctx.enter_context(tc.tile_pool(name="cb", bufs=1))
            ps_mm = _ctx.enter_context(tc.tile_pool(name="ps_mm", bufs=2, space="PSUM"))
            ps_tp = _ctx.enter_context(tc.tile_pool(name="ps_tp", bufs=2, space="PSUM"))
            ps_lt = _ctx.enter_context(tc.tile_pool(name="ps_lt", bufs=2, space="PSUM"))
            ps_oa = _ctx.enter_context(tc.tile_pool(name="ps_oa", bufs=2, space="PSUM"))

            idf = constp.tile([P, P], F32)
            make_identity(nc, idf[:])
            idb = constp.tile([P, P], BF16)
            nc.vector.tensor_copy(idb[:], idf[:])
            idr = constp.tile([P, P], F32R)
            nc.vector.tensor_copy(idr[:], idf[:])

            # broadcast w_scales to all partitions
            wsrow = constp.tile([1, 4 * depth], F32)
            nc.sync.dma_start(wsrow[:], wscl_d[:])
            wsb = constp.tile([P, 4 * depth], F32)
            nc.gpsimd.partition_broadcast(wsb[:], wsrow[0:1, :])

            z = residp.tile([P, NT, DIM], F32)
            v_aug = residp.tile([P, NT, HEADS, HD + 1], F32)
            nc.vector.memset(v_aug[:, :, :, HD], 1.0)

            # ---------------- patch embed (fp32r matmuls) ----------------
            posb_sb = wp.tile([P, 2, DIM], F32, tag="w")
            nc.sync.dma_start(posb_sb[:], posb_d.rearrange("(a p) d -> p a d", p=P))
            xpT = fm6p.tile([P, KD, T], F32R, tag="fm6")
            nc.sync.dma_start(xpT[:], xpT_d.rearrange("(o p) t -> p o t", p=P))
            pw = wp.tile([P, KD, DIM], F32R, tag="w")
            nc.sync.dma_start(pw[:], patchWT_d.rearrange("(o p) d -> p o d", p=P))
            for t in range(NT):
                for (cs, ce) in _mm_chunks(DIM):
                    pt = ps_mm.tile([P, 512], F32, tag="mm", name="pmm")[:, : ce - cs]
                    for k in range(KD):
                        nc.tensor.matmul(pt[:], xpT[:, k, t * P:(t + 1) * P],
                                         pw[:, k, cs:ce], start=(k == 0), stop=(k == KD - 1))
                    nc.vector.tensor_tensor(z[:, t, cs:ce], pt[:], posb_sb[:, t % 2, cs:ce], OP.add)

            def load_w(dram, b, kchunks, width, dtype=BF16):
                half = kchunks // 2
                tiles = []
                for i in range(2):
                    wt = wp.tile([P, half, width], dtype, tag="w")
                    nc.sync.dma_start(
                        wt[:],
                        dram[b, i * half * P:(i + 1) * half * P, :].rearrange(
                            "(o p) f -> p o f", p=P))
                    tiles.append(wt)
                return tiles, half

            xqT = xqTp.tile([P, KH, T], BF16, tag="xqT")

            def amax_of(src_ap, dst_slice, eng=None):
                (eng or nc.vector).tensor_reduce(dst_slice, src_ap, axis=AX.X, op=OP.max,
                                                 apply_absolute_value=True)

            def scales_of(amax_pack, ws_idx, n):
                """[P,n] packed: s127 = 127/clip(amax,1e-5); c = clip*ws."""
                acs = scp.tile([P, 8], F32, tag="sc", name="acs")[:, :n]
                nc.vector.tensor_scalar_max(acs[:], amax_pack, 1e-5)
                rs = scp.tile([P, 8], F32, tag="sc", name="rs")[:, :n]
                nc.vector.reciprocal(rs[:], acs[:])
                s127 = scp.tile([P, 8], F32, tag="sc", name="s127")[:, :n]
                nc.vector.tensor_scalar_mul(s127[:], rs[:], 127.0)
                cs = scp.tile([P, 8], F32, tag="sc", name="cs")[:, :n]
                nc.vector.tensor_scalar(cs[:], acs[:], wsb[:, ws_idx:ws_idx + 1],
                                        None, OP.mult)
                return s127, cs

            def quant_data(src_ap, t, kchunks, s127_slice):
                """round(src*s127) -> bf16 ints, transposed into xqT cols t."""
                xq = xqp.tile([P, HID], BF16, tag="xq", name="xq")[:, :kchunks * P]
                for g0 in range(0, kchunks, 12):
                    gn = min(12, kchunks - g0)
                    tmp = qtp.tile([P, 12 * P], F32, tag="qt", name="qtmp")[:, :gn * P]
                    nc.gpsimd.tensor_scalar(tmp[:], src_ap[:, g0 * P:(g0 + gn) * P],
                                            s127_slice, MAGIC, OP.mult, OP.add)
                    nc.vector.tensor_scalar(xq[:, g0 * P:(g0 + gn) * P], tmp[:],
                                            MAGIC, None, OP.subtract)
                k0 = 0
                while k0 < kchunks:
                    g = min(8, kchunks - k0)
                    pack = ps_tp.tile([P, 8, P], BF16, tag="tp", name="tpack")
                    for j in range(g):
                        nc.tensor.transpose(pack[:, j, :], xq[:, (k0 + j) * P:(k0 + j + 1) * P],
                                            idb[:])
                    nc.vector.tensor_copy(xqT[:, k0:k0 + g, t * P:(t + 1) * P],
                                          pack[:, :g, :])
                    k0 += g

            def rstd_of(ssq_pack, n):
                """rstd = 1/sqrt(ssq/DIM + EPS) packed [P,n]."""
                ms = scp.tile([P, 8], F32, tag="sc", name="ms")[:, :n]
                nc.vector.tensor_scalar(ms[:], ssq_pack, 1.0 / DIM, EPS, OP.mult, OP.add)
                rr = scp.tile([P, 8], F32, tag="sc", name="rr")[:, :n]
                nc.vector.reciprocal(rr[:], ms[:])
                rstd = scp.tile([P, 8], F32, tag="sc", name="rstd")[:, :n]
                nc.scalar.activation(rstd[:], rr[:], AF.Sqrt)
                return rstd

            def norm_mod(t, mt, rstd_slice, dst):
                img = t // 2
                nc.vector.scalar_tensor_tensor(dst, z[:, t, :], rstd_slice,
                                               mt[:, img, 0, :], OP.mult, OP.mult)
                nc.vector.tensor_tensor(dst, dst, mt[:, img, 1, :], OP.add)

            def make_cB(crow, cs_pack, j0, n):
                """cs [P,n] (token-partitions) -> crow row segs j0.. (partition 0)."""
                cT = ps_tp.tile([P, NT, P], F32, tag="tp", name="cT")
                for j in range(n):
                    nc.tensor.transpose(cT[0:1, j, :], cs_pack[:, j:j + 1], idf[:])
                nc.vector.tensor_copy(crow[0:1, j0:j0 + n, :], cT[0:1, 0:n, :])

            def bcast_cB(cB, crow):
                for j in range(NT):
                    nc.gpsimd.partition_broadcast(cB[:, j * P:(j + 1) * P], crow[0:1, j, :])

            def ssq_of(src_ap, dst_slice):
                sq = tmp_.tile([P, DIM], F32, tag="tm", name="sqscratch")
                nc.scalar.activation(sq[:], src_ap, AF.Square, accum_out=dst_slice)

            # ---- prologue: norm1+quant of block 0 ----
            mt1 = modp.tile([P, 2, 2, DIM], F32, tag="mod", name="mt1")
            nc.sync.dma_start(mt1[:], mods_d[0, 0])
            ssq0 = scp.tile([P, 8], F32, tag="sc", name="ssq0")[:, :NT]
            for t in range(NT):
                ssq_of(z[:, t, :], ssq0[:, t:t + 1])
            rstd0 = rstd_of(ssq0[:], NT)
            h_nxt = gp.tile([P, NT, DIM], F32, tag="g", name="h_nxt")
            for t in range(NT):
                norm_mod(t, mt1, rstd0[:, t:t + 1], h_nxt[:, t, :])
            amax0 = scp.tile([P, 8], F32, tag="sc", name="amax0")[:, :NT]
            for t in range(NT):
                amax_of(h_nxt[:, t, :], amax0[:, t:t + 1])
            s127n, csn = scales_of(amax0[:], 0, NT)
            crow1 = cbp.tile([1, NT, P], F32, tag="crow", name="crow1")
            make_cB(crow1, csn[:], 0, NT)
            cB1 = cbp.tile([P, T], F32, tag="cb", name="cB1")
            bcast_cB(cB1, crow1)
            for t in range(NT):
                quant_data(h_nxt[:, t, :], t, KD, s127n[:, t:t + 1])
            csn_sl = [csn[:, t:t + 1] for t in range(NT)]
            wq_tiles, wq_half = load_w(wqkv_d, 0, KD, 3 * DIM)

            for b in range(depth):
                import contextlib
                def sc_(nm):
                    return nc.named_scope(f"b{b}_{nm}") if b == 5 else contextlib.nullcontext()
                mt2 = modp.tile([P, 2, 2, DIM], F32, tag="mod", name="mt2")
                nc.sync.dma_start(mt2[:], mods_d[b, 1])

                _p2 = sc_("p2qkv"); _p2.__enter__()
                # --- v (token layout) ---
                for t in range(NT):
                    for (cs0, ce0) in _mm_chunks(DIM):
                        pt = ps_mm.tile([P, 512], F32, tag="mm", name="pmm")[:, : ce0 - cs0]
                        for k in range(KD):
                            wt = wq_tiles[k // wq_half]
                            nc.tensor.matmul(pt[:], xqT[:, k, t * P:(t + 1) * P],
                                             wt[:, k % wq_half, 2 * DIM + cs0:2 * DIM + ce0],
                                             start=(k == 0), stop=(k == KD - 1))
                        h0 = cs0 // HD
                        h1 = ce0 // HD
                        nc.scalar.activation(
                            v_aug[:, t, h0:h1, 0:HD], pt[:],
                            AF.Identity, scale=csn_sl[t])
                # --- qT / kT (feature-partition layout, no transposes) ---
                q_fm = fm6p.tile([P, KD, T], F32R, tag="fm6", name="q_fm")
                k_fm = fm6p.tile([P, KD, T], F32R, tag="fm6", name="k_fm")
                for which, fm, coff in ((0, q_fm, 0), (1, k_fm, DIM)):
                    for qc in range(KD):
                        pt = ps_mm.tile([P, 512], F32, tag="mm", name="pmm")
                        for k in range(KD):
                            wt = wq_tiles[k // wq_half]
                            nc.tensor.matmul(pt[:], wt[:, k % wq_half, coff + qc * P:coff + (qc + 1) * P],
                                             xqT[:, k, :], start=(k == 0), stop=(k == KD - 1))
                        nc.vector.tensor_tensor(fm[:, qc, :], pt[:], cB1[:], OP.mult)
                _p2.__exit__(None, None, None)

                # --- attention: pipelined heads; o-quant, proj and norm2
                #     chains overlapped into the head pipeline ---
                _p3 = sc_("p3attn"); _p3.__enter__()
                wp_tiles, wp_half = load_w(wproj_d, b, KD, DIM)
                o_tm = gp.tile([P, NT, DIM], F32, tag="g", name="o_tm")
                h_all2 = gp.tile([P, NT, DIM], F32, tag="g", name="h_all2")
                amax_o01 = scp.tile([P, 8], F32, tag="sc", name="amaxo01")[:, :2]
                amax_o23 = scp.tile([P, 8], F32, tag="sc", name="amaxo23")[:, :2]
                ssq2 = scp.tile([P, 8], F32, tag="sc", name="ssq2")[:, :NT]
                amax2 = scp.tile([P, 8], F32, tag="sc", name="amax2")[:, :NT]
                s127o = [None, None]
                cpso = [None, None]

                state = [None] * (2 * HEADS)

                def lt_of(i):
                    img, hh = divmod(i, HEADS)
                    po = (hh % 2) * HD
                    ch = hh // 2
                    lt = ps_lt.tile([P, 2, NTOK], F32, tag="lt", name="lt")
                    for mt in range(2):
                        nc.tensor.matmul(
                            lt[:, mt, :],
                            k_fm[po:po + HD, ch, img * NTOK + mt * P: img * NTOK + (mt + 1) * P],
                            q_fm[po:po + HD, ch, img * NTOK: (img + 1) * NTOK],
                            start=True, stop=True)
                    state[i] = lt

                def exp_of(i):
                    lt = state[i]
                    eT = eTp.tile([P, 2, NTOK], F32, tag="eT", name="eT")
                    nc.scalar.activation(eT[:], lt[:], AF.Exp, scale=0.125)
                    state[i] = eT

                def oa_of(i):
                    img, hh = divmod(i, HEADS)
                    eT = state[i]
                    for nt in range(2):
                        oa = ps_oa.tile([P, HD + 1], F32, tag="oa", name="oa")
                        for mt in range(2):
                            nc.tensor.matmul(
                                oa[:], eT[:, mt, nt * P:(nt + 1) * P],
                                v_aug[:, img * 2 + mt, hh, :],
                                start=(mt == 0), stop=(mt == 1))
                        rinv = scp.tile([P, 1], F32, tag="sc", name="rinv")
                        nc.vector.reciprocal(rinv[:], oa[:, HD:HD + 1])
                        nc.scalar.activation(
                            o_tm[:, img * 2 + nt, hh * HD:(hh + 1) * HD],
                            oa[:, 0:HD], AF.Identity, scale=rinv[:])
                    state[i] = None

                def oquant(t):
                    half = t // 2
                    quant_data(o_tm[:, t, :], t, KD, s127o[half][:, (t % 2):(t % 2) + 1])

                def proj_tile(t):
                    cps_sl = cpso[t // 2][:, (t % 2):(t % 2) + 1]
                    for (cs0, ce0) in _mm_chunks(DIM):
                        pt = ps_mm.tile([P, 512], F32, tag="mm", name="pmm")[:, : ce0 - cs0]
                        for k in range(KD):
                            wt = wp_tiles[k // wp_half]
                            nc.tensor.matmul(pt[:], xqT[:, k, t * P:(t + 1) * P],
                                             wt[:, k % wp_half, cs0:ce0],
                                             start=(k == 0), stop=(k == KD - 1))
                        tmp2 = tmp_.tile([P, DIM], F32, tag="tm", name="tmp2")[:, : ce0 - cs0]
                        nc.scalar.activation(tmp2[:], pt[:], AF.Identity, scale=cps_sl)
                        nc.vector.tensor_tensor(z[:, t, cs0:ce0], z[:, t, cs0:ce0], tmp2[:], OP.add)

                def n2chain(t):
                    ssq_of(z[:, t, :], ssq2[:, t:t + 1])
                    r = rstd_of(ssq2[:, t:t + 1], 1)
                    norm_mod(t, mt2, r[:, 0:1], h_all2[:, t, :])
                    amax_of(h_all2[:, t, :], amax2[:, t:t + 1])

                lt_of(0)
                for i in range(1, 2 * HEADS):
                    exp_of(i - 1)
                    lt_of(i)
                    oa_of(i - 1)
                    if i == 13:
                        amax_of(o_tm[:, 0, :], amax_o01[:, 0:1])
                        amax_of(o_tm[:, 1, :], amax_o01[:, 1:2])
                        s127o[0], cpso[0] = scales_of(amax_o01[:], 4 * b + 1, 2)
                        oquant(0)
                    if i == 15:
                        oquant(1)
                    if i == 17:
                        proj_tile(0)
                        n2chain(0)
                    if i == 20:
                        proj_tile(1)
                        n2chain(1)
                exp_of(2 * HEADS - 1)
                oa_of(2 * HEADS - 1)
                amax_of(o_tm[:, 2, :], amax_o23[:, 0:1])
                amax_of(o_tm[:, 3, :], amax_o23[:, 1:2])
                s127o[1], cpso[1] = scales_of(amax_o23[:], 4 * b + 1, 2)
                oquant(2)
                proj_tile(2)
                n2chain(2)
                oquant(3)
                proj_tile(3)
                s1272_01, c3_01 = scales_of(amax2[:, 0:2], 4 * b + 2, 2)
                quant_data(h_all2[:, 0, :], 0, KD, s1272_01[:, 0:1])
                quant_data(h_all2[:, 1, :], 1, KD, s1272_01[:, 1:2])
                n2chain(3)
                s1272_23, c3_23 = scales_of(amax2[:, 2:4], 4 * b + 2, 2)
                c3_sl = [c3_01[:, 0:1], c3_01[:, 1:2], c3_23[:, 0:1], c3_23[:, 1:2]]
                _p3.__exit__(None, None, None)

                # --- fc1/gelu + g-quant pipeline ---
                _p5 = sc_("p5fc1"); _p5.__enter__()
                wf1_tiles, wf1_half = load_w(wfc1_d, b, KD, HID)
                gs = [None] * NT
                c4g = [None] * NT

                def gquant(t):
                    amax_g = scp.tile([P, 1], F32, tag="sc", name="amaxg")
                    amax_of(gs[t][:], amax_g[:])
                    s127g, cg = scales_of(amax_g[:], 4 * b + 3, 1)
                    quant_data(gs[t][:], t, KH, s127g[:, 0:1])
                    c4g[t] = cg

                for t in range(NT):
                    if t == 0:
                        quant_data(h_all2[:, 2, :], 2, KD, s1272_23[:, 0:1])
                    if t == 1:
                        quant_data(h_all2[:, 3, :], 3, KD, s1272_23[:, 1:2])
                    g = gp.tile([P, HID], F32, tag="g")
                    gs[t] = g
                    for (cs0, ce0) in _mm_chunks(HID):
                        pt = ps_mm.tile([P, 512], F32, tag="mm", name="pmm")[:, : ce0 - cs0]
                        for k in range(KD):
                            wt = wf1_tiles[k // wf1_half]
                            nc.tensor.matmul(pt[:], xqT[:, k, t * P:(t + 1) * P],
                                             wt[:, k % wf1_half, cs0:ce0],
                                             start=(k == 0), stop=(k == KD - 1))
                        nc.scalar.activation(g[:, cs0:ce0], pt[:], AF.Gelu_apprx_tanh,
                                             scale=c3_sl[t])
                    if t > 0:
                        gquant(t - 1)
                gquant(NT - 1)
                _p5.__exit__(None, None, None)

                # --- fc2 + residual, fused with next block's norm1 ---
                _p6 = sc_("p6fc2"); _p6.__enter__()
                wf2_tiles, wf2_half = load_w(wfc2_d, b, KH, DIM)
                fuse = b + 1 < depth
                if fuse:
                    mt1_nxt = modp.tile([P, 2, 2, DIM], F32, tag="mod", name="mt1n")
                    nc.sync.dma_start(mt1_nxt[:], mods_d[b + 1, 0])
                    h_nxt = gp.tile([P, NT, DIM], F32, tag="g", name="h_nxt")
                    ssqn = scp.tile([P, 8], F32, tag="sc", name="ssqn")[:, :NT]
                    amaxn = scp.tile([P, 8], F32, tag="sc", name="amaxn")[:, :NT]
                    s127n_h = [None, None]
                    csn_h = [None, None]

                    def p1chain(t):
                        ssq_of(z[:, t, :], ssqn[:, t:t + 1])
                        r = rstd_of(ssqn[:, t:t + 1], 1)
                        norm_mod(t, mt1_nxt, r[:, 0:1], h_nxt[:, t, :])
                        amax_of(h_nxt[:, t, :], amaxn[:, t:t + 1])

                for t in range(NT):
                    for (cs0, ce0) in _mm_chunks(DIM):
                        pt = ps_mm.tile([P, 512], F32, tag="mm", name="pmm")[:, : ce0 - cs0]
                        for k in range(KH):
                            wt = wf2_tiles[k // wf2_half]
                            nc.tensor.matmul(pt[:], xqT[:, k, t * P:(t + 1) * P],
                                             wt[:, k % wf2_half, cs0:ce0],
                                             start=(k == 0), stop=(k == KH - 1))
                        tmp2 = tmp_.tile([P, DIM], F32, tag="tm", name="tmp2")[:, : ce0 - cs0]
                        nc.scalar.activation(tmp2[:], pt[:], AF.Identity, scale=c4g[t][:, 0:1])
                        nc.vector.tensor_tensor(z[:, t, cs0:ce0], z[:, t, cs0:ce0], tmp2[:], OP.add)
                    if fuse:
                        p1chain(t)
                        if t == 1:
                            s127n_h[0], csn_h[0] = scales_of(amaxn[:, 0:2], 4 * (b + 1), 2)
                        if t == 2:
                            quant_data(h_nxt[:, 0, :], 0, KD, s127n_h[0][:, 0:1])
                        if t == 3:
                            quant_data(h_nxt[:, 1, :], 1, KD, s127n_h[0][:, 1:2])
                if fuse:
                    s127n_h[1], csn_h[1] = scales_of(amaxn[:, 2:4], 4 * (b + 1), 2)
                    crow1 = cbp.tile([1, NT, P], F32, tag="crow", name="crow1")
                    make_cB(crow1, csn_h[0][:], 0, 2)
                    make_cB(crow1, csn_h[1][:], 2, 2)
                    cB1 = cbp.tile([P, T], F32, tag="cb", name="cB1")
                    bcast_cB(cB1, crow1)
                    quant_data(h_nxt[:, 2, :], 2, KD, s127n_h[1][:, 0:1])
                    quant_data(h_nxt[:, 3, :], 3, KD, s127n_h[1][:, 1:2])
                    csn_sl = [csn_h[0][:, 0:1], csn_h[0][:, 1:2],
                              csn_h[1][:, 0:1], csn_h[1][:, 1:2]]
                    wq_tiles, wq_half = load_w(wqkv_d, b + 1, KD, 3 * DIM)
                _p6.__exit__(None, None, None)

            # ---------------- final norm + head (fp32r) ----------------
            hw = wp.tile([P, KD, DIM], F32R, tag="w")
            nc.sync.dma_start(hw[:], headWT_d.rearrange("(o p) d -> p o d", p=P))
            hbrow = tmp_.tile([1, DIM], F32, tag="tm", name="hbrow")
            nc.sync.dma_start(hbrow[:], headb_d[:])
            hbb = gp.tile([P, DIM], F32, tag="g", name="hbb")
            nc.gpsimd.partition_broadcast(hbb[:], hbrow[0:1, :])
            ssqf = scp.tile([P, 8], F32, tag="sc", name="ssqf")[:, :NT]
            for t in range(NT):
                ssq_of(z[:, t, :], ssqf[:, t:t + 1])
            rstdf = rstd_of(ssqf[:], NT)
            for t in range(NT):
                zn = tmp_.tile([P, DIM], F32R, tag="tm")
                nc.vector.tensor_scalar_mul(zn[:], z[:, t, :], rstdf[:, t:t + 1])
                znT = tmp_.tile([P, DIM], F32R, tag="tm")
                for k in range(KD):
                    ptf = ps_tp.tile([P, P], F32R, tag="tp", name="ptf")
                    nc.tensor.transpose(ptf[:], zn[:, k * P:(k + 1) * P], idr[:])
                    nc.vector.tensor_copy(znT[:, k * P:(k + 1) * P], ptf[:])
                for (cs0, ce0) in _mm_chunks(DIM):
                    pt = ps_mm.tile([P, 512], F32, tag="mm", name="pmm")[:, : ce0 - cs0]
                    for k in range(KD):
                        nc.tensor.matmul(pt[:], znT[:, k * P:(k + 1) * P],
                                         hw[:, k, cs0:ce0], start=(k == 0), stop=(k == KD - 1))
                    ot = tmp_.tile([P, DIM], F32, tag="tm", name="ot")[:, : ce0 - cs0]
                    nc.vector.tensor_tensor(ot[:], pt[:], hbb[:, cs0:ce0], OP.add)
                    nc.sync.dma_start(out_d[t * P:(t + 1) * P, cs0:ce0], ot[:])

    nc.compile()
    return nc


# ---------------------------------------------------------------------------
# host-side numerics (numpy, fp32 — matches jax CPU within ~1e-7)

def _gelu_tanh(x):
    x = x.astype(np.float32)
    c = np.float32(math.sqrt(2.0 / math.pi))
    return np.float32(0.5) * x * (np.float32(1.0) +
                                  np.tanh(c * (x + np.float32(0.044715) * x * x * x)))


def _time_embedding(t, t_w1, t_b1, t_w2, t_b2):
    half = DIM // 2
    freqs = np.exp(-np.log(10000.0) * np.arange(half, dtype=np.float32) / (half - 1)).astype(np.float32)
    args = t[:, None].astype(np.float32) * freqs[None, :]
    emb = np.concatenate([np.sin(args), np.cos(args)], axis=-1).astype(np.float32)
    h = _gelu_tanh(emb @ t_w1.T + t_b1)
    return (h @ t_w2.T + t_b2).astype(np.float32)


def _quant_w(w):
    ws = np.float32(np.mean(np.abs(w), dtype=np.float64)) + np.float32(1e-5)
    wq = np.clip(np.round(w.astype(np.float32) / ws), -1.0, 1.0)
    return wq, ws


def _prepare(inputs):
    x = np.asarray(inputs["x"], np.float32)
    t = np.asarray(inputs["t"], np.float32)
    B = x.shape[0]
    n_cores = 8
    per = B // n_cores  # 2
    p = PATCH
    hh = IMG // p

    xp = x.reshape(B, CIN, hh, p, hh, p).transpose(0, 2, 4, 1, 3, 5).reshape(B, hh * hh, CIN * p * p)

    t_emb = _time_embedding(t, inputs["t_w1"], inputs["t_b1"], inputs["t_w2"], inputs["t_b2"])
    silu = (t_emb / (1.0 + np.exp(-t_emb))).astype(np.float32)

    depth = DEPTH
    mods = np.zeros((depth, 2, B, 2, DIM), np.float32)  # [blk, norm, img, A/B, D]
    wscl = np.zeros((4 * depth,), np.float32)
    wq_all, wp_all, wf1_all, wf2_all = [], [], [], []
    for b in range(depth):
        mod = silu @ np.asarray(inputs["blk_ada_w"][b], np.float32).T + np.asarray(
            inputs["blk_ada_b"][b], np.float32)
        sh1, sc1, sh2, sc2 = np.split(mod, 4, axis=-1)
        n1 = np.asarray(inputs["blk_norm1"][b], np.float32)
        n2 = np.asarray(inputs["blk_norm2"][b], np.float32)
        mods[b, 0, :, 0, :] = n1[None, :] * (1.0 + sc1)
        mods[b, 0, :, 1, :] = sh1
        mods[b, 1, :, 0, :] = n2[None, :] * (1.0 + sc2)
        mods[b, 1, :, 1, :] = sh2

        for j, (nm, lst) in enumerate([("blk_qkv", wq_all), ("blk_proj", wp_all),
                                       ("blk_fc1", wf1_all), ("blk_fc2", wf2_all)]):
            wq, ws = _quant_w(np.asarray(inputs[nm][b], np.float32))
            lst.append(np.ascontiguousarray(wq.T).astype(ml_dtypes.bfloat16))
            wscl[4 * b + j] = ws / np.float32(127.0)

    wqkv = np.stack(wq_all)
    wproj = np.stack(wp_all)
    wfc1 = np.stack(wf1_all)
    wfc2 = np.stack(wf2_all)

    posb = (np.asarray(inputs["pos_embed"][0], np.float32) +
            np.asarray(inputs["patch_b"], np.float32)[None, :]).astype(np.float32)
    patchWT = np.ascontiguousarray(np.asarray(inputs["patch_w"], np.float32).T)
    norm_w = np.asarray(inputs["norm_w"], np.float32)
    headWT = np.ascontiguousarray(np.asarray(inputs["head_w"], np.float32).T * norm_w[:, None])
    headb = np.asarray(inputs["head_b"], np.float32)[None, :]

    key = ("prog", depth)
    if key not in _CACHED:
        _CACHED[key] = build_program(depth)
    nc = _CACHED[key]

    in_maps = []
    for c in range(n_cores):
        imgs = slice(c * per, (c + 1) * per)
        xpT = np.ascontiguousarray(xp[imgs].reshape(per * hh * hh, CIN * p * p).T)
        in_maps.append(dict(
            xpT=xpT, posb=posb, patchWT=patchWT, headWT=headWT, headb=headb,
            wqkv=wqkv, wproj=wproj, wfc1=wfc1, wfc2=wfc2,
            mods=np.ascontiguousarray(
                np.broadcast_to(mods[:, :, None, imgs], (depth, 2, 128, per, 2, DIM))),
            wscl=wscl[None, :],
        ))

    return nc, in_maps


def _assemble(res, B=16, per=2):
    p = PATCH
    hh = IMG // p
    out = np.zeros((B, CIN, IMG, IMG), np.float32)
    for c in range(B // per):
        zo = res.results[c]["zout"]  # [512, 768]
        for i in range(per):
            zi = zo[i * 256:(i + 1) * 256]
            out[c * per + i] = zi.reshape(hh, hh, CIN, p, p).transpose(2, 0, 3, 1, 4).reshape(CIN, IMG, IMG)
    return out


def kernel(**inputs):
    nc, in_maps = _prepare(inputs)
    res = run_bass_kernel_spmd(nc, in_maps, list(range(len(in_maps))), trace=False)
    return _assemble(res)


# revision 32
# speedup vs baseline: 1.2855x; 1.0093x over previous
"""BitNet DiT on 8 Trainium2 NeuronCores — data-parallel over batch (2 images/core).

Host: patchify, time-embedding + adaLN modulation vectors, BitNet weight
quantization (ternary * per-tensor scale) -> bf16 upload.
Device: full 12-block DiT forward per core in a single Bass/Tile kernel.
BitNet matmuls run as exact integer arithmetic in bf16 (|values| <= 127,
fp32 accumulate). Attention runs in fp32r via transposed-logits + ones-column
softmax-denominator trick.

v2: qT/kT produced directly by weight-side matmuls (no activation
transposes for attention), pipelined attention heads, packed scalar
chains, Sqrt-based rstd (no act-table thrash), Pool-engine offload,
batched quantize-transpose packs, fp32r patch/head matmuls.
"""
import math
import os
import sys
import numpy as np

sys.path.insert(0, "/opt/trn_rl_repo")

import ml_dtypes  # noqa: E402
import concourse.bass as bass  # noqa: E402
import concourse.mybir as mybir  # noqa: E402
import concourse.tile as tile  # noqa: E402
from concourse import bacc  # noqa: E402
from concourse.bass_utils import run_bass_kernel_spmd  # noqa: E402
from concourse.masks import make_identity  # noqa: E402

F32 = mybir.dt.float32
F32R = mybir.dt.float32r
BF16 = mybir.dt.bfloat16
AX = mybir.AxisListType
OP = mybir.AluOpType
AF = mybir.ActivationFunctionType

DIM = 768
DEPTH = int(os.environ.get("KERNEL_DEPTH", "12"))
HEADS = 12
HD = 64
PATCH = 16
IMG = 256
CIN = 3
HID = 4 * DIM
EPS = 1e-6
P = 128
T = 512            # tokens per core (2 images x 256)
NT = T // P        # 4 token tiles
NTOK = 256         # tokens per image
KD = DIM // P      # 6
KH = HID // P      # 24
MAGIC = float(np.float32(3 * 2**22))  # 12582912.0 RNE round-to-int magic

_CACHED = {}


def _mm_chunks(n, c=512):
    out = []
    s = 0
    while s < n:
        e = min(s + c, n)
        out.append((s, e))
        s = e
    return out


def build_program(depth=DEPTH):
    nc = bacc.Bacc("TRN2", target_bir_lowering=False, debug=False, num_devices=8)

    xpT_d = nc.declare_dram_parameter("xpT", [DIM, T], F32R, isOutput=False)
    posb_d = nc.declare_dram_parameter("posb", [NTOK, DIM], F32, isOutput=False)
    patchWT_d = nc.declare_dram_parameter("patchWT", [DIM, DIM], F32R, isOutput=False)
    headWT_d = nc.declare_dram_parameter("headWT", [DIM, DIM], F32R, isOutput=False)
    headb_d = nc.declare_dram_parameter("headb", [1, DIM], F32, isOutput=False)
    wqkv_d = nc.declare_dram_parameter("wqkv", [depth, DIM, 3 * DIM], BF16, isOutput=False)
    wproj_d = nc.declare_dram_parameter("wproj", [depth, DIM, DIM], BF16, isOutput=False)
    wfc1_d = nc.declare_dram_parameter("wfc1", [depth, DIM, HID], BF16, isOutput=False)
    wfc2_d = nc.declare_dram_parameter("wfc2", [depth, HID, DIM], BF16, isOutput=False)
    # modulation vectors: [block, norm(2), img(2), A/B(2), 768] host-broadcast to 128 parts
    mods_d = nc.declare_dram_parameter("mods", [depth, 2, P, 2, 2, DIM], F32, isOutput=False)
    wscl_d = nc.declare_dram_parameter("wscl", [1, 4 * depth], F32, isOutput=False)
    out_d = nc.declare_dram_parameter("zout", [T, DIM], F32, isOutput=True)

    with tile.TileContext(nc) as tc:
        from contextlib import ExitStack
        with ExitStack() as _ctx:
            constp = _ctx.enter_context(tc.tile_pool(name="const", bufs=1))
            residp = _ctx.enter_context(tc.tile_pool(name="resid", bufs=1))
            fm6p = _ctx.enter_context(tc.tile_pool(name="fm6", bufs=2))
            xqTp = _ctx.enter_context(tc.tile_pool(name="xqT", bufs=1))
            wp = _ctx.enter_context(tc.tile_pool(name="w", bufs=3))
            modp = _ctx.enter_context(tc.tile_pool(name="mod", bufs=1))
            tmp_ = _ctx.enter_context(tc.tile_pool(name="tm", bufs=3))
            gp = _ctx.enter_context(tc.tile_pool(name="g", bufs=3))
            qtp = _ctx.enter_context(tc.tile_pool(name="qt", bufs=1))
            xqp = _ctx.enter_context(tc.tile_pool(name="xq", bufs=1))
            eTp = _ctx.enter_context(tc.tile_pool(name="eT", bufs=2))
            scp = _ctx.enter_context(tc.tile_pool(name="sc", bufs=48))
            cbp = _ctx.enter_context(tc.tile_pool(name="cb", bufs=1))
            ps_mm = _ctx.enter_context(tc.tile_pool(name="ps_mm", bufs=3, space="PSUM"))
            ps_tp = _ctx.enter_context(tc.tile_pool(name="ps_tp", bufs=2, space="PSUM"))
            ps_lt = _ctx.enter_context(tc.tile_pool(name="ps_lt", bufs=3, space="PSUM"))

            idf = constp.tile([P, P], F32)
            make_identity(nc, idf[:])
            idb = constp.tile([P, P], BF16)
            nc.vector.tensor_copy(idb[:], idf[:])
            idr = constp.tile([P, P], F32R)
            nc.vector.tensor_copy(idr[:], idf[:])

            # broadcast w_scales to all partitions
            wsrow = constp.tile([1, 4 * depth], F32)
            nc.sync.dma_start(wsrow[:], wscl_d[:])
            wsb = constp.tile([P, 4 * depth], F32)
            nc.gpsimd.partition_broadcast(wsb[:], wsrow[0:1, :])

            z = residp.tile([P, NT, DIM], F32)
            v_aug = residp.tile([P, NT, HEADS, HD + 1], F32)
            nc.vector.memset(v_aug[:, :, :, HD], 1.0)

            # ---------------- patch embed (fp32r matmuls) ----------------
            posb_sb = wp.tile([P, 2, DIM], F32, tag="w")
            nc.sync.dma_start(posb_sb[:], posb_d.rearrange("(a p) d -> p a d", p=P))
            xpT = fm6p.tile([P, KD, T], F32R, tag="fm6")
            nc.sync.dma_start(xpT[:], xpT_d.rearrange("(o p) t -> p o t", p=P))
            pw = wp.tile([P, KD, DIM], F32R, tag="w")
            nc.sync.dma_start(pw[:], patchWT_d.rearrange("(o p) d -> p o d", p=P))
            for t in range(NT):
                for (cs, ce) in _mm_chunks(DIM):
                    pt = ps_mm.tile([P, 512], F32, tag="mm", name="pmm")[:, : ce - cs]
                    for k in range(KD):
                        nc.tensor.matmul(pt[:], xpT[:, k, t * P:(t + 1) * P],
                                         pw[:, k, cs:ce], start=(k == 0), stop=(k == KD - 1))
                    nc.vector.tensor_tensor(z[:, t, cs:ce], pt[:], posb_sb[:, t % 2, cs:ce], OP.add)

            def load_w(dram, b, kchunks, width, dtype=BF16):
                half = kchunks // 2
                tiles = []
                for i in range(2):
                    wt = wp.tile([P, half, width], dtype, tag="w")
                    nc.sync.dma_start(
                        wt[:],
                        dram[b, i * half * P:(i + 1) * half * P, :].rearrange(
                            "(o p) f -> p o f", p=P))
                    tiles.append(wt)
                return tiles, half

            xqT = xqTp.tile([P, KH, T], BF16, tag="xqT")

            def amax_of(src_ap, dst_slice, eng=None):
                (eng or nc.vector).tensor_reduce(dst_slice, src_ap, axis=AX.X, op=OP.max,
                                                 apply_absolute_value=True)

            def scales_of(amax_pack, ws_idx, n):
                """[P,n] packed: s127 = 127/clip(amax,1e-5); c = clip*ws."""
                acs = scp.tile([P, 8], F32, tag="sc", name="acs")[:, :n]
                nc.vector.tensor_scalar_max(acs[:], amax_pack, 1e-5)
                rs = scp.tile([P, 8], F32, tag="sc", name="rs")[:, :n]
                nc.vector.reciprocal(rs[:], acs[:])
                s127 = scp.tile([P, 8], F32, tag="sc", name="s127")[:, :n]
                nc.vector.tensor_scalar_mul(s127[:], rs[:], 127.0)
                cs = scp.tile([P, 8], F32, tag="sc", name="cs")[:, :n]
                nc.vector.tensor_scalar(cs[:], acs[:], wsb[:, ws_idx:ws_idx + 1],
                                        None, OP.mult)
                return s127, cs

            def quant_data(src_ap, t, kchunks, s127_slice):
                """round(src*s127) -> bf16 ints, transposed into xqT cols t."""
                xq = xqp.tile([P, HID], BF16, tag="xq", name="xq")[:, :kchunks * P]
                for g0 in range(0, kchunks, 12):
                    gn = min(12, kchunks - g0)
                    tmp = qtp.tile([P, 12 * P], F32, tag="qt", name="qtmp")[:, :gn * P]
                    nc.gpsimd.tensor_scalar(tmp[:], src_ap[:, g0 * P:(g0 + gn) * P],
                                            s127_slice, MAGIC, OP.mult, OP.add)
                    nc.vector.tensor_scalar(xq[:, g0 * P:(g0 + gn) * P], tmp[:],
                                            MAGIC, None, OP.subtract)
                k0 = 0
                while k0 < kchunks:
                    g = min(8, kchunks - k0)
                    pack = ps_tp.tile([P, 8, P], BF16, tag="tp", name="tpack")
                    for j in range(g):
                        nc.tensor.transpose(pack[:, j, :], xq[:, (k0 + j) * P:(k0 + j + 1) * P],
                                            idb[:])
                    nc.vector.tensor_copy(xqT[:, k0:k0 + g, t * P:(t + 1) * P],
                                          pack[:, :g, :])
                    k0 += g

            def rstd_of(ssq_pack, n):
                """rstd = 1/sqrt(ssq/DIM + EPS) packed [P,n]."""
                ms = scp.tile([P, 8], F32, tag="sc", name="ms")[:, :n]
                nc.vector.tensor_scalar(ms[:], ssq_pack, 1.0 / DIM, EPS, OP.mult, OP.add)
                rr = scp.tile([P, 8], F32, tag="sc", name="rr")[:, :n]
                nc.vector.reciprocal(rr[:], ms[:])
                rstd = scp.tile([P, 8], F32, tag="sc", name="rstd")[:, :n]
                nc.scalar.activation(rstd[:], rr[:], AF.Sqrt)
                return rstd

            def norm_mod(t, mt, rstd_slice, dst):
                img = t // 2
                nc.vector.scalar_tensor_tensor(dst, z[:, t, :], rstd_slice,
                                               mt[:, img, 0, :], OP.mult, OP.mult)
                nc.vector.tensor_tensor(dst, dst, mt[:, img, 1, :], OP.add)

            def make_cB(crow, cs_pack, j0, n):
                """cs [P,n] (token-partitions) -> crow row segs j0.. (partition 0)."""
                cT = ps_tp.tile([P, NT, P], F32, tag="tp", name="cT")
                for j in range(n):
                    nc.tensor.transpose(cT[0:1, j, :], cs_pack[:, j:j + 1], idf[:])
                nc.vector.tensor_copy(crow[0:1, j0:j0 + n, :], cT[0:1, 0:n, :])

            def bcast_cB(cB, crow):
                for j in range(NT):
                    nc.gpsimd.partition_broadcast(cB[:, j * P:(j + 1) * P], crow[0:1, j, :])

            def ssq_of(src_ap, dst_slice):
                sq = tmp_.tile([P, DIM], F32, tag="tm", name="sqscratch")
                nc.scalar.activation(sq[:], src_ap, AF.Square, accum_out=dst_slice)

            # ---- prologue: norm1+quant of block 0 ----
            mt1 = modp.tile([P, 2, 2, DIM], F32, tag="mod", name="mt1")
            nc.sync.dma_start(mt1[:], mods_d[0, 0])
            ssq0 = scp.tile([P, 8], F32, tag="sc", name="ssq0")[:, :NT]
            for t in range(NT):
                ssq_of(z[:, t, :], ssq0[:, t:t + 1])
            rstd0 = rstd_of(ssq0[:], NT)
            h_nxt = gp.tile([P, NT, DIM], F32, tag="g", name="h_nxt")
            for t in range(NT):
                norm_mod(t, mt1, rstd0[:, t:t + 1], h_nxt[:, t, :])
            amax0 = scp.tile([P, 8], F32, tag="sc", name="amax0")[:, :NT]
            for t in range(NT):
                amax_of(h_nxt[:, t, :], amax0[:, t:t + 1])
            s127n, csn = scales_of(amax0[:], 0, NT)
            crow1 = cbp.tile([1, NT, P], F32, tag="crow", name="crow1")
            make_cB(crow1, csn[:], 0, NT)
            cB1 = cbp.tile([P, T], F32, tag="cb", name="cB1")
            bcast_cB(cB1, crow1)
            for t in range(NT):
                quant_data(h_nxt[:, t, :], t, KD, s127n[:, t:t + 1])
            csn_sl = [csn[:, t:t + 1] for t in range(NT)]
            wq_tiles, wq_half = load_w(wqkv_d, 0, KD, 3 * DIM)

            for b in range(depth):
                import contextlib
                def sc_(nm):
                    return nc.named_scope(f"b{b}_{nm}") if b == 5 else contextlib.nullcontext()
                mt2 = modp.tile([P, 2, 2, DIM], F32, tag="mod", name="mt2")
                nc.sync.dma_start(mt2[:], mods_d[b, 1])

                _p2 = sc_("p2qkv"); _p2.__enter__()
                # --- v (token layout) ---
                for t in range(NT):
                    for (cs0, ce0) in _mm_chunks(DIM):
                        pt = ps_mm.tile([P, 512], F32, tag="mm", name="pmm")[:, : ce0 - cs0]
                        for k in range(KD):
                            wt = wq_tiles[k // wq_half]
                            nc.tensor.matmul(pt[:], xqT[:, k, t * P:(t + 1) * P],
                                             wt[:, k % wq_half, 2 * DIM + cs0:2 * DIM + ce0],
                                             start=(k == 0), stop=(k == KD - 1))
                        h0 = cs0 // HD
                        h1 = ce0 // HD
                        nc.scalar.activation(
                            v_aug[:, t, h0:h1, 0:HD], pt[:],
                            AF.Identity, scale=csn_sl[t])
                # --- qT / kT (feature-partition layout, no transposes) ---
                q_fm = fm6p.tile([P, KD, T], F32R, tag="fm6", name="q_fm")
                k_fm = fm6p.tile([P, KD, T], F32R, tag="fm6", name="k_fm")
                for which, fm, coff in ((0, q_fm, 0), (1, k_fm, DIM)):
                    for qc in range(KD):
                        pt = ps_mm.tile([P, 512], F32, tag="mm", name="pmm")
                        for k in range(KD):
                            wt = wq_tiles[k // wq_half]
                            nc.tensor.matmul(pt[:], wt[:, k % wq_half, coff + qc * P:coff + (qc + 1) * P],
                                             xqT[:, k, :], start=(k == 0), stop=(k == KD - 1))
                        nc.vector.tensor_tensor(fm[:, qc, :], pt[:], cB1[:], OP.mult)
                _p2.__exit__(None, None, None)

                # --- attention: pipelined heads; o-quant, proj and norm2
                #     chains overlapped into the head pipeline ---
                _p3 = sc_("p3attn"); _p3.__enter__()
                wp_tiles, wp_half = load_w(wproj_d, b, KD, DIM)
                o_tm = gp.tile([P, NT, DIM], F32, tag="g", name="o_tm")
                h_all2 = gp.tile([P, NT, DIM], F32, tag="g", name="h_all2")
                amax_o01 = scp.tile([P, 8], F32, tag="sc", name="amaxo01")[:, :2]
                amax_o23 = scp.tile([P, 8], F32, tag="sc", name="amaxo23")[:, :2]
                ssq2 = scp.tile([P, 8], F32, tag="sc", name="ssq2")[:, :NT]
                amax2 = scp.tile([P, 8], F32, tag="sc", name="amax2")[:, :NT]
                s127o = [None, None]
                cpso = [None, None]

                state = [None] * (2 * HEADS)

                def lt_of(i):
                    img, hh = divmod(i, HEADS)
                    po = (hh % 2) * HD
                    ch = hh // 2
                    lt = ps_lt.tile([P, 2, NTOK], F32, tag="lt", name="lt")
                    for mt in range(2):
                        nc.tensor.matmul(
                            lt[:, mt, :],
                            k_fm[po:po + HD, ch, img * NTOK + mt * P: img * NTOK + (mt + 1) * P],
                            q_fm[po:po + HD, ch, img * NTOK: (img + 1) * NTOK],
                            start=True, stop=True)
                    state[i] = lt

                def exp_of(i):
                    lt = state[i]
                    eT = eTp.tile([P, 2, NTOK], F32, tag="eT", name="eT")
                    nc.scalar.activation(eT[:], lt[:], AF.Exp, scale=0.125)
                    state[i] = eT

                def oa_of(i):
                    img, hh = divmod(i, HEADS)
                    eT = state[i]
                    for nt in range(2):
                        oa = ps_lt.tile([P, HD + 1], F32, tag="lt", name="oa")
                        for mt in range(2):
                            nc.tensor.matmul(
                                oa[:], eT[:, mt, nt * P:(nt + 1) * P],
                                v_aug[:, img * 2 + mt, hh, :],
                                start=(mt == 0), stop=(mt == 1))
                        rinv = scp.tile([P, 1], F32, tag="sc", name="rinv")
                        nc.vector.reciprocal(rinv[:], oa[:, HD:HD + 1])
                        nc.scalar.activation(
                            o_tm[:, img * 2 + nt, hh * HD:(hh + 1) * HD],
                            oa[:, 0:HD], AF.Identity, scale=rinv[:])
                    state[i] = None

                def oquant(t):
                    half = t // 2
                    quant_data(o_tm[:, t, :], t, KD, s127o[half][:, (t % 2):(t % 2) + 1])

                def proj_tile(t):
                    cps_sl = cpso[t // 2][:, (t % 2):(t % 2) + 1]
                    for (cs0, ce0) in _mm_chunks(DIM):
                        pt = ps_mm.tile([P, 512], F32, tag="mm", name="pmm")[:, : ce0 - cs0]
                        for k in range(KD):
                            wt = wp_tiles[k // wp_half]
                            nc.tensor.matmul(pt[:], xqT[:, k, t * P:(t + 1) * P],
                                             wt[:, k % wp_half, cs0:ce0],
                                             start=(k == 0), stop=(k == KD - 1))
                        tmp2 = tmp_.tile([P, DIM], F32, tag="tm", name="tmp2")[:, : ce0 - cs0]
                        nc.scalar.activation(tmp2[:], pt[:], AF.Identity, scale=cps_sl)
                        nc.vector.tensor_tensor(z[:, t, cs0:ce0], z[:, t, cs0:ce0], tmp2[:], OP.add)

                def n2chain(t):
                    ssq_of(z[:, t, :], ssq2[:, t:t + 1])
                    r = rstd_of(ssq2[:, t:t + 1], 1)
                    norm_mod(t, mt2, r[:, 0:1], h_all2[:, t, :])
                    amax_of(h_all2[:, t, :], amax2[:, t:t + 1])

                lt_of(0)
                for i in range(1, 2 * HEADS):
                    exp_of(i - 1)
                    lt_of(i)
                    oa_of(i - 1)
                    if i == 13:
                        amax_of(o_tm[:, 0, :], amax_o01[:, 0:1])
                        amax_of(o_tm[:, 1, :], amax_o01[:, 1:2])
                        s127o[0], cpso[0] = scales_of(amax_o01[:], 4 * b + 1, 2)
                        oquant(0)
                    if i == 15:
                        oquant(1)
                    if i == 17:
                        proj_tile(0)
                        n2chain(0)
                    if i == 20:
                        proj_tile(1)
                        n2chain(1)
                exp_of(2 * HEADS - 1)
                oa_of(2 * HEADS - 1)
                amax_of(o_tm[:, 2, :], amax_o23[:, 0:1])
                amax_of(o_tm[:, 3, :], amax_o23[:, 1:2])
                s127o[1], cpso[1] = scales_of(amax_o23[:], 4 * b + 1, 2)
                oquant(2)
                proj_tile(2)
                n2chain(2)
                oquant(3)
                proj_tile(3)
                s1272_01, c3_01 = scales_of(amax2[:, 0:2], 4 * b + 2, 2)
                quant_data(h_all2[:, 0, :], 0, KD, s1272_01[:, 0:1])
                quant_data(h_all2[:, 1, :], 1, KD, s1272_01[:, 1:2])
                n2chain(3)
                s1272_23, c3_23 = scales_of(amax2[:, 2:4], 4 * b + 2, 2)
                c3_sl = [c3_01[:, 0:1], c3_01[:, 1:2], c3_23[:, 0:1], c3_23[:, 1:2]]
                _p3.__exit__(None, None, None)

                # --- fc1/gelu + g-quant pipeline ---
                _p5 = sc_("p5fc1"); _p5.__enter__()
                wf1_tiles, wf1_half = load_w(wfc1_d, b, KD, HID)
                gs = [None] * NT
                c4g = [None] * NT

                def gquant(t):
                    amax_g = scp.tile([P, 1], F32, tag="sc", name="amaxg")
                    amax_of(gs[t][:], amax_g[:])
                    s127g, cg = scales_of(amax_g[:], 4 * b + 3, 1)
                    quant_data(gs[t][:], t, KH, s127g[:, 0:1])
                    c4g[t] = cg

                for t in range(NT):
                    if t == 0:
                        quant_data(h_all2[:, 2, :], 2, KD, s1272_23[:, 0:1])
                    if t == 1:
                        quant_data(h_all2[:, 3, :], 3, KD, s1272_23[:, 1:2])
                    g = gp.tile([P, HID], F32, tag="g")
                    gs[t] = g
                    for (cs0, ce0) in _mm_chunks(HID):
                        pt = ps_mm.tile([P, 512], F32, tag="mm", name="pmm")[:, : ce0 - cs0]
                        for k in range(KD):
                            wt = wf1_tiles[k // wf1_half]
                            nc.tensor.matmul(pt[:], xqT[:, k, t * P:(t + 1) * P],
                                             wt[:, k % wf1_half, cs0:ce0],
                                             start=(k == 0), stop=(k == KD - 1))
                        nc.scalar.activation(g[:, cs0:ce0], pt[:], AF.Gelu_apprx_tanh,
                                             scale=c3_sl[t])
                    if t > 0:
                        gquant(t - 1)
                gquant(NT - 1)
                _p5.__exit__(None, None, None)

                # --- fc2 + residual, fused with next block's norm1 ---
                _p6 = sc_("p6fc2"); _p6.__enter__()
                wf2_tiles, wf2_half = load_w(wfc2_d, b, KH, DIM)
                fuse = b + 1 < depth
                if fuse:
                    mt1_nxt = modp.tile([P, 2, 2, DIM], F32, tag="mod", name="mt1n")
                    nc.sync.dma_start(mt1_nxt[:], mods_d[b + 1, 0])
                    h_nxt = gp.tile([P, NT, DIM], F32, tag="g", name="h_nxt")
                    ssqn = scp.tile([P, 8], F32, tag="sc", name="ssqn")[:, :NT]
                    amaxn = scp.tile([P, 8], F32, tag="sc", name="amaxn")[:, :NT]
                    s127n_h = [None, None]
                    csn_h = [None, None]

                    def p1chain(t):
                        ssq_of(z[:, t, :], ssqn[:, t:t + 1])
                        r = rstd_of(ssqn[:, t:t + 1], 1)
                        norm_mod(t, mt1_nxt, r[:, 0:1], h_nxt[:, t, :])
                        amax_of(h_nxt[:, t, :], amaxn[:, t:t + 1])

                for t in range(NT):
                    for (cs0, ce0) in _mm_chunks(DIM):
                        pt = ps_mm.tile([P, 512], F32, tag="mm", name="pmm")[:, : ce0 - cs0]
                        for k in range(KH):
                            wt = wf2_tiles[k // wf2_half]
                            nc.tensor.matmul(pt[:], xqT[:, k, t * P:(t + 1) * P],
                                             wt[:, k % wf2_half, cs0:ce0],
                                             start=(k == 0), stop=(k == KH - 1))
                        tmp2 = tmp_.tile([P, DIM], F32, tag="tm", name="tmp2")[:, : ce0 - cs0]
                        nc.scalar.activation(tmp2[:], pt[:], AF.Identity, scale=c4g[t][:, 0:1])
                        nc.vector.tensor_tensor(z[:, t, cs0:ce0], z[:, t, cs0:ce0], tmp2[:], OP.add)
                    if fuse:
                        p1chain(t)
                        if t == 1:
                            s127n_h[0], csn_h[0] = scales_of(amaxn[:, 0:2], 4 * (b + 1), 2)
                        if t == 2:
                            quant_data(h_nxt[:, 0, :], 0, KD, s127n_h[0][:, 0:1])
                        if t == 3:
                            quant_data(h_nxt[:, 1, :], 1, KD, s127n_h[0][:, 1:2])
                if fuse:
                    s127n_h[1], csn_h[1] = scales_of(amaxn[:, 2:4], 4 * (b + 1), 2)
                    crow1 = cbp.tile([1, NT, P], F32, tag="crow", name="crow1")
                    make_cB(crow1, csn_h[0][:], 0, 2)
                    make_cB(crow1, csn_h[1][:], 2, 2)
                    cB1 = cbp.tile([P, T], F32, tag="cb", name="cB1")
                    bcast_cB(cB1, crow1)
                    quant_data(h_nxt[:, 2, :], 2, KD, s127n_h[1][:, 0:1])
                    quant_data(h_nxt[:, 3, :], 3, KD, s127n_h[1][:, 1:2])
                    csn_sl = [csn_h[0][:, 0:1], csn_h[0][:, 1:2],
                              csn_h[1][:, 0:1], csn_h[1][:, 1:2]]
                    wq_tiles, wq_half = load_w(wqkv_d, b + 1, KD, 3 * DIM)
                _p6.__exit__(None, None, None)

            # ---------------- final norm + head (fp32r) ----------------
            hw = wp.tile([P, KD, DIM], F32R, tag="w")
            nc.sync.dma_start(hw[:], headWT_d.rearrange("(o p) d -> p o d", p=P))
            hbrow = tmp_.tile([1, DIM], F32, tag="tm", name="hbrow")
            nc.sync.dma_start(hbrow[:], headb_d[:])
            hbb = gp.tile([P, DIM], F32, tag="g", name="hbb")
            nc.gpsimd.partition_broadcast(hbb[:], hbrow[0:1, :])
            ssqf = scp.tile([P, 8], F32, tag="sc", name="ssqf")[:, :NT]
            for t in range(NT):
                ssq_of(z[:, t, :], ssqf[:, t:t + 1])
            rstdf = rstd_of(ssqf[:], NT)
            for t in range(NT):
                zn = tmp_.tile([P, DIM], F32R, tag="tm")
                nc.vector.tensor_scalar_mul(zn[:], z[:, t, :], rstdf[:, t:t + 1])
                znT = tmp_.tile([P, DIM], F32R, tag="tm")
                for k in range(KD):
                    ptf = ps_tp.tile([P, P], F32R, tag="tp", name="ptf")
                    nc.tensor.transpose(ptf[:], zn[:, k * P:(k + 1) * P], idr[:])
                    nc.vector.tensor_copy(znT[:, k * P:(k + 1) * P], ptf[:])
                for (cs0, ce0) in _mm_chunks(DIM):
                    pt = ps_mm.tile([P, 512], F32, tag="mm", name="pmm")[:, : ce0 - cs0]
                    for k in range(KD):
                        nc.tensor.matmul(pt[:], znT[:, k * P:(k + 1) * P],
                                         hw[:, k, cs0:ce0], start=(k == 0), stop=(k == KD - 1))
                    ot = tmp_.tile([P, DIM], F32, tag="tm", name="ot")[:, : ce0 - cs0]
                    nc.vector.tensor_tensor(ot[:], pt[:], hbb[:, cs0:ce0], OP.add)
                    nc.sync.dma_start(out_d[t * P:(t + 1) * P, cs0:ce0], ot[:])

    nc.compile()
    return nc


# ---------------------------------------------------------------------------
# host-side numerics (numpy, fp32 — matches jax CPU within ~1e-7)

def _gelu_tanh(x):
    x = x.astype(np.float32)
    c = np.float32(math.sqrt(2.0 / math.pi))
    return np.float32(0.5) * x * (np.float32(1.0) +
                                  np.tanh(c * (x + np.float32(0.044715) * x * x * x)))


def _time_embedding(t, t_w1, t_b1, t_w2, t_b2):
    half = DIM // 2
    freqs = np.exp(-np.log(10000.0) * np.arange(half, dtype=np.float32) / (half - 1)).astype(np.float32)
    args = t[:, None].astype(np.float32) * freqs[None, :]
    emb = np.concatenate([np.sin(args), np.cos(args)], axis=-1).astype(np.float32)
    h = _gelu_tanh(emb @ t_w1.T + t_b1)
    return (h @ t_w2.T + t_b2).astype(np.float32)


def _quant_w(w):
    ws = np.float32(np.mean(np.abs(w), dtype=np.float64)) + np.float32(1e-5)
    wq = np.clip(np.round(w.astype(np.float32) / ws), -1.0, 1.0)
    return wq, ws


def _prepare(inputs):
    x = np.asarray(inputs["x"], np.float32)
    t = np.asarray(inputs["t"], np.float32)
    B = x.shape[0]
    n_cores = 8
    per = B // n_cores  # 2
    p = PATCH
    hh = IMG // p

    xp = x.reshape(B, CIN, hh, p, hh, p).transpose(0, 2, 4, 1, 3, 5).reshape(B, hh * hh, CIN * p * p)

    t_emb = _time_embedding(t, inputs["t_w1"], inputs["t_b1"], inputs["t_w2"], inputs["t_b2"])
    silu = (t_emb / (1.0 + np.exp(-t_emb))).astype(np.float32)

    depth = DEPTH
    mods = np.zeros((depth, 2, B, 2, DIM), np.float32)  # [blk, norm, img, A/B, D]
    wscl = np.zeros((4 * depth,), np.float32)
    wq_all, wp_all, wf1_all, wf2_all = [], [], [], []
    for b in range(depth):
        mod = silu @ np.asarray(inputs["blk_ada_w"][b], np.float32).T + np.asarray(
            inputs["blk_ada_b"][b], np.float32)
        sh1, sc1, sh2, sc2 = np.split(mod, 4, axis=-1)
        n1 = np.asarray(inputs["blk_norm1"][b], np.float32)
        n2 = np.asarray(inputs["blk_norm2"][b], np.float32)
        mods[b, 0, :, 0, :] = n1[None, :] * (1.0 + sc1)
        mods[b, 0, :, 1, :] = sh1
        mods[b, 1, :, 0, :] = n2[None, :] * (1.0 + sc2)
        mods[b, 1, :, 1, :] = sh2

        for j, (nm, lst) in enumerate([("blk_qkv", wq_all), ("blk_proj", wp_all),
                                       ("blk_fc1", wf1_all), ("blk_fc2", wf2_all)]):
            wq, ws = _quant_w(np.asarray(inputs[nm][b], np.float32))
            lst.append(np.ascontiguousarray(wq.T).astype(ml_dtypes.bfloat16))
            wscl[4 * b + j] = ws / np.float32(127.0)

    wqkv = np.stack(wq_all)
    wproj = np.stack(wp_all)
    wfc1 = np.stack(wf1_all)
    wfc2 = np.stack(wf2_all)

    posb = (np.asarray(inputs["pos_embed"][0], np.float32) +
            np.asarray(inputs["patch_b"], np.float32)[None, :]).astype(np.float32)
    patchWT = np.ascontiguousarray(np.asarray(inputs["patch_w"], np.float32).T)
    norm_w = np.asarray(inputs["norm_w"], np.float32)
    headWT = np.ascontiguousarray(np.asarray(inputs["head_w"], np.float32).T * norm_w[:, None])
    headb = np.asarray(inputs["head_b"], np.float32)[None, :]

    key = ("prog", depth)
    if key not in _CACHED:
        _CACHED[key] = build_program(depth)
    nc = _CACHED[key]

    in_maps = []
    for c in range(n_cores):
        imgs = slice(c * per, (c + 1) * per)
        xpT = np.ascontiguousarray(xp[imgs].reshape(per * hh * hh, CIN * p * p).T)
        in_maps.append(dict(
            xpT=xpT, posb=posb, patchWT=patchWT, headWT=headWT, headb=headb,
            wqkv=wqkv, wproj=wproj, wfc1=wfc1, wfc2=wfc2,
            mods=np.ascontiguousarray(
                np.broadcast_to(mods[:, :, None, imgs], (depth, 2, 128, per, 2, DIM))),
            wscl=wscl[None, :],
        ))

    return nc, in_maps


def _assemble(res, B=16, per=2):
    p = PATCH
    hh = IMG // p
    out = np.zeros((B, CIN, IMG, IMG), np.float32)
    for c in range(B // per):
        zo = res.results[c]["zout"]  # [512, 768]
        for i in range(per):
            zi = zo[i * 256:(i + 1) * 256]
            out[c * per + i] = zi.reshape(hh, hh, CIN, p, p).transpose(2, 0, 3, 1, 4).reshape(CIN, IMG, IMG)
    return out


def kernel(**inputs):
    nc, in_maps = _prepare(inputs)
    res = run_bass_kernel_spmd(nc, in_maps, list(range(len(in_maps))), trace=False)
    return _assemble(res)


# revision 34
# speedup vs baseline: 1.4335x; 1.1152x over previous
"""BitNet DiT on 8 Trainium2 NeuronCores — data-parallel over batch (2 images/core).

Host: patchify, time-embedding + adaLN modulation vectors, BitNet weight
quantization (ternary * per-tensor scale) -> bf16 upload.
Device: full 12-block DiT forward per core in a single Bass/Tile kernel.
BitNet matmuls run as exact integer arithmetic in bf16 (|values| <= 127,
fp32 accumulate). Attention runs in fp32r via transposed-logits + ones-column
softmax-denominator trick.

v2: qT/kT produced directly by weight-side matmuls (no activation
transposes for attention), pipelined attention heads, packed scalar
chains, Sqrt-based rstd (no act-table thrash), Pool-engine offload,
batched quantize-transpose packs, fp32r patch/head matmuls.
"""
import math
import os
import sys
import numpy as np

sys.path.insert(0, "/opt/trn_rl_repo")

import ml_dtypes  # noqa: E402
import concourse.bass as bass  # noqa: E402
import concourse.mybir as mybir  # noqa: E402
import concourse.tile as tile  # noqa: E402
from concourse import bacc  # noqa: E402
from concourse.bass_utils import run_bass_kernel_spmd  # noqa: E402
from concourse.masks import make_identity  # noqa: E402

F32 = mybir.dt.float32
F32R = mybir.dt.float32r
BF16 = mybir.dt.bfloat16
AX = mybir.AxisListType
OP = mybir.AluOpType
AF = mybir.ActivationFunctionType

DIM = 768
DEPTH = int(os.environ.get("KERNEL_DEPTH", "12"))
HEADS = 12
HD = 64
PATCH = 16
IMG = 256
CIN = 3
HID = 4 * DIM
EPS = 1e-6
P = 128
T = 512            # tokens per core (2 images x 256)
NT = T // P        # 4 token tiles
NTOK = 256         # tokens per image
KD = DIM // P      # 6
KH = HID // P      # 24
MAGIC = float(np.float32(3 * 2**22))  # 12582912.0 RNE round-to-int magic

_CACHED = {}


def _mm_chunks(n, c=512):
    out = []
    s = 0
    while s < n:
        e = min(s + c, n)
        out.append((s, e))
        s = e
    return out


def build_program(depth=DEPTH):
    nc = bacc.Bacc("TRN2", target_bir_lowering=False, debug=False, num_devices=8)

    xpT_d = nc.declare_dram_parameter("xpT", [DIM, T], F32R, isOutput=False)
    posb_d = nc.declare_dram_parameter("posb", [NTOK, DIM], F32, isOutput=False)
    patchWT_d = nc.declare_dram_parameter("patchWT", [DIM, DIM], F32R, isOutput=False)
    headWT_d = nc.declare_dram_parameter("headWT", [DIM, DIM], F32R, isOutput=False)
    headb_d = nc.declare_dram_parameter("headb", [1, DIM], F32, isOutput=False)
    wqkv_d = nc.declare_dram_parameter("wqkv", [depth, DIM, 3 * DIM], BF16, isOutput=False)
    wproj_d = nc.declare_dram_parameter("wproj", [depth, DIM, DIM], BF16, isOutput=False)
    wfc1_d = nc.declare_dram_parameter("wfc1", [depth, DIM, HID], BF16, isOutput=False)
    wfc2_d = nc.declare_dram_parameter("wfc2", [depth, HID, DIM], BF16, isOutput=False)
    # modulation vectors: [block, norm(2), img(2), A/B(2), 768] host-broadcast to 128 parts
    mods_d = nc.declare_dram_parameter("mods", [depth, 2, P, 2, 2, DIM], F32, isOutput=False)
    wscl_d = nc.declare_dram_parameter("wscl", [1, 4 * depth], F32, isOutput=False)
    out_d = nc.declare_dram_parameter("zout", [T, DIM], F32, isOutput=True)

    with tile.TileContext(nc) as tc:
        from contextlib import ExitStack
        with ExitStack() as _ctx:
            constp = _ctx.enter_context(tc.tile_pool(name="const", bufs=1))
            residp = _ctx.enter_context(tc.tile_pool(name="resid", bufs=1))
            fm6p = _ctx.enter_context(tc.tile_pool(name="fm6", bufs=2))
            xqTp = _ctx.enter_context(tc.tile_pool(name="xqT", bufs=1))
            wp = _ctx.enter_context(tc.tile_pool(name="w", bufs=3))
            modp = _ctx.enter_context(tc.tile_pool(name="mod", bufs=1))
            tmp_ = _ctx.enter_context(tc.tile_pool(name="tm", bufs=3))
            gp = _ctx.enter_context(tc.tile_pool(name="g", bufs=3))
            qtp = _ctx.enter_context(tc.tile_pool(name="qt", bufs=1))
            xqp = _ctx.enter_context(tc.tile_pool(name="xq", bufs=1))
            eTp = _ctx.enter_context(tc.tile_pool(name="eT", bufs=2))
            scp = _ctx.enter_context(tc.tile_pool(name="sc", bufs=48))
            cbp = _ctx.enter_context(tc.tile_pool(name="cb", bufs=1))
            ps_mm = _ctx.enter_context(tc.tile_pool(name="ps_mm", bufs=3, space="PSUM"))
            ps_tp = _ctx.enter_context(tc.tile_pool(name="ps_tp", bufs=2, space="PSUM"))
            ps_lt = _ctx.enter_context(tc.tile_pool(name="ps_lt", bufs=3, space="PSUM"))

            idf = constp.tile([P, P], F32)
            make_identity(nc, idf[:])
            idb = constp.tile([P, P], BF16)
            nc.vector.tensor_copy(idb[:], idf[:])
            idr = constp.tile([P, P], F32R)
            nc.vector.tensor_copy(idr[:], idf[:])

            # broadcast w_scales to all partitions
            wsrow = constp.tile([1, 4 * depth], F32)
            nc.sync.dma_start(wsrow[:], wscl_d[:])
            wsb = constp.tile([P, 4 * depth], F32)
            nc.gpsimd.partition_broadcast(wsb[:], wsrow[0:1, :])

            z = residp.tile([P, NT, DIM], F32)
            v_aug = residp.tile([P, NT, HEADS, HD + 1], BF16)
            nc.vector.memset(v_aug[:, :, :, HD], 1.0)

            # ---------------- patch embed (fp32r matmuls) ----------------
            posb_sb = wp.tile([P, 2, DIM], F32, tag="w")
            nc.sync.dma_start(posb_sb[:], posb_d.rearrange("(a p) d -> p a d", p=P))
            xpT = fm6p.tile([P, KD, T], F32R, tag="fm6")
            nc.sync.dma_start(xpT[:], xpT_d.rearrange("(o p) t -> p o t", p=P))
            pw = wp.tile([P, KD, DIM], F32R, tag="w")
            nc.sync.dma_start(pw[:], patchWT_d.rearrange("(o p) d -> p o d", p=P))
            for t in range(NT):
                for (cs, ce) in _mm_chunks(DIM):
                    pt = ps_mm.tile([P, 512], F32, tag="mm", name="pmm")[:, : ce - cs]
                    for k in range(KD):
                        nc.tensor.matmul(pt[:], xpT[:, k, t * P:(t + 1) * P],
                                         pw[:, k, cs:ce], start=(k == 0), stop=(k == KD - 1))
                    nc.vector.tensor_tensor(z[:, t, cs:ce], pt[:], posb_sb[:, t % 2, cs:ce], OP.add)

            def load_w(dram, b, kchunks, width, dtype=BF16):
                half = kchunks // 2
                tiles = []
                for i in range(2):
                    wt = wp.tile([P, half, width], dtype, tag="w")
                    nc.sync.dma_start(
                        wt[:],
                        dram[b, i * half * P:(i + 1) * half * P, :].rearrange(
                            "(o p) f -> p o f", p=P))
                    tiles.append(wt)
                return tiles, half

            xqT = xqTp.tile([P, KH, T], BF16, tag="xqT")

            def amax_of(src_ap, dst_slice, eng=None):
                (eng or nc.vector).tensor_reduce(dst_slice, src_ap, axis=AX.X, op=OP.max,
                                                 apply_absolute_value=True)

            def scales_of(amax_pack, ws_idx, n):
                """[P,n] packed: s127 = 127/clip(amax,1e-5); c = clip*ws."""
                acs = scp.tile([P, 8], F32, tag="sc", name="acs")[:, :n]
                nc.vector.tensor_scalar_max(acs[:], amax_pack, 1e-5)
                rs = scp.tile([P, 8], F32, tag="sc", name="rs")[:, :n]
                nc.vector.reciprocal(rs[:], acs[:])
                s127 = scp.tile([P, 8], F32, tag="sc", name="s127")[:, :n]
                nc.vector.tensor_scalar_mul(s127[:], rs[:], 127.0)
                cs = scp.tile([P, 8], F32, tag="sc", name="cs")[:, :n]
                nc.vector.tensor_scalar(cs[:], acs[:], wsb[:, ws_idx:ws_idx + 1],
                                        None, OP.mult)
                return s127, cs

            def quant_data(src_ap, t, kchunks, s127_slice):
                """round(src*s127) -> bf16 ints, transposed into xqT cols t."""
                xq = xqp.tile([P, HID], BF16, tag="xq", name="xq")[:, :kchunks * P]
                for g0 in range(0, kchunks, 12):
                    gn = min(12, kchunks - g0)
                    tmp = qtp.tile([P, 12 * P], F32, tag="qt", name="qtmp")[:, :gn * P]
                    nc.gpsimd.tensor_scalar(tmp[:], src_ap[:, g0 * P:(g0 + gn) * P],
                                            s127_slice, MAGIC, OP.mult, OP.add)
                    nc.vector.tensor_scalar(xq[:, g0 * P:(g0 + gn) * P], tmp[:],
                                            MAGIC, None, OP.subtract)
                k0 = 0
                while k0 < kchunks:
                    g = min(8, kchunks - k0)
                    pack = ps_tp.tile([P, 8, P], BF16, tag="tp", name="tpack")
                    for j in range(g):
                        nc.tensor.transpose(pack[:, j, :], xq[:, (k0 + j) * P:(k0 + j + 1) * P],
                                            idb[:])
                    nc.vector.tensor_copy(xqT[:, k0:k0 + g, t * P:(t + 1) * P],
                                          pack[:, :g, :])
                    k0 += g

            def rstd_of(ssq_pack, n):
                """rstd = 1/sqrt(ssq/DIM + EPS) packed [P,n]."""
                ms = scp.tile([P, 8], F32, tag="sc", name="ms")[:, :n]
                nc.vector.tensor_scalar(ms[:], ssq_pack, 1.0 / DIM, EPS, OP.mult, OP.add)
                rr = scp.tile([P, 8], F32, tag="sc", name="rr")[:, :n]
                nc.vector.reciprocal(rr[:], ms[:])
                rstd = scp.tile([P, 8], F32, tag="sc", name="rstd")[:, :n]
                nc.scalar.activation(rstd[:], rr[:], AF.Sqrt)
                return rstd

            def norm_mod(t, mt, rstd_slice, dst):
                img = t // 2
                nc.vector.scalar_tensor_tensor(dst, z[:, t, :], rstd_slice,
                                               mt[:, img, 0, :], OP.mult, OP.mult)
                nc.vector.tensor_tensor(dst, dst, mt[:, img, 1, :], OP.add)

            def make_cB(crow, cs_pack, j0, n):
                """cs [P,n] (token-partitions) -> crow row segs j0.. (partition 0)."""
                cT = ps_tp.tile([P, NT, P], F32, tag="tp", name="cT")
                for j in range(n):
                    nc.tensor.transpose(cT[0:1, j, :], cs_pack[:, j:j + 1], idf[:])
                nc.vector.tensor_copy(crow[0:1, j0:j0 + n, :], cT[0:1, 0:n, :])

            def bcast_cB(cB, crow):
                for j in range(NT):
                    nc.gpsimd.partition_broadcast(cB[:, j * P:(j + 1) * P], crow[0:1, j, :])

            def ssq_of(src_ap, dst_slice):
                sq = tmp_.tile([P, DIM], F32, tag="tm", name="sqscratch")
                nc.scalar.activation(sq[:], src_ap, AF.Square, accum_out=dst_slice)

            # ---- prologue: norm1+quant of block 0 ----
            mt1 = modp.tile([P, 2, 2, DIM], F32, tag="mod", name="mt1")
            nc.sync.dma_start(mt1[:], mods_d[0, 0])
            ssq0 = scp.tile([P, 8], F32, tag="sc", name="ssq0")[:, :NT]
            for t in range(NT):
                ssq_of(z[:, t, :], ssq0[:, t:t + 1])
            rstd0 = rstd_of(ssq0[:], NT)
            h_nxt = gp.tile([P, NT, DIM], F32, tag="g", name="h_nxt")
            for t in range(NT):
                norm_mod(t, mt1, rstd0[:, t:t + 1], h_nxt[:, t, :])
            amax0 = scp.tile([P, 8], F32, tag="sc", name="amax0")[:, :NT]
            for t in range(NT):
                amax_of(h_nxt[:, t, :], amax0[:, t:t + 1])
            s127n, csn = scales_of(amax0[:], 0, NT)
            crow1 = cbp.tile([1, NT, P], F32, tag="crow", name="crow1")
            make_cB(crow1, csn[:], 0, NT)
            cB1 = cbp.tile([P, T], F32, tag="cb", name="cB1")
            bcast_cB(cB1, crow1)
            for t in range(NT):
                quant_data(h_nxt[:, t, :], t, KD, s127n[:, t:t + 1])
            csn_sl = [csn[:, t:t + 1] for t in range(NT)]
            wq_tiles, wq_half = load_w(wqkv_d, 0, KD, 3 * DIM)

            for b in range(depth):
                import contextlib
                def sc_(nm):
                    return nc.named_scope(f"b{b}_{nm}") if b == 5 else contextlib.nullcontext()
                mt2 = modp.tile([P, 2, 2, DIM], F32, tag="mod", name="mt2")
                nc.sync.dma_start(mt2[:], mods_d[b, 1])

                _p2 = sc_("p2qkv"); _p2.__enter__()
                # --- v (token layout) ---
                for t in range(NT):
                    for (cs0, ce0) in _mm_chunks(DIM):
                        pt = ps_mm.tile([P, 512], F32, tag="mm", name="pmm")[:, : ce0 - cs0]
                        for k in range(KD):
                            wt = wq_tiles[k // wq_half]
                            nc.tensor.matmul(pt[:], xqT[:, k, t * P:(t + 1) * P],
                                             wt[:, k % wq_half, 2 * DIM + cs0:2 * DIM + ce0],
                                             start=(k == 0), stop=(k == KD - 1))
                        h0 = cs0 // HD
                        h1 = ce0 // HD
                        nc.scalar.activation(
                            v_aug[:, t, h0:h1, 0:HD], pt[:],
                            AF.Identity, scale=csn_sl[t])
                # --- qT / kT (feature-partition layout, no transposes) ---
                q_fm = fm6p.tile([P, KD, T], F32R, tag="fm6", name="q_fm")
                k_fm = fm6p.tile([P, KD, T], F32R, tag="fm6", name="k_fm")
                for which, fm, coff in ((0, q_fm, 0), (1, k_fm, DIM)):
                    for qc in range(KD):
                        pt = ps_mm.tile([P, 512], F32, tag="mm", name="pmm")
                        for k in range(KD):
                            wt = wq_tiles[k // wq_half]
                            nc.tensor.matmul(pt[:], wt[:, k % wq_half, coff + qc * P:coff + (qc + 1) * P],
                                             xqT[:, k, :], start=(k == 0), stop=(k == KD - 1))
                        nc.vector.tensor_tensor(fm[:, qc, :], pt[:], cB1[:], OP.mult)
                _p2.__exit__(None, None, None)

                # --- attention: pipelined heads; o-quant, proj and norm2
                #     chains overlapped into the head pipeline ---
                _p3 = sc_("p3attn"); _p3.__enter__()
                wp_tiles, wp_half = load_w(wproj_d, b, KD, DIM)
                o_tm = gp.tile([P, NT, DIM], F32, tag="g", name="o_tm")
                h_all2 = gp.tile([P, NT, DIM], F32, tag="g", name="h_all2")
                amax_o01 = scp.tile([P, 8], F32, tag="sc", name="amaxo01")[:, :2]
                amax_o23 = scp.tile([P, 8], F32, tag="sc", name="amaxo23")[:, :2]
                ssq2 = scp.tile([P, 8], F32, tag="sc", name="ssq2")[:, :NT]
                amax2 = scp.tile([P, 8], F32, tag="sc", name="amax2")[:, :NT]
                s127o = [None, None]
                cpso = [None, None]

                state = [None] * (2 * HEADS)

                def lt_of(i):
                    img, hh = divmod(i, HEADS)
                    po = (hh % 2) * HD
                    ch = hh // 2
                    lt = ps_lt.tile([P, 2, NTOK], F32, tag="lt", name="lt")
                    for mt in range(2):
                        nc.tensor.matmul(
                            lt[:, mt, :],
                            k_fm[po:po + HD, ch, img * NTOK + mt * P: img * NTOK + (mt + 1) * P],
                            q_fm[po:po + HD, ch, img * NTOK: (img + 1) * NTOK],
                            start=True, stop=True)
                    state[i] = lt

                def exp_of(i):
                    lt = state[i]
                    eT = eTp.tile([P, 2, NTOK], BF16, tag="eT", name="eT")
                    nc.scalar.activation(eT[:], lt[:], AF.Exp, scale=0.125)
                    state[i] = eT

                def oa_of(i):
                    img, hh = divmod(i, HEADS)
                    eT = state[i]
                    for nt in range(2):
                        oa = ps_lt.tile([P, HD + 1], F32, tag="lt", name="oa")
                        for mt in range(2):
                            nc.tensor.matmul(
                                oa[:], eT[:, mt, nt * P:(nt + 1) * P],
                                v_aug[:, img * 2 + mt, hh, :],
                                start=(mt == 0), stop=(mt == 1))
                        rinv = scp.tile([P, 1], F32, tag="sc", name="rinv")
                        nc.vector.reciprocal(rinv[:], oa[:, HD:HD + 1])
                        nc.scalar.activation(
                            o_tm[:, img * 2 + nt, hh * HD:(hh + 1) * HD],
                            oa[:, 0:HD], AF.Identity, scale=rinv[:])
                    state[i] = None

                def oquant(t):
                    half = t // 2
                    quant_data(o_tm[:, t, :], t, KD, s127o[half][:, (t % 2):(t % 2) + 1])

                def proj_tile(t):
                    cps_sl = cpso[t // 2][:, (t % 2):(t % 2) + 1]
                    for (cs0, ce0) in _mm_chunks(DIM):
                        pt = ps_mm.tile([P, 512], F32, tag="mm", name="pmm")[:, : ce0 - cs0]
                        for k in range(KD):
                            wt = wp_tiles[k // wp_half]
                            nc.tensor.matmul(pt[:], xqT[:, k, t * P:(t + 1) * P],
                                             wt[:, k % wp_half, cs0:ce0],
                                             start=(k == 0), stop=(k == KD - 1))
                        tmp2 = tmp_.tile([P, DIM], F32, tag="tm", name="tmp2")[:, : ce0 - cs0]
                        nc.scalar.activation(tmp2[:], pt[:], AF.Identity, scale=cps_sl)
                        nc.vector.tensor_tensor(z[:, t, cs0:ce0], z[:, t, cs0:ce0], tmp2[:], OP.add)

                def n2chain(t):
                    ssq_of(z[:, t, :], ssq2[:, t:t + 1])
                    r = rstd_of(ssq2[:, t:t + 1], 1)
                    norm_mod(t, mt2, r[:, 0:1], h_all2[:, t, :])
                    amax_of(h_all2[:, t, :], amax2[:, t:t + 1])

                lt_of(0)
                for i in range(1, 2 * HEADS):
                    exp_of(i - 1)
                    lt_of(i)
                    oa_of(i - 1)
                    if i == 13:
                        amax_of(o_tm[:, 0, :], amax_o01[:, 0:1])
                        amax_of(o_tm[:, 1, :], amax_o01[:, 1:2])
                        s127o[0], cpso[0] = scales_of(amax_o01[:], 4 * b + 1, 2)
                        oquant(0)
                    if i == 15:
                        oquant(1)
                    if i == 17:
                        proj_tile(0)
                        n2chain(0)
                    if i == 20:
                        proj_tile(1)
                        n2chain(1)
                exp_of(2 * HEADS - 1)
                oa_of(2 * HEADS - 1)
                amax_of(o_tm[:, 2, :], amax_o23[:, 0:1])
                amax_of(o_tm[:, 3, :], amax_o23[:, 1:2])
                s127o[1], cpso[1] = scales_of(amax_o23[:], 4 * b + 1, 2)
                oquant(2)
                proj_tile(2)
                n2chain(2)
                oquant(3)
                proj_tile(3)
                s1272_01, c3_01 = scales_of(amax2[:, 0:2], 4 * b + 2, 2)
                quant_data(h_all2[:, 0, :], 0, KD, s1272_01[:, 0:1])
                quant_data(h_all2[:, 1, :], 1, KD, s1272_01[:, 1:2])
                n2chain(3)
                s1272_23, c3_23 = scales_of(amax2[:, 2:4], 4 * b + 2, 2)
                c3_sl = [c3_01[:, 0:1], c3_01[:, 1:2], c3_23[:, 0:1], c3_23[:, 1:2]]
                _p3.__exit__(None, None, None)

                # --- fc1/gelu + g-quant pipeline ---
                _p5 = sc_("p5fc1"); _p5.__enter__()
                wf1_tiles, wf1_half = load_w(wfc1_d, b, KD, HID)
                gs = [None] * NT
                c4g = [None] * NT

                def gquant(t):
                    amax_g = scp.tile([P, 1], F32, tag="sc", name="amaxg")
                    amax_of(gs[t][:], amax_g[:])
                    s127g, cg = scales_of(amax_g[:], 4 * b + 3, 1)
                    quant_data(gs[t][:], t, KH, s127g[:, 0:1])
                    c4g[t] = cg

                for t in range(NT):
                    if t == 0:
                        quant_data(h_all2[:, 2, :], 2, KD, s1272_23[:, 0:1])
                    if t == 1:
                        quant_data(h_all2[:, 3, :], 3, KD, s1272_23[:, 1:2])
                    g = gp.tile([P, HID], F32, tag="g")
                    gs[t] = g
                    for (cs0, ce0) in _mm_chunks(HID):
                        pt = ps_mm.tile([P, 512], F32, tag="mm", name="pmm")[:, : ce0 - cs0]
                        for k in range(KD):
                            wt = wf1_tiles[k // wf1_half]
                            nc.tensor.matmul(pt[:], xqT[:, k, t * P:(t + 1) * P],
                                             wt[:, k % wf1_half, cs0:ce0],
                                             start=(k == 0), stop=(k == KD - 1))
                        nc.scalar.activation(g[:, cs0:ce0], pt[:], AF.Gelu_apprx_tanh,
                                             scale=c3_sl[t])
                    if t > 0:
                        gquant(t - 1)
                gquant(NT - 1)
                _p5.__exit__(None, None, None)

                # --- fc2 + residual, fused with next block's norm1 ---
                _p6 = sc_("p6fc2"); _p6.__enter__()
                wf2_tiles, wf2_half = load_w(wfc2_d, b, KH, DIM)
                fuse = b + 1 < depth
                if fuse:
                    mt1_nxt = modp.tile([P, 2, 2, DIM], F32, tag="mod", name="mt1n")
                    nc.sync.dma_start(mt1_nxt[:], mods_d[b + 1, 0])
                    h_nxt = gp.tile([P, NT, DIM], F32, tag="g", name="h_nxt")
                    ssqn = scp.tile([P, 8], F32, tag="sc", name="ssqn")[:, :NT]
                    amaxn = scp.tile([P, 8], F32, tag="sc", name="amaxn")[:, :NT]
                    s127n_h = [None, None]
                    csn_h = [None, None]

                    def p1chain(t):
                        ssq_of(z[:, t, :], ssqn[:, t:t + 1])
                        r = rstd_of(ssqn[:, t:t + 1], 1)
                        norm_mod(t, mt1_nxt, r[:, 0:1], h_nxt[:, t, :])
                        amax_of(h_nxt[:, t, :], amaxn[:, t:t + 1])

                for t in range(NT):
                    for (cs0, ce0) in _mm_chunks(DIM):
                        pt = ps_mm.tile([P, 512], F32, tag="mm", name="pmm")[:, : ce0 - cs0]
                        for k in range(KH):
                            wt = wf2_tiles[k // wf2_half]
                            nc.tensor.matmul(pt[:], xqT[:, k, t * P:(t + 1) * P],
                                             wt[:, k % wf2_half, cs0:ce0],
                                             start=(k == 0), stop=(k == KH - 1))
                        tmp2 = tmp_.tile([P, DIM], F32, tag="tm", name="tmp2")[:, : ce0 - cs0]
                        nc.scalar.activation(tmp2[:], pt[:], AF.Identity, scale=c4g[t][:, 0:1])
                        nc.vector.tensor_tensor(z[:, t, cs0:ce0], z[:, t, cs0:ce0], tmp2[:], OP.add)
                    if fuse:
                        p1chain(t)
                        if t == 1:
                            s127n_h[0], csn_h[0] = scales_of(amaxn[:, 0:2], 4 * (b + 1), 2)
                        if t == 2:
                            quant_data(h_nxt[:, 0, :], 0, KD, s127n_h[0][:, 0:1])
                        if t == 3:
                            quant_data(h_nxt[:, 1, :], 1, KD, s127n_h[0][:, 1:2])
                if fuse:
                    s127n_h[1], csn_h[1] = scales_of(amaxn[:, 2:4], 4 * (b + 1), 2)
                    crow1 = cbp.tile([1, NT, P], F32, tag="crow", name="crow1")
                    make_cB(crow1, csn_h[0][:], 0, 2)
                    make_cB(crow1, csn_h[1][:], 2, 2)
                    cB1 = cbp.tile([P, T], F32, tag="cb", name="cB1")
                    bcast_cB(cB1, crow1)
                    quant_data(h_nxt[:, 2, :], 2, KD, s127n_h[1][:, 0:1])
                    quant_data(h_nxt[:, 3, :], 3, KD, s127n_h[1][:, 1:2])
                    csn_sl = [csn_h[0][:, 0:1], csn_h[0][:, 1:2],
                              csn_h[1][:, 0:1], csn_h[1][:, 1:2]]
                    wq_tiles, wq_half = load_w(wqkv_d, b + 1, KD, 3 * DIM)
                _p6.__exit__(None, None, None)

            # ---------------- final norm + head (fp32r) ----------------
            hw = wp.tile([P, KD, DIM], F32R, tag="w")
            nc.sync.dma_start(hw[:], headWT_d.rearrange("(o p) d -> p o d", p=P))
            hbrow = tmp_.tile([1, DIM], F32, tag="tm", name="hbrow")
            nc.sync.dma_start(hbrow[:], headb_d[:])
            hbb = gp.tile([P, DIM], F32, tag="g", name="hbb")
            nc.gpsimd.partition_broadcast(hbb[:], hbrow[0:1, :])
            ssqf = scp.tile([P, 8], F32, tag="sc", name="ssqf")[:, :NT]
            for t in range(NT):
                ssq_of(z[:, t, :], ssqf[:, t:t + 1])
            rstdf = rstd_of(ssqf[:], NT)
            for t in range(NT):
                zn = tmp_.tile([P, DIM], F32R, tag="tm")
                nc.vector.tensor_scalar_mul(zn[:], z[:, t, :], rstdf[:, t:t + 1])
                znT = tmp_.tile([P, DIM], F32R, tag="tm")
                for k in range(KD):
                    ptf = ps_tp.tile([P, P], F32R, tag="tp", name="ptf")
                    nc.tensor.transpose(ptf[:], zn[:, k * P:(k + 1) * P], idr[:])
                    nc.vector.tensor_copy(znT[:, k * P:(k + 1) * P], ptf[:])
                for (cs0, ce0) in _mm_chunks(DIM):
                    pt = ps_mm.tile([P, 512], F32, tag="mm", name="pmm")[:, : ce0 - cs0]
                    for k in range(KD):
                        nc.tensor.matmul(pt[:], znT[:, k * P:(k + 1) * P],
                                         hw[:, k, cs0:ce0], start=(k == 0), stop=(k == KD - 1))
                    ot = tmp_.tile([P, DIM], F32, tag="tm", name="ot")[:, : ce0 - cs0]
                    nc.vector.tensor_tensor(ot[:], pt[:], hbb[:, cs0:ce0], OP.add)
                    nc.sync.dma_start(out_d[t * P:(t + 1) * P, cs0:ce0], ot[:])

    nc.compile()
    return nc


# ---------------------------------------------------------------------------
# host-side numerics (numpy, fp32 — matches jax CPU within ~1e-7)

def _gelu_tanh(x):
    x = x.astype(np.float32)
    c = np.float32(math.sqrt(2.0 / math.pi))
    return np.float32(0.5) * x * (np.float32(1.0) +
                                  np.tanh(c * (x + np.float32(0.044715) * x * x * x)))


def _time_embedding(t, t_w1, t_b1, t_w2, t_b2):
    half = DIM // 2
    freqs = np.exp(-np.log(10000.0) * np.arange(half, dtype=np.float32) / (half - 1)).astype(np.float32)
    args = t[:, None].astype(np.float32) * freqs[None, :]
    emb = np.concatenate([np.sin(args), np.cos(args)], axis=-1).astype(np.float32)
    h = _gelu_tanh(emb @ t_w1.T + t_b1)
    return (h @ t_w2.T + t_b2).astype(np.float32)


def _quant_w(w):
    ws = np.float32(np.mean(np.abs(w), dtype=np.float64)) + np.float32(1e-5)
    wq = np.clip(np.round(w.astype(np.float32) / ws), -1.0, 1.0)
    return wq, ws


def _prepare(inputs):
    x = np.asarray(inputs["x"], np.float32)
    t = np.asarray(inputs["t"], np.float32)
    B = x.shape[0]
    n_cores = 8
    per = B // n_cores  # 2
    p = PATCH
    hh = IMG // p

    xp = x.reshape(B, CIN, hh, p, hh, p).transpose(0, 2, 4, 1, 3, 5).reshape(B, hh * hh, CIN * p * p)

    t_emb = _time_embedding(t, inputs["t_w1"], inputs["t_b1"], inputs["t_w2"], inputs["t_b2"])
    silu = (t_emb / (1.0 + np.exp(-t_emb))).astype(np.float32)

    depth = DEPTH
    mods = np.zeros((depth, 2, B, 2, DIM), np.float32)  # [blk, norm, img, A/B, D]
    wscl = np.zeros((4 * depth,), np.float32)
    wq_all, wp_all, wf1_all, wf2_all = [], [], [], []
    for b in range(depth):
        mod = silu @ np.asarray(inputs["blk_ada_w"][b], np.float32).T + np.asarray(
            inputs["blk_ada_b"][b], np.float32)
        sh1, sc1, sh2, sc2 = np.split(mod, 4, axis=-1)
        n1 = np.asarray(inputs["blk_norm1"][b], np.float32)
        n2 = np.asarray(inputs["blk_norm2"][b], np.float32)
        mods[b, 0, :, 0, :] = n1[None, :] * (1.0 + sc1)
        mods[b, 0, :, 1, :] = sh1
        mods[b, 1, :, 0, :] = n2[None, :] * (1.0 + sc2)
        mods[b, 1, :, 1, :] = sh2

        for j, (nm, lst) in enumerate([("blk_qkv", wq_all), ("blk_proj", wp_all),
                                       ("blk_fc1", wf1_all), ("blk_fc2", wf2_all)]):
            wq, ws = _quant_w(np.asarray(inputs[nm][b], np.float32))
            lst.append(np.ascontiguousarray(wq.T).astype(ml_dtypes.bfloat16))
            wscl[4 * b + j] = ws / np.float32(127.0)

    wqkv = np.stack(wq_all)
    wproj = np.stack(wp_all)
    wfc1 = np.stack(wf1_all)
    wfc2 = np.stack(wf2_all)

    posb = (np.asarray(inputs["pos_embed"][0], np.float32) +
            np.asarray(inputs["patch_b"], np.float32)[None, :]).astype(np.float32)
    patchWT = np.ascontiguousarray(np.asarray(inputs["patch_w"], np.float32).T)
    norm_w = np.asarray(inputs["norm_w"], np.float32)
    headWT = np.ascontiguousarray(np.asarray(inputs["head_w"], np.float32).T * norm_w[:, None])
    headb = np.asarray(inputs["head_b"], np.float32)[None, :]

    key = ("prog", depth)
    if key not in _CACHED:
        _CACHED[key] = build_program(depth)
    nc = _CACHED[key]

    in_maps = []
    for c in range(n_cores):
        imgs = slice(c * per, (c + 1) * per)
        xpT = np.ascontiguousarray(xp[imgs].reshape(per * hh * hh, CIN * p * p).T)
        in_maps.append(dict(
            xpT=xpT, posb=posb, patchWT=patchWT, headWT=headWT, headb=headb,
            wqkv=wqkv, wproj=wproj, wfc1=wfc1, wfc2=wfc2,
            mods=np.ascontiguousarray(
                np.broadcast_to(mods[:, :, None, imgs], (depth, 2, 128, per, 2, DIM))),
            wscl=wscl[None, :],
        ))

    return nc, in_maps


def _assemble(res, B=16, per=2):
    p = PATCH
    hh = IMG // p
    out = np.zeros((B, CIN, IMG, IMG), np.float32)
    for c in range(B // per):
        zo = res.results[c]["zout"]  # [512, 768]
        for i in range(per):
            zi = zo[i * 256:(i + 1) * 256]
            out[c * per + i] = zi.reshape(hh, hh, CIN, p, p).transpose(2, 0, 3, 1, 4).reshape(CIN, IMG, IMG)
    return out


def kernel(**inputs):
    nc, in_maps = _prepare(inputs)
    res = run_bass_kernel_spmd(nc, in_maps, list(range(len(in_maps))), trace=False)
    return _assemble(res)


# revision 37
# speedup vs baseline: 1.4353x; 1.0012x over previous
"""BitNet DiT on 8 Trainium2 NeuronCores — data-parallel over batch (2 images/core).

Host: patchify, time-embedding + adaLN modulation vectors, BitNet weight
quantization (ternary * per-tensor scale) -> bf16 upload.
Device: full 12-block DiT forward per core in a single Bass/Tile kernel.
BitNet matmuls run as exact integer arithmetic in bf16 (|values| <= 127,
fp32 accumulate). Attention runs in fp32r via transposed-logits + ones-column
softmax-denominator trick.

v2: qT/kT produced directly by weight-side matmuls (no activation
transposes for attention), pipelined attention heads, packed scalar
chains, Sqrt-based rstd (no act-table thrash), Pool-engine offload,
batched quantize-transpose packs, fp32r patch/head matmuls.
"""
import math
import os
import sys
import numpy as np

sys.path.insert(0, "/opt/trn_rl_repo")

import ml_dtypes  # noqa: E402
import concourse.bass as bass  # noqa: E402
import concourse.mybir as mybir  # noqa: E402
import concourse.tile as tile  # noqa: E402
from concourse import bacc  # noqa: E402
from concourse.bass_utils import run_bass_kernel_spmd  # noqa: E402
from concourse.masks import make_identity  # noqa: E402

F32 = mybir.dt.float32
F32R = mybir.dt.float32r
BF16 = mybir.dt.bfloat16
AX = mybir.AxisListType
OP = mybir.AluOpType
AF = mybir.ActivationFunctionType

DIM = 768
DEPTH = int(os.environ.get("KERNEL_DEPTH", "12"))
HEADS = 12
HD = 64
PATCH = 16
IMG = 256
CIN = 3
HID = 4 * DIM
EPS = 1e-6
P = 128
T = 512            # tokens per core (2 images x 256)
NT = T // P        # 4 token tiles
NTOK = 256         # tokens per image
KD = DIM // P      # 6
KH = HID // P      # 24
MAGIC = float(np.float32(3 * 2**22))  # 12582912.0 RNE round-to-int magic

_CACHED = {}


def _mm_chunks(n, c=512):
    out = []
    s = 0
    while s < n:
        e = min(s + c, n)
        out.append((s, e))
        s = e
    return out


def build_program(depth=DEPTH):
    nc = bacc.Bacc("TRN2", target_bir_lowering=False, debug=False, num_devices=8)

    xpT_d = nc.declare_dram_parameter("xpT", [DIM, T], F32R, isOutput=False)
    posb_d = nc.declare_dram_parameter("posb", [NTOK, DIM], F32, isOutput=False)
    patchWT_d = nc.declare_dram_parameter("patchWT", [DIM, DIM], F32R, isOutput=False)
    headWT_d = nc.declare_dram_parameter("headWT", [DIM, DIM], F32R, isOutput=False)
    headb_d = nc.declare_dram_parameter("headb", [1, DIM], F32, isOutput=False)
    wqkv_d = nc.declare_dram_parameter("wqkv", [depth, DIM, 3 * DIM], BF16, isOutput=False)
    wproj_d = nc.declare_dram_parameter("wproj", [depth, DIM, DIM], BF16, isOutput=False)
    wfc1_d = nc.declare_dram_parameter("wfc1", [depth, DIM, HID], BF16, isOutput=False)
    wfc2_d = nc.declare_dram_parameter("wfc2", [depth, HID, DIM], BF16, isOutput=False)
    # modulation vectors: [block, norm(2), img(2), A/B(2), 768] host-broadcast to 128 parts
    mods_d = nc.declare_dram_parameter("mods", [depth, 2, P, 2, 2, DIM], F32, isOutput=False)
    wscl_d = nc.declare_dram_parameter("wscl", [1, 4 * depth], F32, isOutput=False)
    out_d = nc.declare_dram_parameter("zout", [T, DIM], F32, isOutput=True)

    with tile.TileContext(nc) as tc:
        from contextlib import ExitStack
        with ExitStack() as _ctx:
            constp = _ctx.enter_context(tc.tile_pool(name="const", bufs=1))
            residp = _ctx.enter_context(tc.tile_pool(name="resid", bufs=1))
            fm6p = _ctx.enter_context(tc.tile_pool(name="fm6", bufs=2))
            xqTp = _ctx.enter_context(tc.tile_pool(name="xqT", bufs=1))
            wp = _ctx.enter_context(tc.tile_pool(name="w", bufs=3))
            modp = _ctx.enter_context(tc.tile_pool(name="mod", bufs=1))
            tmp_ = _ctx.enter_context(tc.tile_pool(name="tm", bufs=3))
            gp = _ctx.enter_context(tc.tile_pool(name="g", bufs=3))
            qtp = _ctx.enter_context(tc.tile_pool(name="qt", bufs=1))
            xqp = _ctx.enter_context(tc.tile_pool(name="xq", bufs=1))
            eTp = _ctx.enter_context(tc.tile_pool(name="eT", bufs=2))
            scp = _ctx.enter_context(tc.tile_pool(name="sc", bufs=48))
            cbp = _ctx.enter_context(tc.tile_pool(name="cb", bufs=1))
            ps_mm = _ctx.enter_context(tc.tile_pool(name="ps_mm", bufs=3, space="PSUM"))
            ps_tp = _ctx.enter_context(tc.tile_pool(name="ps_tp", bufs=2, space="PSUM"))
            ps_lt = _ctx.enter_context(tc.tile_pool(name="ps_lt", bufs=3, space="PSUM"))

            idf = constp.tile([P, P], F32)
            make_identity(nc, idf[:])
            idb = constp.tile([P, P], BF16)
            nc.vector.tensor_copy(idb[:], idf[:])
            idr = constp.tile([P, P], F32R)
            nc.vector.tensor_copy(idr[:], idf[:])

            # broadcast w_scales to all partitions
            wsrow = constp.tile([1, 4 * depth], F32)
            nc.sync.dma_start(wsrow[:], wscl_d[:])
            wsb = constp.tile([P, 4 * depth], F32)
            nc.gpsimd.partition_broadcast(wsb[:], wsrow[0:1, :])

            z = residp.tile([P, NT, DIM], F32)
            v_aug = residp.tile([P, NT, HEADS, HD + 1], BF16)
            nc.vector.memset(v_aug[:, :, :, HD], 1.0)

            # ---------------- patch embed (fp32r matmuls) ----------------
            posb_sb = wp.tile([P, 2, DIM], F32, tag="w")
            nc.sync.dma_start(posb_sb[:], posb_d.rearrange("(a p) d -> p a d", p=P))
            xpT = fm6p.tile([P, KD, T], F32R, tag="fm6")
            nc.sync.dma_start(xpT[:], xpT_d.rearrange("(o p) t -> p o t", p=P))
            pw = wp.tile([P, KD, DIM], F32R, tag="w")
            nc.sync.dma_start(pw[:], patchWT_d.rearrange("(o p) d -> p o d", p=P))
            for t in range(NT):
                for (cs, ce) in _mm_chunks(DIM):
                    pt = ps_mm.tile([P, 512], F32, tag="mm", name="pmm")[:, : ce - cs]
                    for k in range(KD):
                        nc.tensor.matmul(pt[:], xpT[:, k, t * P:(t + 1) * P],
                                         pw[:, k, cs:ce], start=(k == 0), stop=(k == KD - 1))
                    nc.vector.tensor_tensor(z[:, t, cs:ce], pt[:], posb_sb[:, t % 2, cs:ce], OP.add)

            def load_w(dram, b, kchunks, width, dtype=BF16):
                half = kchunks // 2
                tiles = []
                for i in range(2):
                    wt = wp.tile([P, half, width], dtype, tag="w")
                    nc.sync.dma_start(
                        wt[:],
                        dram[b, i * half * P:(i + 1) * half * P, :].rearrange(
                            "(o p) f -> p o f", p=P))
                    tiles.append(wt)
                return tiles, half

            xqT = xqTp.tile([P, KH, T], BF16, tag="xqT")

            def amax_of(src_ap, dst_slice, eng=None):
                (eng or nc.vector).tensor_reduce(dst_slice, src_ap, axis=AX.X, op=OP.max,
                                                 apply_absolute_value=True)

            def scales_of(amax_pack, ws_idx, n):
                """[P,n] packed: s127 = 127/clip(amax,1e-5); c = clip*ws."""
                acs = scp.tile([P, 8], F32, tag="sc", name="acs")[:, :n]
                nc.vector.tensor_scalar_max(acs[:], amax_pack, 1e-5)
                rs = scp.tile([P, 8], F32, tag="sc", name="rs")[:, :n]
                nc.vector.reciprocal(rs[:], acs[:])
                s127 = scp.tile([P, 8], F32, tag="sc", name="s127")[:, :n]
                nc.vector.tensor_scalar_mul(s127[:], rs[:], 127.0)
                cs = scp.tile([P, 8], F32, tag="sc", name="cs")[:, :n]
                nc.vector.tensor_scalar(cs[:], acs[:], wsb[:, ws_idx:ws_idx + 1],
                                        None, OP.mult)
                return s127, cs

            def quant_data(src_ap, t, kchunks, s127_slice):
                """round(src*s127) -> bf16 ints, transposed into xqT cols t."""
                xq = xqp.tile([P, HID], BF16, tag="xq", name="xq")[:, :kchunks * P]
                for g0 in range(0, kchunks, 12):
                    gn = min(12, kchunks - g0)
                    tmp = qtp.tile([P, 12 * P], F32, tag="qt", name="qtmp")[:, :gn * P]
                    nc.gpsimd.tensor_scalar(tmp[:], src_ap[:, g0 * P:(g0 + gn) * P],
                                            s127_slice, MAGIC, OP.mult, OP.add)
                    nc.vector.tensor_scalar(xq[:, g0 * P:(g0 + gn) * P], tmp[:],
                                            MAGIC, None, OP.subtract)
                k0 = 0
                while k0 < kchunks:
                    g = min(8, kchunks - k0)
                    pack = ps_tp.tile([P, 8, P], BF16, tag="tp", name="tpack")
                    for j in range(g):
                        nc.tensor.transpose(pack[:, j, :], xq[:, (k0 + j) * P:(k0 + j + 1) * P],
                                            idb[:])
                    nc.vector.tensor_copy(xqT[:, k0:k0 + g, t * P:(t + 1) * P],
                                          pack[:, :g, :])
                    k0 += g

            INT32 = mybir.dt.int32
            qmagic = constp.tile([P, 8], F32)
            nc.vector.memset(qmagic[:], float(np.frombuffer(
                np.uint32(0x5F3759DF).tobytes(), np.float32)[0]))

            def rstd_of(ssq_pack, n):
                """rstd = 1/sqrt(ssq/DIM + EPS) packed [P,n] (ACT Sqrt)."""
                ms = scp.tile([P, 8], F32, tag="sc", name="ms")[:, :n]
                nc.vector.tensor_scalar(ms[:], ssq_pack, 1.0 / DIM, EPS, OP.mult, OP.add)
                rr = scp.tile([P, 8], F32, tag="sc", name="rr")[:, :n]
                nc.vector.reciprocal(rr[:], ms[:])
                rstd = scp.tile([P, 8], F32, tag="sc", name="rstd")[:, :n]
                nc.scalar.activation(rstd[:], rr[:], AF.Sqrt)
                return rstd

            def rstd_quake(ssq_pack, n):
                """DVE-only rsqrt (quake seed + 2 Newton iters); avoids ACT
                table swaps when used inside the attention exp window."""
                ms = scp.tile([P, 8], F32, tag="sc", name="ms")[:, :n]
                nc.vector.tensor_scalar(ms[:], ssq_pack, 1.0 / DIM, EPS, OP.mult, OP.add)
                yi = scp.tile([P, 8], INT32, tag="sc", name="yi")[:, :n]
                nc.vector.tensor_scalar(yi[:], ms.bitcast(INT32), 1, None,
                                        OP.arith_shift_right)
                y0i = scp.tile([P, 8], INT32, tag="sc", name="y0i")[:, :n]
                nc.vector.tensor_tensor(y0i[:], qmagic.bitcast(INT32)[:, :n], yi[:],
                                        OP.subtract)
                y = y0i.bitcast(F32)
                for _ in range(2):
                    t1 = scp.tile([P, 8], F32, tag="sc", name="t1")[:, :n]
                    nc.vector.tensor_tensor(t1[:], y, y, OP.mult)
                    t2 = scp.tile([P, 8], F32, tag="sc", name="t2")[:, :n]
                    nc.vector.scalar_tensor_tensor(t2[:], t1[:], -0.5, ms[:],
                                                   OP.mult, OP.mult)
                    t3 = scp.tile([P, 8], F32, tag="sc", name="t3")[:, :n]
                    nc.vector.tensor_scalar(t3[:], t2[:], 1.5, None, OP.add)
                    yn = scp.tile([P, 8], F32, tag="sc", name="yn")[:, :n]
                    nc.vector.tensor_tensor(yn[:], y, t3[:], OP.mult)
                    y = yn[:]
                return y

            def norm_mod(t, mt, rstd_slice, dst):
                img = t // 2
                nc.vector.scalar_tensor_tensor(dst, z[:, t, :], rstd_slice,
                                               mt[:, img, 0, :], OP.mult, OP.mult)
                nc.vector.tensor_tensor(dst, dst, mt[:, img, 1, :], OP.add)

            def make_cB(crow, cs_pack, j0, n):
                """cs [P,n] (token-partitions) -> crow row segs j0.. (partition 0)."""
                cT = ps_tp.tile([P, NT, P], F32, tag="tp", name="cT")
                for j in range(n):
                    nc.tensor.transpose(cT[0:1, j, :], cs_pack[:, j:j + 1], idf[:])
                nc.vector.tensor_copy(crow[0:1, j0:j0 + n, :], cT[0:1, 0:n, :])

            def bcast_cB(cB, crow):
                for j in range(NT):
                    nc.gpsimd.partition_broadcast(cB[:, j * P:(j + 1) * P], crow[0:1, j, :])

            def ssq_of(src_ap, dst_slice):
                sq = tmp_.tile([P, DIM], F32, tag="tm", name="sqscratch")
                nc.scalar.activation(sq[:], src_ap, AF.Square, accum_out=dst_slice)

            # ---- prologue: norm1+quant of block 0 ----
            mt1 = modp.tile([P, 2, 2, DIM], F32, tag="mod", name="mt1")
            nc.sync.dma_start(mt1[:], mods_d[0, 0])
            ssq0 = scp.tile([P, 8], F32, tag="sc", name="ssq0")[:, :NT]
            for t in range(NT):
                ssq_of(z[:, t, :], ssq0[:, t:t + 1])
            rstd0 = rstd_of(ssq0[:], NT)
            h_nxt = gp.tile([P, NT, DIM], F32, tag="g", name="h_nxt")
            for t in range(NT):
                norm_mod(t, mt1, rstd0[:, t:t + 1], h_nxt[:, t, :])
            amax0 = scp.tile([P, 8], F32, tag="sc", name="amax0")[:, :NT]
            for t in range(NT):
                amax_of(h_nxt[:, t, :], amax0[:, t:t + 1])
            s127n, csn = scales_of(amax0[:], 0, NT)
            crow1 = cbp.tile([1, NT, P], F32, tag="crow", name="crow1")
            make_cB(crow1, csn[:], 0, NT)
            cB1 = cbp.tile([P, T], F32, tag="cb", name="cB1")
            bcast_cB(cB1, crow1)
            for t in range(NT):
                quant_data(h_nxt[:, t, :], t, KD, s127n[:, t:t + 1])
            csn_sl = [csn[:, t:t + 1] for t in range(NT)]
            wq_tiles, wq_half = load_w(wqkv_d, 0, KD, 3 * DIM)

            for b in range(depth):
                import contextlib
                def sc_(nm):
                    return nc.named_scope(f"b{b}_{nm}") if b == 5 else contextlib.nullcontext()
                mt2 = modp.tile([P, 2, 2, DIM], F32, tag="mod", name="mt2")
                nc.sync.dma_start(mt2[:], mods_d[b, 1])

                _p2 = sc_("p2qkv"); _p2.__enter__()
                # --- v (token layout) ---
                for t in range(NT):
                    for (cs0, ce0) in _mm_chunks(DIM):
                        pt = ps_mm.tile([P, 512], F32, tag="mm", name="pmm")[:, : ce0 - cs0]
                        for k in range(KD):
                            wt = wq_tiles[k // wq_half]
                            nc.tensor.matmul(pt[:], xqT[:, k, t * P:(t + 1) * P],
                                             wt[:, k % wq_half, 2 * DIM + cs0:2 * DIM + ce0],
                                             start=(k == 0), stop=(k == KD - 1))
                        h0 = cs0 // HD
                        h1 = ce0 // HD
                        nc.scalar.activation(
                            v_aug[:, t, h0:h1, 0:HD], pt[:],
                            AF.Identity, scale=csn_sl[t])
                # --- qT / kT (feature-partition layout, no transposes) ---
                q_fm = fm6p.tile([P, KD, T], F32R, tag="fm6", name="q_fm")
                k_fm = fm6p.tile([P, KD, T], F32R, tag="fm6", name="k_fm")
                for which, fm, coff in ((0, q_fm, 0), (1, k_fm, DIM)):
                    for qc in range(KD):
                        pt = ps_mm.tile([P, 512], F32, tag="mm", name="pmm")
                        for k in range(KD):
                            wt = wq_tiles[k // wq_half]
                            nc.tensor.matmul(pt[:], wt[:, k % wq_half, coff + qc * P:coff + (qc + 1) * P],
                                             xqT[:, k, :], start=(k == 0), stop=(k == KD - 1))
                        nc.vector.tensor_tensor(fm[:, qc, :], pt[:], cB1[:], OP.mult)
                _p2.__exit__(None, None, None)

                # --- attention: pipelined heads; o-quant, proj and norm2
                #     chains overlapped into the head pipeline ---
                _p3 = sc_("p3attn"); _p3.__enter__()
                wp_tiles, wp_half = load_w(wproj_d, b, KD, DIM)
                o_tm = gp.tile([P, NT, DIM], F32, tag="g", name="o_tm")
                h_all2 = gp.tile([P, NT, DIM], F32, tag="g", name="h_all2")
                amax_o01 = scp.tile([P, 8], F32, tag="sc", name="amaxo01")[:, :2]
                amax_o23 = scp.tile([P, 8], F32, tag="sc", name="amaxo23")[:, :2]
                ssq2 = scp.tile([P, 8], F32, tag="sc", name="ssq2")[:, :NT]
                amax2 = scp.tile([P, 8], F32, tag="sc", name="amax2")[:, :NT]
                s127o = [None, None]
                cpso = [None, None]

                state = [None] * (2 * HEADS)

                def lt_of(i):
                    img, hh = divmod(i, HEADS)
                    po = (hh % 2) * HD
                    ch = hh // 2
                    lt = ps_lt.tile([P, 2, NTOK], F32, tag="lt", name="lt")
                    for mt in range(2):
                        nc.tensor.matmul(
                            lt[:, mt, :],
                            k_fm[po:po + HD, ch, img * NTOK + mt * P: img * NTOK + (mt + 1) * P],
                            q_fm[po:po + HD, ch, img * NTOK: (img + 1) * NTOK],
                            start=True, stop=True)
                    state[i] = lt

                def exp_of(i):
                    lt = state[i]
                    eT = eTp.tile([P, 2, NTOK], BF16, tag="eT", name="eT")
                    nc.scalar.activation(eT[:], lt[:], AF.Exp, scale=0.125)
                    state[i] = eT

                def oa_of(i):
                    img, hh = divmod(i, HEADS)
                    eT = state[i]
                    for nt in range(2):
                        oa = ps_lt.tile([P, HD + 1], F32, tag="lt", name="oa")
                        for mt in range(2):
                            nc.tensor.matmul(
                                oa[:], eT[:, mt, nt * P:(nt + 1) * P],
                                v_aug[:, img * 2 + mt, hh, :],
                                start=(mt == 0), stop=(mt == 1))
                        rinv = scp.tile([P, 1], F32, tag="sc", name="rinv")
                        nc.vector.reciprocal(rinv[:], oa[:, HD:HD + 1])
                        nc.scalar.activation(
                            o_tm[:, img * 2 + nt, hh * HD:(hh + 1) * HD],
                            oa[:, 0:HD], AF.Identity, scale=rinv[:])
                    state[i] = None

                def oquant(t):
                    half = t // 2
                    quant_data(o_tm[:, t, :], t, KD, s127o[half][:, (t % 2):(t % 2) + 1])

                def proj_tile(t):
                    cps_sl = cpso[t // 2][:, (t % 2):(t % 2) + 1]
                    for (cs0, ce0) in _mm_chunks(DIM):
                        pt = ps_mm.tile([P, 512], F32, tag="mm", name="pmm")[:, : ce0 - cs0]
                        for k in range(KD):
                            wt = wp_tiles[k // wp_half]
                            nc.tensor.matmul(pt[:], xqT[:, k, t * P:(t + 1) * P],
                                             wt[:, k % wp_half, cs0:ce0],
                                             start=(k == 0), stop=(k == KD - 1))
                        tmp2 = tmp_.tile([P, DIM], F32, tag="tm", name="tmp2")[:, : ce0 - cs0]
                        nc.scalar.activation(tmp2[:], pt[:], AF.Identity, scale=cps_sl)
                        nc.vector.tensor_tensor(z[:, t, cs0:ce0], z[:, t, cs0:ce0], tmp2[:], OP.add)

                def n2chain(t):
                    ssq_of(z[:, t, :], ssq2[:, t:t + 1])
                    r = rstd_quake(ssq2[:, t:t + 1], 1)
                    norm_mod(t, mt2, r[:, 0:1], h_all2[:, t, :])
                    amax_of(h_all2[:, t, :], amax2[:, t:t + 1])

                lt_of(0)
                for i in range(1, 2 * HEADS):
                    exp_of(i - 1)
                    lt_of(i)
                    oa_of(i - 1)
                    if i == 13:
                        amax_of(o_tm[:, 0, :], amax_o01[:, 0:1])
                        amax_of(o_tm[:, 1, :], amax_o01[:, 1:2])
                        s127o[0], cpso[0] = scales_of(amax_o01[:], 4 * b + 1, 2)
                        oquant(0)
                    if i == 15:
                        oquant(1)
                    if i == 17:
                        proj_tile(0)
                        n2chain(0)
                    if i == 20:
                        proj_tile(1)
                        n2chain(1)
                exp_of(2 * HEADS - 1)
                oa_of(2 * HEADS - 1)
                amax_of(o_tm[:, 2, :], amax_o23[:, 0:1])
                amax_of(o_tm[:, 3, :], amax_o23[:, 1:2])
                s127o[1], cpso[1] = scales_of(amax_o23[:], 4 * b + 1, 2)
                oquant(2)
                proj_tile(2)
                n2chain(2)
                oquant(3)
                proj_tile(3)
                s1272_01, c3_01 = scales_of(amax2[:, 0:2], 4 * b + 2, 2)
                quant_data(h_all2[:, 0, :], 0, KD, s1272_01[:, 0:1])
                quant_data(h_all2[:, 1, :], 1, KD, s1272_01[:, 1:2])
                n2chain(3)
                s1272_23, c3_23 = scales_of(amax2[:, 2:4], 4 * b + 2, 2)
                c3_sl = [c3_01[:, 0:1], c3_01[:, 1:2], c3_23[:, 0:1], c3_23[:, 1:2]]
                _p3.__exit__(None, None, None)

                # --- fc1/gelu + g-quant pipeline ---
                _p5 = sc_("p5fc1"); _p5.__enter__()
                wf1_tiles, wf1_half = load_w(wfc1_d, b, KD, HID)
                gs = [None] * NT
                c4g = [None] * NT

                def gquant(t):
                    amax_g = scp.tile([P, 1], F32, tag="sc", name="amaxg")
                    amax_of(gs[t][:], amax_g[:])
                    s127g, cg = scales_of(amax_g[:], 4 * b + 3, 1)
                    quant_data(gs[t][:], t, KH, s127g[:, 0:1])
                    c4g[t] = cg

                for t in range(NT):
                    if t == 0:
                        quant_data(h_all2[:, 2, :], 2, KD, s1272_23[:, 0:1])
                    if t == 1:
                        quant_data(h_all2[:, 3, :], 3, KD, s1272_23[:, 1:2])
                    g = gp.tile([P, HID], F32, tag="g")
                    gs[t] = g
                    for (cs0, ce0) in _mm_chunks(HID):
                        pt = ps_mm.tile([P, 512], F32, tag="mm", name="pmm")[:, : ce0 - cs0]
                        for k in range(KD):
                            wt = wf1_tiles[k // wf1_half]
                            nc.tensor.matmul(pt[:], xqT[:, k, t * P:(t + 1) * P],
                                             wt[:, k % wf1_half, cs0:ce0],
                                             start=(k == 0), stop=(k == KD - 1))
                        nc.scalar.activation(g[:, cs0:ce0], pt[:], AF.Gelu_apprx_tanh,
                                             scale=c3_sl[t])
                    if t > 0:
                        gquant(t - 1)
                gquant(NT - 1)
                _p5.__exit__(None, None, None)

                # --- fc2 + residual, fused with next block's norm1 ---
                _p6 = sc_("p6fc2"); _p6.__enter__()
                wf2_tiles, wf2_half = load_w(wfc2_d, b, KH, DIM)
                fuse = b + 1 < depth
                if fuse:
                    mt1_nxt = modp.tile([P, 2, 2, DIM], F32, tag="mod", name="mt1n")
                    nc.sync.dma_start(mt1_nxt[:], mods_d[b + 1, 0])
                    h_nxt = gp.tile([P, NT, DIM], F32, tag="g", name="h_nxt")
                    ssqn = scp.tile([P, 8], F32, tag="sc", name="ssqn")[:, :NT]
                    amaxn = scp.tile([P, 8], F32, tag="sc", name="amaxn")[:, :NT]
                    s127n_h = [None, None]
                    csn_h = [None, None]

                    def p1chain(t):
                        ssq_of(z[:, t, :], ssqn[:, t:t + 1])
                        r = rstd_of(ssqn[:, t:t + 1], 1)
                        norm_mod(t, mt1_nxt, r[:, 0:1], h_nxt[:, t, :])
                        amax_of(h_nxt[:, t, :], amaxn[:, t:t + 1])

                for t in range(NT):
                    for (cs0, ce0) in _mm_chunks(DIM):
                        pt = ps_mm.tile([P, 512], F32, tag="mm", name="pmm")[:, : ce0 - cs0]
                        for k in range(KH):
                            wt = wf2_tiles[k // wf2_half]
                            nc.tensor.matmul(pt[:], xqT[:, k, t * P:(t + 1) * P],
                                             wt[:, k % wf2_half, cs0:ce0],
                                             start=(k == 0), stop=(k == KH - 1))
                        tmp2 = tmp_.tile([P, DIM], F32, tag="tm", name="tmp2")[:, : ce0 - cs0]
                        nc.scalar.activation(tmp2[:], pt[:], AF.Identity, scale=c4g[t][:, 0:1])
                        nc.vector.tensor_tensor(z[:, t, cs0:ce0], z[:, t, cs0:ce0], tmp2[:], OP.add)
                    if fuse:
                        p1chain(t)
                        if t == 1:
                            s127n_h[0], csn_h[0] = scales_of(amaxn[:, 0:2], 4 * (b + 1), 2)
                        if t == 2:
                            quant_data(h_nxt[:, 0, :], 0, KD, s127n_h[0][:, 0:1])
                        if t == 3:
                            quant_data(h_nxt[:, 1, :], 1, KD, s127n_h[0][:, 1:2])
                if fuse:
                    s127n_h[1], csn_h[1] = scales_of(amaxn[:, 2:4], 4 * (b + 1), 2)
                    crow1 = cbp.tile([1, NT, P], F32, tag="crow", name="crow1")
                    make_cB(crow1, csn_h[0][:], 0, 2)
                    make_cB(crow1, csn_h[1][:], 2, 2)
                    cB1 = cbp.tile([P, T], F32, tag="cb", name="cB1")
                    bcast_cB(cB1, crow1)
                    quant_data(h_nxt[:, 2, :], 2, KD, s127n_h[1][:, 0:1])
                    quant_data(h_nxt[:, 3, :], 3, KD, s127n_h[1][:, 1:2])
                    csn_sl = [csn_h[0][:, 0:1], csn_h[0][:, 1:2],
                              csn_h[1][:, 0:1], csn_h[1][:, 1:2]]
                    wq_tiles, wq_half = load_w(wqkv_d, b + 1, KD, 3 * DIM)
                _p6.__exit__(None, None, None)

            # ---------------- final norm + head (fp32r) ----------------
            hw = wp.tile([P, KD, DIM], F32R, tag="w")
            nc.sync.dma_start(hw[:], headWT_d.rearrange("(o p) d -> p o d", p=P))
            hbrow = tmp_.tile([1, DIM], F32, tag="tm", name="hbrow")
            nc.sync.dma_start(hbrow[:], headb_d[:])
            hbb = gp.tile([P, DIM], F32, tag="g", name="hbb")
            nc.gpsimd.partition_broadcast(hbb[:], hbrow[0:1, :])
            ssqf = scp.tile([P, 8], F32, tag="sc", name="ssqf")[:, :NT]
            for t in range(NT):
                ssq_of(z[:, t, :], ssqf[:, t:t + 1])
            rstdf = rstd_of(ssqf[:], NT)
            for t in range(NT):
                zn = tmp_.tile([P, DIM], F32R, tag="tm")
                nc.vector.tensor_scalar_mul(zn[:], z[:, t, :], rstdf[:, t:t + 1])
                znT = tmp_.tile([P, DIM], F32R, tag="tm")
                for k in range(KD):
                    ptf = ps_tp.tile([P, P], F32R, tag="tp", name="ptf")
                    nc.tensor.transpose(ptf[:], zn[:, k * P:(k + 1) * P], idr[:])
                    nc.vector.tensor_copy(znT[:, k * P:(k + 1) * P], ptf[:])
                for (cs0, ce0) in _mm_chunks(DIM):
                    pt = ps_mm.tile([P, 512], F32, tag="mm", name="pmm")[:, : ce0 - cs0]
                    for k in range(KD):
                        nc.tensor.matmul(pt[:], znT[:, k * P:(k + 1) * P],
                                         hw[:, k, cs0:ce0], start=(k == 0), stop=(k == KD - 1))
                    ot = tmp_.tile([P, DIM], F32, tag="tm", name="ot")[:, : ce0 - cs0]
                    nc.vector.tensor_tensor(ot[:], pt[:], hbb[:, cs0:ce0], OP.add)
                    nc.sync.dma_start(out_d[t * P:(t + 1) * P, cs0:ce0], ot[:])

    nc.compile()
    return nc


# ---------------------------------------------------------------------------
# host-side numerics (numpy, fp32 — matches jax CPU within ~1e-7)

def _gelu_tanh(x):
    x = x.astype(np.float32)
    c = np.float32(math.sqrt(2.0 / math.pi))
    return np.float32(0.5) * x * (np.float32(1.0) +
                                  np.tanh(c * (x + np.float32(0.044715) * x * x * x)))


def _time_embedding(t, t_w1, t_b1, t_w2, t_b2):
    half = DIM // 2
    freqs = np.exp(-np.log(10000.0) * np.arange(half, dtype=np.float32) / (half - 1)).astype(np.float32)
    args = t[:, None].astype(np.float32) * freqs[None, :]
    emb = np.concatenate([np.sin(args), np.cos(args)], axis=-1).astype(np.float32)
    h = _gelu_tanh(emb @ t_w1.T + t_b1)
    return (h @ t_w2.T + t_b2).astype(np.float32)


def _quant_w(w):
    ws = np.float32(np.mean(np.abs(w), dtype=np.float64)) + np.float32(1e-5)
    wq = np.clip(np.round(w.astype(np.float32) / ws), -1.0, 1.0)
    return wq, ws


def _prepare(inputs):
    x = np.asarray(inputs["x"], np.float32)
    t = np.asarray(inputs["t"], np.float32)
    B = x.shape[0]
    n_cores = 8
    per = B // n_cores  # 2
    p = PATCH
    hh = IMG // p

    xp = x.reshape(B, CIN, hh, p, hh, p).transpose(0, 2, 4, 1, 3, 5).reshape(B, hh * hh, CIN * p * p)

    t_emb = _time_embedding(t, inputs["t_w1"], inputs["t_b1"], inputs["t_w2"], inputs["t_b2"])
    silu = (t_emb / (1.0 + np.exp(-t_emb))).astype(np.float32)

    depth = DEPTH
    mods = np.zeros((depth, 2, B, 2, DIM), np.float32)  # [blk, norm, img, A/B, D]
    wscl = np.zeros((4 * depth,), np.float32)
    wq_all, wp_all, wf1_all, wf2_all = [], [], [], []
    for b in range(depth):
        mod = silu @ np.asarray(inputs["blk_ada_w"][b], np.float32).T + np.asarray(
            inputs["blk_ada_b"][b], np.float32)
        sh1, sc1, sh2, sc2 = np.split(mod, 4, axis=-1)
        n1 = np.asarray(inputs["blk_norm1"][b], np.float32)
        n2 = np.asarray(inputs["blk_norm2"][b], np.float32)
        mods[b, 0, :, 0, :] = n1[None, :] * (1.0 + sc1)
        mods[b, 0, :, 1, :] = sh1
        mods[b, 1, :, 0, :] = n2[None, :] * (1.0 + sc2)
        mods[b, 1, :, 1, :] = sh2

        for j, (nm, lst) in enumerate([("blk_qkv", wq_all), ("blk_proj", wp_all),
                                       ("blk_fc1", wf1_all), ("blk_fc2", wf2_all)]):
            wq, ws = _quant_w(np.asarray(inputs[nm][b], np.float32))
            lst.append(np.ascontiguousarray(wq.T).astype(ml_dtypes.bfloat16))
            wscl[4 * b + j] = ws / np.float32(127.0)

    wqkv = np.stack(wq_all)
    wproj = np.stack(wp_all)
    wfc1 = np.stack(wf1_all)
    wfc2 = np.stack(wf2_all)

    posb = (np.asarray(inputs["pos_embed"][0], np.float32) +
            np.asarray(inputs["patch_b"], np.float32)[None, :]).astype(np.float32)
    patchWT = np.ascontiguousarray(np.asarray(inputs["patch_w"], np.float32).T)
    norm_w = np.asarray(inputs["norm_w"], np.float32)
    headWT = np.ascontiguousarray(np.asarray(inputs["head_w"], np.float32).T * norm_w[:, None])
    headb = np.asarray(inputs["head_b"], np.float32)[None, :]

    key = ("prog", depth)
    if key not in _CACHED:
        _CACHED[key] = build_program(depth)
    nc = _CACHED[key]

    in_maps = []
    for c in range(n_cores):
        imgs = slice(c * per, (c + 1) * per)
        xpT = np.ascontiguousarray(xp[imgs].reshape(per * hh * hh, CIN * p * p).T)
        in_maps.append(dict(
            xpT=xpT, posb=posb, patchWT=patchWT, headWT=headWT, headb=headb,
            wqkv=wqkv, wproj=wproj, wfc1=wfc1, wfc2=wfc2,
            mods=np.ascontiguousarray(
                np.broadcast_to(mods[:, :, None, imgs], (depth, 2, 128, per, 2, DIM))),
            wscl=wscl[None, :],
        ))

    return nc, in_maps


def _assemble(res, B=16, per=2):
    p = PATCH
    hh = IMG // p
    out = np.zeros((B, CIN, IMG, IMG), np.float32)
    for c in range(B // per):
        zo = res.results[c]["zout"]  # [512, 768]
        for i in range(per):
            zi = zo[i * 256:(i + 1) * 256]
            out[c * per + i] = zi.reshape(hh, hh, CIN, p, p).transpose(2, 0, 3, 1, 4).reshape(CIN, IMG, IMG)
    return out


def kernel(**inputs):
    nc, in_maps = _prepare(inputs)
    res = run_bass_kernel_spmd(nc, in_maps, list(range(len(in_maps))), trace=False)
    return _assemble(res)
